# revision 17
# baseline (speedup 1.0000x reference)
"""GNN message-passing (3x GraphConv+BN+ReLU, final GraphConv) on 8 trn2 cores.

Source-sharded graph parallelism:
  - Nodes are partitioned across 8 cores (6272 slots each, 49 chunks of 128).
    Each core processes the edges whose SOURCE it owns, so per-edge feature
    gathers read a small local fp16 table (6272 rows, int16 indices).
  - Per layer: indirect-DMA gather of the core's edge source rows, one-hot
    matmuls accumulate partial aggregates for ALL 392 destination chunks in
    PSUM, partials stream to a DRAM buffer, and a ReduceScatter (split in
    three pieces, overlapped with the gather phase) reduces them onto the
    destination owner.  Dense transforms + BatchNorm stats/apply are local;
    only a tiny [128,2] stats AllGather crosses cores per layer.
  - One-hot masks are built in a [edge, dst, tile] layout so every DVE
    operand is packed 2-byte (2x DVE mode); tiles read them back with a
    strided matmul AP.
  - The GraphConv bias cancels inside BatchNorm and is skipped; every core
    holds exactly 6250 real nodes with its 22 pad slots pinned to the tail
    of chunk 48, so BN stats are exact and pads are re-zeroed by one memset.
  - Final layer: transpose-mode gather delivers gathT [feat, edge] tiles of
    h3, a per-tile PE projection through Wrel2 gives 2-col edge values, and
    the segment-sum emits feature-major partials; Wroot2+b2 ride along as
    per-rank masked matmuls so the final ReduceScatter yields the output.
"""

import hashlib
import heapq
import sys

import numpy as np

sys.path.insert(0, "/opt/trn_rl_repo")

import concourse.bass as bass  # noqa: E402
import concourse.mybir as mybir  # noqa: E402
import concourse.tile as tile  # noqa: E402
from concourse.vector_clock import ScopedClock  # noqa: E402
from concourse import library_config  # noqa: E402
from concourse.library_overlay import lower_extended_insts  # noqa: E402

N = 50000
E = 800000
D = 128
L = 3
OUT = 2
EPS = 1e-5
N_CORES = 8
P = 128
CHUNKS = 49                 # local dst chunks per core
SLOTS = CHUNKS * P          # 6272
NBINS = N_CORES * CHUNKS    # 392 global dst chunks
N_PAD = N_CORES * SLOTS     # 50176
QUOTA = N // N_CORES        # 6250 real nodes per core
SHORT = QUOTA - 48 * P      # 106 real slots in chunk 48
PIECES = (25, 12, 12)       # local chunks per RS piece
PIECE_J0 = (0, 25, 37)
GGRPS = ((13, 12), (12,), (12,))   # gather-group sizes per piece
PGRP = 4                    # chunks per PSUM bank / staging DMA group
RMAX = 36                   # max tiles covered by one sel build (12 chunks x T3)
AGRP = 13                   # chunks per BN-apply / table-write group

F16 = mybir.dt.float16
F32 = mybir.dt.float32

# ---------------------------------------------------------------------------
# walrus in this container accepts at most ONE semaphore wait per instruction.
# Patch the Tile exit drain and add a post-pass splitting multi-wait insts.
# ---------------------------------------------------------------------------
_MAX_WAITS = 1


def _drain_and_barrier(self, tick_clock, wait_clock):
    nc = self.nc
    drain_inst = nc.sync.drain()
    wait_clock.add_sem_waits(
        drain_inst.ins, ScopedClock({None: tick_clock.global_clock})
    )
    si = drain_inst.ins.sync_info
    if si is not None and si.on_wait is not None and len(si.on_wait) > _MAX_WAITS:
        waits = list(si.on_wait)
        si.on_wait = waits[:_MAX_WAITS]
        rest = waits[_MAX_WAITS:]
        for i in range(0, len(rest), _MAX_WAITS):
            nop = nc.sync.nop(nofuse=True)
            nop.ins.sync_info = mybir.SyncInfo(
                on_wait=rest[i : i + _MAX_WAITS], on_update=[]
            )
    nc.all_engine_barrier()
    assert self.sems is not None
    popped = nc._tile_sem_poison_stack.pop()
    assert popped is self._sem_poison
    nc.clear_and_free_semaphores(list(self.sems.allocated().values()))
    nc.all_engine_barrier()


tile.TileContext._drain_and_barrier = _drain_and_barrier


def _split_multiwait(nc):
    n_split = 0
    for fn in nc.m.functions:
        for blk in fn.blocks:
            out = []
            for inst in blk.instructions:
                si = inst.sync_info
                if si is not None and si.on_wait and len(si.on_wait) > _MAX_WAITS:
                    waits = list(si.on_wait)
                    si.on_wait = waits[-_MAX_WAITS:]
                    rest = waits[:-_MAX_WAITS]
                    for i in range(0, len(rest), _MAX_WAITS):
                        n_split += 1
                        out.append(
                            mybir.InstNoOp(
                                name=f"{inst.name}-ws{i}",
                                engine=inst.engine,
                                ins=[],
                                outs=[],
                                bass_nofuse=True,
                                sync_info=mybir.SyncInfo(
                                    on_wait=rest[i : i + _MAX_WAITS], on_update=[]
                                ),
                                debug=inst.debug,
                            )
                        )
                out.append(inst)
            blk.instructions[:] = out
    return n_split


# ---------------------------------------------------------------------------
# Host-side graph partitioning
# ---------------------------------------------------------------------------
def _lpt(nodes, deg_in, bins, caps, bin_of, slot_of, fill):
    heap = [(0, b) for b in bins]
    heapq.heapify(heap)
    for node in nodes:
        d = int(deg_in[node])
        ld, b = heapq.heappop(heap)
        bin_of[node] = b
        slot_of[node] = fill[b]
        fill[b] += 1
        if fill[b] < caps[b]:
            heapq.heappush(heap, (ld + d, b))


def _partition_nodes(deg_in):
    """Assign nodes to (bin, slot): bin b -> core b%8, local chunk b//8.
    Every bin is filled exactly to its cap (128, or 106 for chunk 48), so
    each core holds exactly 6250 real nodes and pads sit at the tail of
    chunk 48.  The heaviest nodes fill a set of "heavy" bins; the rest are
    LPT'd over "light" bins so per-(core,bin) edge counts pack tightly."""
    caps = np.full(NBINS, P, np.int64)
    caps[48 * N_CORES :] = SHORT                 # bins (q, j=48)
    order = np.argsort(-deg_in, kind="stable")
    sdeg = deg_in[order].astype(np.float64)
    pref = np.concatenate([[0.0], np.cumsum(sdeg)])
    total = pref[-1]
    best = (None, None)
    for nh in range(0, 200, 8):
        nl = NBINS - nh
        s_h = pref[min(nh * P, N)]
        m_h = s_h / max(nh, 1) / N_CORES
        m_l = (total - s_h) / nl / N_CORES
        t_h = int(np.ceil((m_h + 3.0 * np.sqrt(m_h * 0.875 + 1)) / P)) if nh else 0
        t_l = int(np.ceil((m_l + 3.0 * np.sqrt(m_l * 0.875 + 1)) / P))
        st = nh * max(t_h, 1) + nl * t_l
        if best[0] is None or st < best[0]:
            best = (st, nh)
    n_heavy = best[1]
    j_cut = CHUNKS - n_heavy // N_CORES
    allb = np.arange(NBINS)
    heavy_bins = allb[allb // N_CORES >= j_cut]
    light_bins = allb[allb // N_CORES < j_cut]

    bin_of = np.empty(N, np.int32)
    slot_of = np.empty(N, np.int32)
    fill = np.zeros(NBINS, np.int64)
    nh_nodes = int(caps[heavy_bins].sum())
    _lpt(order[:nh_nodes], deg_in, heavy_bins, caps, bin_of, slot_of, fill)
    _lpt(order[nh_nodes:], deg_in, light_bins, caps, bin_of, slot_of, fill)
    assert (fill == caps).all()
    return bin_of, slot_of


def _preprocess(x, edge_index):
    x = np.asarray(x, np.float32)
    ei = np.asarray(edge_index)
    src = ei[0].astype(np.int64)
    dst = ei[1].astype(np.int64)
    deg_in = np.bincount(dst, minlength=N)
    bin_of, slot_of = _partition_nodes(deg_in)

    core_of = bin_of % N_CORES
    newid = (
        core_of.astype(np.int64) * SLOTS
        + (bin_of // N_CORES).astype(np.int64) * P
        + slot_of
    )

    e_core = core_of[src]
    e_bin = bin_of[dst]

    cnt = np.zeros((N_CORES, NBINS), np.int64)
    np.add.at(cnt, (e_core, e_bin), 1)
    t_bin = np.maximum(1, -(-cnt.max(axis=0) // P))

    # chunk processing order: per piece, (q, j) with j in the piece range
    ordered_bins = []
    for piece in range(len(PIECES)):
        jr = range(PIECE_J0[piece], PIECE_J0[piece] + PIECES[piece])
        for q in range(N_CORES):
            for j in jr:
                ordered_bins.append(j * N_CORES + q)
    ordered_bins = np.array(ordered_bins)
    bin_pos = np.empty(NBINS, np.int64)
    bin_pos[ordered_bins] = np.arange(NBINS)
    t_proc = t_bin[ordered_bins]
    tile_base = np.concatenate([[0], np.cumsum(t_proc)[:-1]])
    SUM_T = int(t_proc.sum())

    e_pos = bin_pos[e_bin]
    order = np.lexsort((e_pos, e_core))
    s_core = e_core[order]
    s_pos = e_pos[order]
    s_srcslot = (newid[src[order]] % SLOTS).astype(np.int64)
    s_dstslot = slot_of[dst[order]].astype(np.int64)

    bucket = s_core * NBINS + s_pos
    bnd = np.concatenate(
        [[0], np.cumsum(np.bincount(bucket, minlength=N_CORES * NBINS))]
    )
    within = np.arange(E) - bnd[bucket]
    assert (within < t_proc[s_pos] * P).all()

    flat_off = tile_base * P
    e_slot = s_core * (SUM_T * P) + flat_off[s_pos] + within

    gidx = np.zeros(N_CORES * SUM_T * P, np.int16)
    dloc = np.full(N_CORES * SUM_T * P, -1.0, np.float16)
    gidx[e_slot] = s_srcslot.astype(np.int16)
    dloc[e_slot] = s_dstslot.astype(np.float16)
    gidx = gidx.reshape(N_CORES, SUM_T, P)
    dloc = dloc.reshape(N_CORES, SUM_T, P)

    dst_cores = np.ascontiguousarray(dloc.transpose(0, 2, 1))  # [c, 128, SUM_T]

    # gather groups per (piece, q): fixed chunk-count splits
    groups = []
    pos = 0
    for piece in range(len(PIECES)):
        for q in range(N_CORES):
            c0 = 0
            for gsz in GGRPS[piece]:
                lo = tile_base[pos + c0]
                last = pos + c0 + gsz - 1
                hi = tile_base[last] + t_proc[last]
                groups.append((int(lo), int(hi)))
                c0 += gsz
            assert c0 == PIECES[piece]
            pos += PIECES[piece]

    blocks = []
    for (lo, hi) in groups:
        n = (hi - lo) * P
        w = gidx[:, lo:hi, :].reshape(N_CORES, n // 16, 16).transpose(0, 2, 1)
        blocks.append(w)
    idxw = np.concatenate(blocks, axis=2)
    I_COLS = idxw.shape[2]
    idx_cores = np.ascontiguousarray(
        np.broadcast_to(idxw[:, None, :, :], (N_CORES, 8, 16, I_COLS)).reshape(
            N_CORES, P, I_COLS
        )
    )

    x_pad = np.zeros((N_PAD, D), np.float32)
    x_pad[newid] = x
    x_loc = np.ascontiguousarray(x_pad.reshape(N_CORES, SLOTS, D).astype(np.float16))
    xT_loc = np.ascontiguousarray(x_loc.transpose(0, 2, 1))
    meta = dict(
        SUM_T=SUM_T,
        t_proc=tuple(int(t) for t in t_proc),
        groups=tuple(groups),
        I_COLS=int(I_COLS),
    )
    return meta, newid, idx_cores, dst_cores, x_loc, xT_loc


# ---------------------------------------------------------------------------
# Device program
# ---------------------------------------------------------------------------
def build_program(meta):
    SUM_T = meta["SUM_T"]
    t_proc = meta["t_proc"]
    groups = meta["groups"]
    I_COLS = meta["I_COLS"]
    tile_base = [0]
    for t in t_proc[:-1]:
        tile_base.append(tile_base[-1] + t)

    nc = bass.Bass(num_devices=N_CORES)

    p_xloc = nc.declare_dram_parameter("x_loc", [SLOTS, D], F16, isOutput=False)
    p_xT = nc.declare_dram_parameter("xT_loc", [D, SLOTS], F16, isOutput=False)
    p_idx = nc.declare_dram_parameter("gidx", [P, I_COLS], mybir.dt.int16, isOutput=False)
    p_dst = nc.declare_dram_parameter("dst_loc", [P, SUM_T], F16, isOutput=False)
    p_wrel = nc.declare_dram_parameter("wrel", [L, D, D], F32, isOutput=False)
    p_wroot = nc.declare_dram_parameter("wroot", [L, D, D], F32, isOutput=False)
    p_wrel2 = nc.declare_dram_parameter("wrel2", [D, OUT], F32, isOutput=False)
    p_wroot2m = nc.declare_dram_parameter(
        "wroot2m", [D, N_CORES * OUT], F16, isOutput=False
    )
    p_b2m = nc.declare_dram_parameter("b2m", [1, N_CORES * OUT], F16, isOutput=False)
    p_gammaT = nc.declare_dram_parameter("gammaT", [D, L], F32, isOutput=False)
    p_betaT = nc.declare_dram_parameter("betaT", [D, L], F32, isOutput=False)
    p_iotar = nc.declare_dram_parameter("iotar", [P, P * RMAX], F16, isOutput=False)
    p_ident = nc.declare_dram_parameter("ident16", [P, P], F16, isOutput=False)
    p_out = nc.declare_dram_parameter("z4", [OUT, SLOTS], F16, isOutput=True)

    rg = [list(range(N_CORES))]
    n_pieces = len(PIECES)
    piece_cols = tuple(p * P for p in PIECES)
    piece_col0 = tuple(j * P for j in PIECE_J0)

    pos_info = []
    for piece in range(n_pieces):
        jr = range(PIECE_J0[piece], PIECE_J0[piece] + PIECES[piece])
        for q in range(N_CORES):
            for j in jr:
                pos_info.append((piece, q, j))
    piece_end_pos = {}
    acc = 0
    for piece in range(n_pieces):
        acc += PIECES[piece] * N_CORES
        piece_end_pos[acc - 1] = piece

    grp_start = {lo: (lo, hi) for (lo, hi) in groups}
    max_grp_t = max(hi - lo for (lo, hi) in groups)

    # same-T runs of chunks within each gather group, for packed sel builds
    pos_of_tb = {tile_base[pos]: pos for pos in range(NBINS)}
    group_runs = {}     # grp_lo -> list of (run_tb, nc_chunks, T)
    for (lo, hi) in groups:
        runs = []
        pos = pos_of_tb[lo]
        tb = lo
        while tb < hi:
            T = t_proc[pos]
            ncr = 0
            rtb = tb
            while tb < hi and t_proc[pos] == T:
                ncr += 1
                tb += T
                pos += 1
            runs.append((rtb, ncr, T))
        group_runs[lo] = runs

    from contextlib import ExitStack

    with tile.TileContext(nc) as tc:
        with ExitStack() as stack:
            ep = stack.enter_context
            dram_tab = ep(tc.tile_pool(name="dram_tab", bufs=2, space="DRAM"))
            dram_rsi = ep(tc.tile_pool(name="dram_rsi", bufs=2, space="DRAM"))
            dram_rso = ep(tc.tile_pool(name="dram_rso", bufs=2, space="DRAM"))
            dram_cc = ep(tc.tile_pool(name="dram_cc", bufs=2, space="DRAM"))
            singles = ep(tc.tile_pool(name="singles", bufs=1))
            hT_pool = ep(tc.tile_pool(name="hT", bufs=2))
            z_pool = ep(tc.tile_pool(name="zb", bufs=1))
            agg_pool = ep(tc.tile_pool(name="aggb", bufs=1))
            g_pool = ep(tc.tile_pool(name="gath", bufs=3))
            s_pool = ep(tc.tile_pool(name="sel", bufs=3))
            stg_pool = ep(tc.tile_pool(name="stg", bufs=3))
            t16_pool = ep(tc.tile_pool(name="t16p", bufs=2))
            bn_pool = ep(tc.tile_pool(name="bns", bufs=2))
            stat_pool = ep(tc.tile_pool(name="stat", bufs=2))
            psA = ep(tc.tile_pool(name="psA", bufs=2, space="PSUM"))
            psZ = ep(tc.tile_pool(name="psZ", bufs=2, space="PSUM"))
            psT = ep(tc.tile_pool(name="psT", bufs=2, space="PSUM"))
            psF = ep(tc.tile_pool(name="psF", bufs=1, space="PSUM"))
            psP = ep(tc.tile_pool(name="psP", bufs=1, space="PSUM"))

            with tc.high_priority():
                nc.gpsimd.load_library(library_config.mlp)

            grp_sizes = sorted({(hi - lo) * P for (lo, hi) in groups})
            nidx_regs = {n: nc.gpsimd.to_reg(n) for n in grp_sizes}

            # --- constants / weights in SBUF ---
            idx_sb = singles.tile([P, I_COLS], mybir.dt.int16)
            nc.sync.dma_start(out=idx_sb[:], in_=p_idx[:])
            dst_sb = singles.tile([P, SUM_T], F16)
            nc.sync.dma_start(out=dst_sb[:], in_=p_dst[:])
            iotar_sb = singles.tile([P, P * RMAX], F16)
            nc.sync.dma_start(out=iotar_sb[:], in_=p_iotar[:])
            ident_sb = singles.tile([P, P], F16)
            nc.sync.dma_start(out=ident_sb[:], in_=p_ident[:])
            wtmp = singles.tile([P, D], F32)
            wrel_sb = singles.tile([P, L * D], F16)
            wroot_sb = singles.tile([P, L * D], F16)
            for l in range(L):
                nc.sync.dma_start(out=wtmp[:], in_=p_wrel[l])
                nc.scalar.activation(
                    out=wrel_sb[:, l * D : (l + 1) * D], in_=wtmp[:],
                    func=mybir.ActivationFunctionType.Copy,
                )
                nc.sync.dma_start(out=wtmp[:], in_=p_wroot[l])
                nc.scalar.activation(
                    out=wroot_sb[:, l * D : (l + 1) * D], in_=wtmp[:],
                    func=mybir.ActivationFunctionType.Copy,
                )
            wrel2_sb = singles.tile([P, OUT], F16)
            nc.sync.dma_start(out=wtmp[:, 0:OUT], in_=p_wrel2[:])
            nc.scalar.activation(
                out=wrel2_sb[:], in_=wtmp[:, 0:OUT],
                func=mybir.ActivationFunctionType.Copy,
            )
            wroot2m_sb = singles.tile([P, N_CORES * OUT], F16)
            nc.sync.dma_start(out=wroot2m_sb[:], in_=p_wroot2m[:])
            b2m_sb = singles.tile([1, N_CORES * OUT], F16)
            nc.sync.dma_start(out=b2m_sb[:], in_=p_b2m[:])
            gammaT_sb = singles.tile([P, L], F32)
            nc.sync.dma_start(out=gammaT_sb[:], in_=p_gammaT[:])
            betaT_sb = singles.tile([P, L], F32)
            nc.sync.dma_start(out=betaT_sb[:], in_=p_betaT[:])
            ones_sb = singles.tile([1, P], F16)
            nc.vector.memset(ones_sb[:], 1.0)
            eps_sb = singles.tile([P, 1], F32)
            nc.vector.memset(eps_sb[:], EPS)

            hT_prev = hT_pool.tile([P, SLOTS], F16, tag="hT")
            nc.sync.dma_start(out=hT_prev[:], in_=p_xT[:])
            h_tab = p_xloc

            def build_sel(grp_lo, grp_hi):
                """One-hot masks for the group's tiles in [e, dst, tile]
                layout: all DVE operands packed 2-byte -> 2x mode."""
                st = s_pool.tile([P, max_grp_t * P], F16, tag="sel")
                sb = st[:]
                db = dst_sb[:]
                ib = iotar_sb[:]
                for (rtb, ncr, T) in group_runs[grp_lo]:
                    nct = ncr * T
                    base = (rtb - grp_lo) * P
                    nc.vector.tensor_tensor(
                        out=bass.AP(
                            tensor=sb.tensor, offset=sb.offset + base,
                            ap=[sb.ap[0], [nct, P], [1, nct]],
                        ),
                        in0=bass.AP(
                            tensor=db.tensor, offset=db.offset + rtb,
                            ap=[db.ap[0], [0, P], [1, nct]],
                        ),
                        in1=bass.AP(
                            tensor=ib.tensor, offset=ib.offset,
                            ap=[ib.ap[0], [RMAX, P], [1, nct]],
                        ),
                        op=mybir.AluOpType.is_equal,
                    )
                return st

            def sel_tile_ap(st, grp_lo, pos, t):
                """Matmul operand AP for (chunk at pos, tile t): [e, 128 dst]
                with dst stride = the run's nc*T."""
                for (rtb, ncr, T) in group_runs[grp_lo]:
                    if rtb <= tile_base[pos] < rtb + ncr * T:
                        nct = ncr * T
                        base = (rtb - grp_lo) * P
                        col = tile_base[pos] - rtb + t
                        sb = st[:]
                        return bass.AP(
                            tensor=sb.tensor, offset=sb.offset + base + col,
                            ap=[sb.ap[0], [nct, P]],
                        )
                raise AssertionError("tile not in any run")

            def emit_dense_piece(piece, l, z_sb, agg_sb, stats):
                w_rel = wrel_sb[:, l * D : (l + 1) * D]
                w_root = wroot_sb[:, l * D : (l + 1) * D]
                j0, npc = PIECE_J0[piece], PIECES[piece]
                for jg in range(j0, j0 + npc, PGRP):
                    jn = min(PGRP, j0 + npc - jg)
                    psz = psZ.tile([P, PGRP * P], F32, space="PSUM")
                    for k in range(jn):
                        cs = slice((jg + k) * P, (jg + k + 1) * P)
                        ks = slice(k * P, (k + 1) * P)
                        nc.tensor.matmul(
                            out=psz[:, ks], lhsT=w_rel, rhs=agg_sb[:, cs],
                            start=True, stop=False,
                        )
                        nc.tensor.matmul(
                            out=psz[:, ks], lhsT=w_root, rhs=hT_prev[:, cs],
                            start=False, stop=True,
                        )
                    zs = slice(jg * P, (jg + jn) * P)
                    nc.scalar.activation(
                        out=z_sb[:, zs], in_=psz[:, 0 : jn * P],
                        func=mybir.ActivationFunctionType.Copy,
                    )
                    for k in range(jn):
                        j = jg + k
                        width = SHORT if j == 48 else P
                        nc.vector.bn_stats(
                            out=stats[:, j, :],
                            in_=z_sb[:, j * P : j * P + width],
                        )

            for l in range(L):
                z_sb = z_pool.tile([P, SLOTS], F16)
                agg_sb = agg_pool.tile([P, SLOTS], F16)
                stats = stat_pool.tile([P, CHUNKS, nc.vector.BN_STATS_DIM], F32)
                rs_inp = []
                for piece in range(n_pieces):
                    rst = dram_rsi.tile(
                        [N_CORES * P, piece_cols[piece]], F16, tag=f"rsi{piece}"
                    )
                    rs_inp.append(rst)

                gath = sel = None
                grp_lo = 0
                ps4 = None
                dense_done = 0
                for pos in range(NBINS):
                    piece, q, j = pos_info[pos]
                    T = t_proc[pos]
                    tb = tile_base[pos]
                    if tb in grp_start:
                        grp_lo, grp_hi = grp_start[tb]
                        ng = (grp_hi - grp_lo) * P
                        gath = g_pool.tile([P, max_grp_t * P], F16, tag="gath")
                        gv = gath.rearrange("p (t d) -> p t d", t=max_grp_t)
                        nc.gpsimd.dma_gather(
                            out_ap=gv[:, 0 : grp_hi - grp_lo, :],
                            in_ap=h_tab[:],
                            idxs_ap=idx_sb[:, grp_lo * 8 : grp_hi * 8],
                            num_idxs=ng,
                            num_idxs_reg=nidx_regs[ng],
                            elem_size=D,
                            single_packet=False,
                        )
                        sel = build_sel(grp_lo, grp_hi)
                    jj = j - PIECE_J0[piece]
                    npc = PIECES[piece]
                    pg = jj % PGRP
                    pgn = min(PGRP, npc - (jj - pg))
                    if pg == 0:
                        ps4 = psA.tile([P, PGRP * P], F32, space="PSUM")
                    for t in range(T):
                        ft = tb + t - grp_lo
                        nc.tensor.matmul(
                            out=ps4[:, pg * P : (pg + 1) * P],
                            lhsT=gath[:, ft * P : (ft + 1) * P],
                            rhs=sel_tile_ap(sel, grp_lo, pos, t),
                            start=(t == 0),
                            stop=(t == T - 1),
                        )
                    if pg == pgn - 1:
                        stg = stg_pool.tile([P, PGRP * P], F16, tag="stg")
                        nc.scalar.activation(
                            out=stg[:, 0 : pgn * P], in_=ps4[:, 0 : pgn * P],
                            func=mybir.ActivationFunctionType.Copy,
                        )
                        rs_in = rs_inp[piece]
                        pw = piece_cols[piece]
                        col0 = (j - pg) * P - piece_col0[piece]
                        dest = bass.AP(
                            tensor=rs_in.tensor,
                            offset=rs_in[:].offset + q * P * pw + col0,
                            ap=[[pw, P], [1, pgn * P]],
                        )
                        nc.sync.dma_start(out=dest, in_=stg[:, 0 : pgn * P])
                    if pos in piece_end_pos:
                        piece_id = piece_end_pos[pos]
                        ncols = piece_cols[piece_id]
                        rs_out = dram_rso.tile([P, ncols], F16, tag=f"rso{piece_id}")
                        nc.gpsimd.collective_compute(
                            "ReduceScatter",
                            mybir.AluOpType.add,
                            replica_groups=rg,
                            ins=[rs_inp[piece_id][:].opt()],
                            outs=[rs_out[:]],
                        )
                        c0 = piece_col0[piece_id]
                        nc.sync.dma_start(
                            out=agg_sb[:, c0 : c0 + ncols], in_=rs_out[:]
                        )
                    # piece-0 dense work interleaves once piece 2 starts
                    if pos == (PIECES[0] + PIECES[1]) * N_CORES - 1:
                        emit_dense_piece(0, l, z_sb, agg_sb, stats)
                        dense_done = 1

                for piece in range(dense_done, n_pieces):
                    emit_dense_piece(piece, l, z_sb, agg_sb, stats)

                # ---- BatchNorm across all nodes (tiny stats AllGather) ----
                bs = bn_pool.tile([P, 16], F32)
                mv = bs[:, 0:2]
                with tc.high_priority():
                    nc.vector.bn_aggr(out=mv, in_=stats[:])
                cc_sb = bs[:, 3:5]
                with tc.high_priority():
                    nc.vector.tensor_copy(out=cc_sb[:, 0:1], in_=mv[:, 0:1])
                    nc.vector.tensor_scalar(
                        out=cc_sb[:, 1:2], in0=mv[:, 0:1], scalar1=mv[:, 0:1],
                        scalar2=mv[:, 1:2], op0=mybir.AluOpType.mult,
                        op1=mybir.AluOpType.add,
                    )
                cc_in = dram_cc.tile([P, 2], F32)
                cc_out = dram_cc.tile([P * N_CORES, 2], F32, addr_space="Shared")
                nc.sync.dma_start(out=cc_in[:], in_=cc_sb)
                nc.gpsimd.collective_compute(
                    "AllGather", mybir.AluOpType.bypass, replica_groups=rg,
                    ins=[cc_in.opt()], outs=[cc_out.opt()],
                )
                cc_all = bn_pool.tile([P, 2, N_CORES], F32)
                cc_src = bass.AP(
                    tensor=cc_out.tensor,
                    offset=cc_out[:].offset,
                    ap=[[2, P], [1, 2], [2 * P, N_CORES]],
                )
                nc.sync.dma_start(out=cc_all[:], in_=cc_src)
                cc_res = bs[:, 5:7]
                nc.vector.tensor_reduce(
                    out=cc_res.rearrange("p (a b) -> p a b", a=2),
                    in_=cc_all[:],
                    axis=mybir.AxisListType.X,
                    op=mybir.AluOpType.add,
                )
                mu = bs[:, 7:8]
                nc.vector.tensor_scalar(
                    out=mu, in0=cc_res[:, 0:1], scalar2=None,
                    op0=mybir.AluOpType.mult, scalar1=1.0 / N_CORES,
                )
                var = bs[:, 8:9]
                nc.vector.tensor_scalar(
                    out=var, in0=cc_res[:, 1:2], scalar2=None,
                    op0=mybir.AluOpType.mult, scalar1=1.0 / N_CORES,
                )
                mu2 = bs[:, 9:10]
                nc.vector.tensor_tensor(
                    out=mu2, in0=mu, in1=mu, op=mybir.AluOpType.mult
                )
                nc.vector.tensor_tensor(
                    out=var, in0=var, in1=mu2, op=mybir.AluOpType.subtract
                )
                rstd = bs[:, 10:11]
                nc.scalar.activation(
                    out=rstd, in_=var,
                    func=mybir.ActivationFunctionType.Sqrt,
                    bias=eps_sb[:], scale=1.0,
                )
                nc.vector.reciprocal(out=rstd, in_=rstd)
                scale = bs[:, 11:12]
                nc.vector.tensor_tensor(
                    out=scale, in0=rstd, in1=gammaT_sb[:, l : l + 1],
                    op=mybir.AluOpType.mult,
                )
                shift = bs[:, 12:13]
                nc.vector.tensor_tensor(
                    out=shift, in0=mu, in1=scale, op=mybir.AluOpType.mult
                )
                nc.vector.tensor_tensor(
                    out=shift, in0=betaT_sb[:, l : l + 1], in1=shift,
                    op=mybir.AluOpType.subtract,
                )

                # BN apply + relu, zero pad slots, rebuild node-major table
                hT_new = hT_pool.tile([P, SLOTS], F16, tag="hT")
                h_tab_new = dram_tab.tile([SLOTS, D], F16, tag="htab")
                for c0 in range(0, CHUNKS, AGRP):
                    ng = min(AGRP, CHUNKS - c0)
                    gs = slice(c0 * P, (c0 + ng) * P)
                    nc.scalar.activation(
                        out=hT_new[:, gs], in_=z_sb[:, gs],
                        func=mybir.ActivationFunctionType.Relu,
                        bias=shift, scale=scale,
                    )
                    if c0 + ng == CHUNKS:
                        nc.vector.memset(hT_new[:, QUOTA:SLOTS], 0.0)
                    t16g = t16_pool.tile([P, AGRP, P], F16)
                    for k in range(ng):
                        c = c0 + k
                        cs2 = slice(c * P, (c + 1) * P)
                        ps_t = psT.tile([P, P], F16, space="PSUM")
                        nc.tensor.transpose(
                            out=ps_t[:], in_=hT_new[:, cs2], identity=ident_sb[:]
                        )
                        if k % 2 == 0:
                            nc.vector.tensor_copy(out=t16g[:, k, :], in_=ps_t[:])
                        else:
                            nc.scalar.activation(
                                out=t16g[:, k, :], in_=ps_t[:],
                                func=mybir.ActivationFunctionType.Copy,
                            )
                    dest = bass.AP(
                        tensor=h_tab_new.tensor,
                        offset=h_tab_new[:].offset + c0 * P * D,
                        ap=[[D, P], [P * D, ng], [1, D]],
                    )
                    nc.sync.dma_start(out=dest, in_=t16g[:, 0:ng, :])
                hT_prev = hT_new
                h_tab = h_tab_new

            # ---------------- final GraphConv (OUT=2) ----------------
            # transpose-mode gather -> gathT [feat, edge]; PE projection
            # through Wrel2 -> proj [edge, 2]; segment-sum emits
            # feature-major partials [2, slots] incl. masked root + bias.
            rs_in_f = dram_rsi.tile([OUT * N_CORES, SLOTS], F16, tag="rsif")
            sel = None
            proj_sb = None
            grp_lo = 0
            psf = None
            for pos in range(NBINS):
                piece, q, j = pos_info[pos]
                T = t_proc[pos]
                tb = tile_base[pos]
                if tb in grp_start:
                    grp_lo, grp_hi = grp_start[tb]
                    ng = (grp_hi - grp_lo) * P
                    nt = grp_hi - grp_lo
                    gathT = g_pool.tile([P, max_grp_t * P], F16, tag="gath")
                    gtb = gathT[:]
                    nc.gpsimd.dma_gather(
                        out_ap=bass.AP(
                            tensor=gtb.tensor,
                            offset=gtb.offset,
                            ap=[gtb.ap[0], [ng, 1], [1, ng]],
                        ),
                        in_ap=h_tab[:],
                        idxs_ap=idx_sb[:, grp_lo * 8 : grp_hi * 8],
                        num_idxs=ng,
                        num_idxs_reg=nidx_regs[ng],
                        elem_size=D,
                        transpose=True,
                        single_packet=False,
                    )
                    psp = psP.tile([P, max_grp_t * OUT], F32, space="PSUM", tag="psp")
                    for t in range(nt):
                        nc.tensor.matmul(
                            out=psp[:, t * OUT : (t + 1) * OUT],
                            lhsT=gathT[:, t * P : (t + 1) * P],
                            rhs=wrel2_sb[:],
                            start=True, stop=True,
                        )
                    proj_sb = stg_pool.tile([P, max_grp_t * OUT], F16, tag="proj")
                    nc.scalar.activation(
                        out=proj_sb[:, 0 : nt * OUT], in_=psp[:, 0 : nt * OUT],
                        func=mybir.ActivationFunctionType.Copy,
                    )
                    sel = build_sel(grp_lo, grp_hi)
                jj = j - PIECE_J0[piece]
                npc = PIECES[piece]
                pg = jj % PGRP
                pgn = min(PGRP, npc - (jj - pg))
                if pg == 0:
                    psf = psF.tile([OUT, PGRP * P], F32, space="PSUM", tag="psf")
                fo = slice(pg * P, (pg + 1) * P)
                for t in range(T):
                    nc.tensor.matmul(
                        out=psf[:, fo],
                        lhsT=proj_sb[:, (tb + t - grp_lo) * OUT : (tb + t - grp_lo + 1) * OUT],
                        rhs=sel_tile_ap(sel, grp_lo, pos, t),
                        start=(t == 0),
                        stop=False,
                    )
                # root + bias: nonzero only on the rank that owns these slots
                nc.tensor.matmul(
                    out=psf[:, fo],
                    lhsT=wroot2m_sb[:, q * OUT : (q + 1) * OUT],
                    rhs=hT_prev[:, j * P : (j + 1) * P],
                    start=False,
                    stop=False,
                )
                nc.tensor.matmul(
                    out=psf[:, fo],
                    lhsT=b2m_sb[:, q * OUT : (q + 1) * OUT],
                    rhs=ones_sb[:],
                    start=False,
                    stop=True,
                )
                if pg == pgn - 1:
                    stgf = stg_pool.tile([OUT, PGRP * P], F16, tag="stgf")
                    nc.scalar.activation(
                        out=stgf[:, 0 : pgn * P], in_=psf[:, 0 : pgn * P],
                        func=mybir.ActivationFunctionType.Copy,
                    )
                    col0 = (j - pg) * P
                    dest = bass.AP(
                        tensor=rs_in_f.tensor,
                        offset=rs_in_f[:].offset + q * OUT * SLOTS + col0,
                        ap=[[SLOTS, OUT], [1, pgn * P]],
                    )
                    nc.sync.dma_start(out=dest, in_=stgf[:, 0 : pgn * P])

            rs_out_f = dram_rso.tile([OUT, SLOTS], F16, tag="rsof")
            nc.gpsimd.collective_compute(
                "ReduceScatter",
                mybir.AluOpType.add,
                replica_groups=rg,
                ins=[rs_in_f[:].opt()],
                outs=[rs_out_f[:]],
            )
            nc.sync.dma_start(out=p_out[:], in_=rs_out_f[:])

    lower_extended_insts(nc)
    _split_multiwait(nc)
    return nc


_PROGRAM_CACHE = {}


def _get_program(meta):
    key = hashlib.sha1(repr(sorted(meta.items())).encode()).hexdigest()
    if key not in _PROGRAM_CACHE:
        _PROGRAM_CACHE[key] = build_program(meta)
    return _PROGRAM_CACHE[key]


def _make_in_maps(idx_cores, dst_cores, x_loc, xT_loc,
                  Wrel, Wroot, gamma, beta, Wrel2, Wroot2, b2):
    iotar = np.zeros((P, P, RMAX), np.float16)
    iotar[:, :, :] = np.arange(P, dtype=np.float16)[None, :, None]
    ident16 = np.eye(P, dtype=np.float16)
    common = dict(
        wrel=np.ascontiguousarray(np.asarray(Wrel, np.float32)),
        wroot=np.ascontiguousarray(np.asarray(Wroot, np.float32)),
        wrel2=np.ascontiguousarray(np.asarray(Wrel2, np.float32)),
        gammaT=np.ascontiguousarray(np.asarray(gamma, np.float32).T),
        betaT=np.ascontiguousarray(np.asarray(beta, np.float32).T),
        iotar=np.ascontiguousarray(iotar.reshape(P, P * RMAX)),
        ident16=ident16,
    )
    wroot2 = np.asarray(Wroot2, np.float16)                 # [D, OUT]
    b2 = np.asarray(b2, np.float16).reshape(1, OUT)
    in_maps = []
    for c in range(N_CORES):
        w2m = np.zeros((D, N_CORES, OUT), np.float16)
        w2m[:, c, :] = wroot2
        b2m = np.zeros((1, N_CORES, OUT), np.float16)
        b2m[:, c, :] = b2
        m = dict(common)
        m["x_loc"] = x_loc[c]
        m["xT_loc"] = xT_loc[c]
        m["gidx"] = idx_cores[c]
        m["dst_loc"] = dst_cores[c]
        m["wroot2m"] = np.ascontiguousarray(w2m.reshape(D, N_CORES * OUT))
        m["b2m"] = np.ascontiguousarray(b2m.reshape(1, N_CORES * OUT))
        in_maps.append(m)
    return in_maps


def run(x, edge_index, Wrel, Wroot, b, gamma, beta, Wrel2, Wroot2, b2):
    """Returns (output [N, OUT] float32, nc, meta) - nc exposed for profiling.
    The per-layer GraphConv bias b cancels inside BatchNorm and is unused."""
    meta, newid, idx_cores, dst_cores, x_loc, xT_loc = _preprocess(x, edge_index)
    nc = _get_program(meta)
    in_maps = _make_in_maps(
        idx_cores, dst_cores, x_loc, xT_loc,
        Wrel, Wroot, gamma, beta, Wrel2, Wroot2, b2,
    )
    from concourse.bass_utils import run_bass_kernel_spmd

    res = run_bass_kernel_spmd(nc, in_maps, list(range(N_CORES)))
    full = np.concatenate(
        [res.results[c]["z4"].T for c in range(N_CORES)], axis=0
    )  # [N_PAD, OUT]
    return full[newid].astype(np.float32), nc, meta


def kernel(**inputs):
    out, _, _ = run(**{k: np.asarray(v) for k, v in inputs.items()})
    return out


# revision 18
# speedup vs baseline: 1.0706x; 1.0706x over previous
"""GNN message-passing (3x GraphConv+BN+ReLU, final GraphConv) on 8 trn2 cores.

Source-sharded graph parallelism:
  - Nodes are partitioned across 8 cores (6272 slots each, 49 chunks of 128).
    Each core processes the edges whose SOURCE it owns, so per-edge feature
    gathers read a small local fp16 table (6272 rows, int16 indices).
  - Per layer: indirect-DMA gather of the core's edge source rows, one-hot
    matmuls accumulate partial aggregates for ALL 392 destination chunks in
    PSUM, partials stream to a DRAM buffer, and a ReduceScatter (split in
    three pieces, overlapped with the gather phase) reduces them onto the
    destination owner.  Dense transforms + BatchNorm stats/apply are local;
    only a tiny [128,2] stats AllGather crosses cores per layer.
  - One-hot masks are built in a [edge, dst, tile] layout so every DVE
    operand is packed 2-byte (2x DVE mode); tiles read them back with a
    strided matmul AP.
  - The GraphConv bias cancels inside BatchNorm and is skipped; every core
    holds exactly 6250 real nodes with its 22 pad slots pinned to the tail
    of chunk 48, so BN stats are exact and pads are re-zeroed by one memset.
  - Final layer: transpose-mode gather delivers gathT [feat, edge] tiles of
    h3, a per-tile PE projection through Wrel2 gives 2-col edge values, and
    the segment-sum emits feature-major partials; Wroot2+b2 ride along as
    per-rank masked matmuls so the final ReduceScatter yields the output.
"""

import hashlib
import heapq
import sys

import numpy as np

sys.path.insert(0, "/opt/trn_rl_repo")

import concourse.bass as bass  # noqa: E402
import concourse.mybir as mybir  # noqa: E402
import concourse.tile as tile  # noqa: E402
from concourse.vector_clock import ScopedClock  # noqa: E402
from concourse import library_config  # noqa: E402
from concourse.library_overlay import lower_extended_insts  # noqa: E402

N = 50000
E = 800000
D = 128
L = 3
OUT = 2
EPS = 1e-5
N_CORES = 8
P = 128
CHUNKS = 49                 # local dst chunks per core
SLOTS = CHUNKS * P          # 6272
NBINS = N_CORES * CHUNKS    # 392 global dst chunks
N_PAD = N_CORES * SLOTS     # 50176
QUOTA = N // N_CORES        # 6250 real nodes per core
SHORT = QUOTA - 48 * P      # 106 real slots in chunk 48
PIECES = (25, 16, 8)        # local chunks per RS piece
PIECE_J0 = (0, 25, 41)
GGRPS = ((13, 12), (8, 8), (8,))   # gather-group sizes per piece
PGRP = 4                    # chunks per PSUM bank / staging DMA group
RMAX = 36                   # max tiles covered by one sel build (12 chunks x T3)
AGRP = 25                   # chunks per BN-apply / table-write group

F16 = mybir.dt.float16
F32 = mybir.dt.float32

# ---------------------------------------------------------------------------
# walrus in this container accepts at most ONE semaphore wait per instruction.
# Patch the Tile exit drain and add a post-pass splitting multi-wait insts.
# ---------------------------------------------------------------------------
_MAX_WAITS = 1


def _drain_and_barrier(self, tick_clock, wait_clock):
    nc = self.nc
    drain_inst = nc.sync.drain()
    wait_clock.add_sem_waits(
        drain_inst.ins, ScopedClock({None: tick_clock.global_clock})
    )
    si = drain_inst.ins.sync_info
    if si is not None and si.on_wait is not None and len(si.on_wait) > _MAX_WAITS:
        waits = list(si.on_wait)
        si.on_wait = waits[:_MAX_WAITS]
        rest = waits[_MAX_WAITS:]
        for i in range(0, len(rest), _MAX_WAITS):
            nop = nc.sync.nop(nofuse=True)
            nop.ins.sync_info = mybir.SyncInfo(
                on_wait=rest[i : i + _MAX_WAITS], on_update=[]
            )
    nc.all_engine_barrier()
    assert self.sems is not None
    popped = nc._tile_sem_poison_stack.pop()
    assert popped is self._sem_poison
    nc.clear_and_free_semaphores(list(self.sems.allocated().values()))
    nc.all_engine_barrier()


tile.TileContext._drain_and_barrier = _drain_and_barrier


def _split_multiwait(nc):
    n_split = 0
    for fn in nc.m.functions:
        for blk in fn.blocks:
            out = []
            for inst in blk.instructions:
                si = inst.sync_info
                if si is not None and si.on_wait and len(si.on_wait) > _MAX_WAITS:
                    waits = list(si.on_wait)
                    si.on_wait = waits[-_MAX_WAITS:]
                    rest = waits[:-_MAX_WAITS]
                    for i in range(0, len(rest), _MAX_WAITS):
                        n_split += 1
                        out.append(
                            mybir.InstNoOp(
                                name=f"{inst.name}-ws{i}",
                                engine=inst.engine,
                                ins=[],
                                outs=[],
                                bass_nofuse=True,
                                sync_info=mybir.SyncInfo(
                                    on_wait=rest[i : i + _MAX_WAITS], on_update=[]
                                ),
                                debug=inst.debug,
                            )
                        )
                out.append(inst)
            blk.instructions[:] = out
    return n_split


# ---------------------------------------------------------------------------
# Host-side graph partitioning
# ---------------------------------------------------------------------------
def _lpt(nodes, deg_in, bins, caps, bin_of, slot_of, fill):
    heap = [(0, b) for b in bins]
    heapq.heapify(heap)
    for node in nodes:
        d = int(deg_in[node])
        ld, b = heapq.heappop(heap)
        bin_of[node] = b
        slot_of[node] = fill[b]
        fill[b] += 1
        if fill[b] < caps[b]:
            heapq.heappush(heap, (ld + d, b))


def _partition_nodes(deg_in):
    """Assign nodes to (bin, slot): bin b -> core b%8, local chunk b//8.
    Every bin is filled exactly to its cap (128, or 106 for chunk 48), so
    each core holds exactly 6250 real nodes and pads sit at the tail of
    chunk 48.  The heaviest nodes fill a set of "heavy" bins; the rest are
    LPT'd over "light" bins so per-(core,bin) edge counts pack tightly."""
    caps = np.full(NBINS, P, np.int64)
    caps[48 * N_CORES :] = SHORT                 # bins (q, j=48)
    order = np.argsort(-deg_in, kind="stable")
    sdeg = deg_in[order].astype(np.float64)
    pref = np.concatenate([[0.0], np.cumsum(sdeg)])
    total = pref[-1]
    best = (None, None)
    for nh in range(0, 200, 8):
        nl = NBINS - nh
        s_h = pref[min(nh * P, N)]
        m_h = s_h / max(nh, 1) / N_CORES
        m_l = (total - s_h) / nl / N_CORES
        t_h = int(np.ceil((m_h + 3.0 * np.sqrt(m_h * 0.875 + 1)) / P)) if nh else 0
        t_l = int(np.ceil((m_l + 3.0 * np.sqrt(m_l * 0.875 + 1)) / P))
        st = nh * max(t_h, 1) + nl * t_l
        if best[0] is None or st < best[0]:
            best = (st, nh)
    n_heavy = best[1]
    j_cut = CHUNKS - n_heavy // N_CORES
    allb = np.arange(NBINS)
    heavy_bins = allb[allb // N_CORES >= j_cut]
    light_bins = allb[allb // N_CORES < j_cut]

    bin_of = np.empty(N, np.int32)
    slot_of = np.empty(N, np.int32)
    fill = np.zeros(NBINS, np.int64)
    nh_nodes = int(caps[heavy_bins].sum())
    _lpt(order[:nh_nodes], deg_in, heavy_bins, caps, bin_of, slot_of, fill)
    _lpt(order[nh_nodes:], deg_in, light_bins, caps, bin_of, slot_of, fill)
    assert (fill == caps).all()
    return bin_of, slot_of


def _preprocess(x, edge_index):
    x = np.asarray(x, np.float32)
    ei = np.asarray(edge_index)
    src = ei[0].astype(np.int64)
    dst = ei[1].astype(np.int64)
    deg_in = np.bincount(dst, minlength=N)
    bin_of, slot_of = _partition_nodes(deg_in)

    core_of = bin_of % N_CORES
    newid = (
        core_of.astype(np.int64) * SLOTS
        + (bin_of // N_CORES).astype(np.int64) * P
        + slot_of
    )

    e_core = core_of[src]
    e_bin = bin_of[dst]

    cnt = np.zeros((N_CORES, NBINS), np.int64)
    np.add.at(cnt, (e_core, e_bin), 1)
    t_bin = np.maximum(1, -(-cnt.max(axis=0) // P))

    # chunk processing order: per piece, (q, j) with j in the piece range
    ordered_bins = []
    for piece in range(len(PIECES)):
        jr = range(PIECE_J0[piece], PIECE_J0[piece] + PIECES[piece])
        for q in range(N_CORES):
            for j in jr:
                ordered_bins.append(j * N_CORES + q)
    ordered_bins = np.array(ordered_bins)
    bin_pos = np.empty(NBINS, np.int64)
    bin_pos[ordered_bins] = np.arange(NBINS)
    t_proc = t_bin[ordered_bins]
    tile_base = np.concatenate([[0], np.cumsum(t_proc)[:-1]])
    SUM_T = int(t_proc.sum())

    e_pos = bin_pos[e_bin]
    order = np.lexsort((e_pos, e_core))
    s_core = e_core[order]
    s_pos = e_pos[order]
    s_srcslot = (newid[src[order]] % SLOTS).astype(np.int64)
    s_dstslot = slot_of[dst[order]].astype(np.int64)

    bucket = s_core * NBINS + s_pos
    bnd = np.concatenate(
        [[0], np.cumsum(np.bincount(bucket, minlength=N_CORES * NBINS))]
    )
    within = np.arange(E) - bnd[bucket]
    assert (within < t_proc[s_pos] * P).all()

    flat_off = tile_base * P
    e_slot = s_core * (SUM_T * P) + flat_off[s_pos] + within

    gidx = np.zeros(N_CORES * SUM_T * P, np.int16)
    dloc = np.full(N_CORES * SUM_T * P, -1.0, np.float16)
    gidx[e_slot] = s_srcslot.astype(np.int16)
    dloc[e_slot] = s_dstslot.astype(np.float16)
    gidx = gidx.reshape(N_CORES, SUM_T, P)
    dloc = dloc.reshape(N_CORES, SUM_T, P)

    dst_cores = np.ascontiguousarray(dloc.transpose(0, 2, 1))  # [c, 128, SUM_T]

    # gather groups per (piece, q): fixed chunk-count splits
    groups = []
    pos = 0
    for piece in range(len(PIECES)):
        for q in range(N_CORES):
            c0 = 0
            for gsz in GGRPS[piece]:
                lo = tile_base[pos + c0]
                last = pos + c0 + gsz - 1
                hi = tile_base[last] + t_proc[last]
                groups.append((int(lo), int(hi)))
                c0 += gsz
            assert c0 == PIECES[piece]
            pos += PIECES[piece]

    blocks = []
    for (lo, hi) in groups:
        n = (hi - lo) * P
        w = gidx[:, lo:hi, :].reshape(N_CORES, n // 16, 16).transpose(0, 2, 1)
        blocks.append(w)
    idxw = np.concatenate(blocks, axis=2)
    I_COLS = idxw.shape[2]
    idx_cores = np.ascontiguousarray(
        np.broadcast_to(idxw[:, None, :, :], (N_CORES, 8, 16, I_COLS)).reshape(
            N_CORES, P, I_COLS
        )
    )

    x_pad = np.zeros((N_PAD, D), np.float32)
    x_pad[newid] = x
    x_loc = np.ascontiguousarray(x_pad.reshape(N_CORES, SLOTS, D).astype(np.float16))
    xT_loc = np.ascontiguousarray(x_loc.transpose(0, 2, 1))
    meta = dict(
        SUM_T=SUM_T,
        t_proc=tuple(int(t) for t in t_proc),
        groups=tuple(groups),
        I_COLS=int(I_COLS),
    )
    return meta, newid, idx_cores, dst_cores, x_loc, xT_loc


# ---------------------------------------------------------------------------
# Device program
# ---------------------------------------------------------------------------
def build_program(meta):
    SUM_T = meta["SUM_T"]
    t_proc = meta["t_proc"]
    groups = meta["groups"]
    I_COLS = meta["I_COLS"]
    tile_base = [0]
    for t in t_proc[:-1]:
        tile_base.append(tile_base[-1] + t)

    nc = bass.Bass(num_devices=N_CORES)

    p_xloc = nc.declare_dram_parameter("x_loc", [SLOTS, D], F16, isOutput=False)
    p_xT = nc.declare_dram_parameter("xT_loc", [D, SLOTS], F16, isOutput=False)
    p_idx = nc.declare_dram_parameter("gidx", [P, I_COLS], mybir.dt.int16, isOutput=False)
    p_dst = nc.declare_dram_parameter("dst_loc", [P, SUM_T], F16, isOutput=False)
    p_wrel = nc.declare_dram_parameter("wrel", [L, D, D], F32, isOutput=False)
    p_wroot = nc.declare_dram_parameter("wroot", [L, D, D], F32, isOutput=False)
    p_wrel2 = nc.declare_dram_parameter("wrel2", [D, OUT], F32, isOutput=False)
    p_wroot2m = nc.declare_dram_parameter(
        "wroot2m", [D, N_CORES * OUT], F16, isOutput=False
    )
    p_b2m = nc.declare_dram_parameter("b2m", [1, N_CORES * OUT], F16, isOutput=False)
    p_gammaT = nc.declare_dram_parameter("gammaT", [D, L], F32, isOutput=False)
    p_betaT = nc.declare_dram_parameter("betaT", [D, L], F32, isOutput=False)
    p_iotar = nc.declare_dram_parameter("iotar", [P, P * RMAX], F16, isOutput=False)
    p_ident = nc.declare_dram_parameter("ident16", [P, P], F16, isOutput=False)
    p_out = nc.declare_dram_parameter("z4", [SLOTS, OUT], F16, isOutput=True)

    rg = [list(range(N_CORES))]
    n_pieces = len(PIECES)
    piece_cols = tuple(p * P for p in PIECES)
    piece_col0 = tuple(j * P for j in PIECE_J0)

    pos_info = []
    for piece in range(n_pieces):
        jr = range(PIECE_J0[piece], PIECE_J0[piece] + PIECES[piece])
        for q in range(N_CORES):
            for j in jr:
                pos_info.append((piece, q, j))
    piece_end_pos = {}
    acc = 0
    for piece in range(n_pieces):
        acc += PIECES[piece] * N_CORES
        piece_end_pos[acc - 1] = piece

    grp_start = {lo: (lo, hi) for (lo, hi) in groups}
    max_grp_t = max(hi - lo for (lo, hi) in groups)

    # same-T runs of chunks within each gather group, for packed sel builds
    pos_of_tb = {tile_base[pos]: pos for pos in range(NBINS)}
    group_runs = {}     # grp_lo -> list of (run_tb, nc_chunks, T)
    for (lo, hi) in groups:
        runs = []
        pos = pos_of_tb[lo]
        tb = lo
        while tb < hi:
            T = t_proc[pos]
            ncr = 0
            rtb = tb
            while tb < hi and t_proc[pos] == T:
                ncr += 1
                tb += T
                pos += 1
            runs.append((rtb, ncr, T))
        group_runs[lo] = runs

    from contextlib import ExitStack

    with tile.TileContext(nc) as tc:
        with ExitStack() as stack:
            ep = stack.enter_context
            dram_tab = ep(tc.tile_pool(name="dram_tab", bufs=2, space="DRAM"))
            dram_rsi = ep(tc.tile_pool(name="dram_rsi", bufs=2, space="DRAM"))
            dram_rso = ep(tc.tile_pool(name="dram_rso", bufs=2, space="DRAM"))
            dram_cc = ep(tc.tile_pool(name="dram_cc", bufs=2, space="DRAM"))
            singles = ep(tc.tile_pool(name="singles", bufs=1))
            hT_pool = ep(tc.tile_pool(name="hT", bufs=2))
            z_pool = ep(tc.tile_pool(name="zb", bufs=1))
            agg_pool = ep(tc.tile_pool(name="aggb", bufs=1))
            g_pool = ep(tc.tile_pool(name="gath", bufs=3))
            s_pool = ep(tc.tile_pool(name="sel", bufs=3))
            stg_pool = ep(tc.tile_pool(name="stg", bufs=3))
            t16_pool = ep(tc.tile_pool(name="t16p", bufs=2))
            bn_pool = ep(tc.tile_pool(name="bns", bufs=2))
            stat_pool = ep(tc.tile_pool(name="stat", bufs=2))
            psA = ep(tc.tile_pool(name="psA", bufs=3, space="PSUM"))
            psZ = ep(tc.tile_pool(name="psZ", bufs=1, space="PSUM"))
            psT = ep(tc.tile_pool(name="psT", bufs=2, space="PSUM"))
            psF = ep(tc.tile_pool(name="psF", bufs=1, space="PSUM"))
            psP = ep(tc.tile_pool(name="psP", bufs=1, space="PSUM"))

            with tc.high_priority():
                nc.gpsimd.load_library(library_config.mlp)

            grp_sizes = sorted({(hi - lo) * P for (lo, hi) in groups})
            nidx_regs = {n: nc.gpsimd.to_reg(n) for n in grp_sizes}

            # --- constants / weights in SBUF ---
            idx_sb = singles.tile([P, I_COLS], mybir.dt.int16)
            nc.sync.dma_start(out=idx_sb[:], in_=p_idx[:])
            dst_sb = singles.tile([P, SUM_T], F16)
            nc.sync.dma_start(out=dst_sb[:], in_=p_dst[:])
            iotar_sb = singles.tile([P, P * RMAX], F16)
            nc.sync.dma_start(out=iotar_sb[:], in_=p_iotar[:])
            ident_sb = singles.tile([P, P], F16)
            nc.sync.dma_start(out=ident_sb[:], in_=p_ident[:])
            wtmp = singles.tile([P, D], F32)
            wrel_sb = singles.tile([P, L * D], F16)
            wroot_sb = singles.tile([P, L * D], F16)
            for l in range(L):
                nc.sync.dma_start(out=wtmp[:], in_=p_wrel[l])
                nc.scalar.activation(
                    out=wrel_sb[:, l * D : (l + 1) * D], in_=wtmp[:],
                    func=mybir.ActivationFunctionType.Copy,
                )
                nc.sync.dma_start(out=wtmp[:], in_=p_wroot[l])
                nc.scalar.activation(
                    out=wroot_sb[:, l * D : (l + 1) * D], in_=wtmp[:],
                    func=mybir.ActivationFunctionType.Copy,
                )
            wrel2_sb = singles.tile([P, OUT], F16)
            nc.sync.dma_start(out=wtmp[:, 0:OUT], in_=p_wrel2[:])
            nc.scalar.activation(
                out=wrel2_sb[:], in_=wtmp[:, 0:OUT],
                func=mybir.ActivationFunctionType.Copy,
            )
            wroot2m_sb = singles.tile([P, N_CORES * OUT], F16)
            nc.sync.dma_start(out=wroot2m_sb[:], in_=p_wroot2m[:])
            b2m_sb = singles.tile([1, N_CORES * OUT], F16)
            nc.sync.dma_start(out=b2m_sb[:], in_=p_b2m[:])
            gammaT_sb = singles.tile([P, L], F32)
            nc.sync.dma_start(out=gammaT_sb[:], in_=p_gammaT[:])
            betaT_sb = singles.tile([P, L], F32)
            nc.sync.dma_start(out=betaT_sb[:], in_=p_betaT[:])
            ones_sb = singles.tile([1, P], F16)
            nc.vector.memset(ones_sb[:], 1.0)
            eps_sb = singles.tile([P, 1], F32)
            nc.vector.memset(eps_sb[:], EPS)

            hT_prev = hT_pool.tile([P, SLOTS], F16, tag="hT")
            nc.sync.dma_start(out=hT_prev[:], in_=p_xT[:])
            h_tab = p_xloc

            def build_sel(grp_lo, grp_hi):
                """One-hot masks for the group's tiles in [e, dst, tile]
                layout: all DVE operands packed 2-byte -> 2x mode."""
                st = s_pool.tile([P, max_grp_t * P], F16, tag="sel")
                sb = st[:]
                db = dst_sb[:]
                ib = iotar_sb[:]
                for (rtb, ncr, T) in group_runs[grp_lo]:
                    nct = ncr * T
                    base = (rtb - grp_lo) * P
                    nc.vector.tensor_tensor(
                        out=bass.AP(
                            tensor=sb.tensor, offset=sb.offset + base,
                            ap=[sb.ap[0], [nct, P], [1, nct]],
                        ),
                        in0=bass.AP(
                            tensor=db.tensor, offset=db.offset + rtb,
                            ap=[db.ap[0], [0, P], [1, nct]],
                        ),
                        in1=bass.AP(
                            tensor=ib.tensor, offset=ib.offset,
                            ap=[ib.ap[0], [RMAX, P], [1, nct]],
                        ),
                        op=mybir.AluOpType.is_equal,
                    )
                return st

            def sel_tile_ap(st, grp_lo, pos, t):
                """Matmul operand AP for (chunk at pos, tile t): [e, 128 dst]
                with dst stride = the run's nc*T."""
                for (rtb, ncr, T) in group_runs[grp_lo]:
                    if rtb <= tile_base[pos] < rtb + ncr * T:
                        nct = ncr * T
                        base = (rtb - grp_lo) * P
                        col = tile_base[pos] - rtb + t
                        sb = st[:]
                        return bass.AP(
                            tensor=sb.tensor, offset=sb.offset + base + col,
                            ap=[sb.ap[0], [nct, P]],
                        )
                raise AssertionError("tile not in any run")

            def emit_dense_piece(piece, l, z_sb, agg_sb, stats):
                w_rel = wrel_sb[:, l * D : (l + 1) * D]
                w_root = wroot_sb[:, l * D : (l + 1) * D]
                j0, npc = PIECE_J0[piece], PIECES[piece]
                for jg in range(j0, j0 + npc, PGRP):
                    jn = min(PGRP, j0 + npc - jg)
                    psz = psZ.tile([P, PGRP * P], F32, space="PSUM")
                    for k in range(jn):
                        cs = slice((jg + k) * P, (jg + k + 1) * P)
                        ks = slice(k * P, (k + 1) * P)
                        nc.tensor.matmul(
                            out=psz[:, ks], lhsT=w_rel, rhs=agg_sb[:, cs],
                            start=True, stop=False,
                        )
                        nc.tensor.matmul(
                            out=psz[:, ks], lhsT=w_root, rhs=hT_prev[:, cs],
                            start=False, stop=True,
                        )
                    zs = slice(jg * P, (jg + jn) * P)
                    nc.scalar.activation(
                        out=z_sb[:, zs], in_=psz[:, 0 : jn * P],
                        func=mybir.ActivationFunctionType.Copy,
                    )
                    for k in range(jn):
                        j = jg + k
                        width = SHORT if j == 48 else P
                        nc.vector.bn_stats(
                            out=stats[:, j, :],
                            in_=z_sb[:, j * P : j * P + width],
                        )

            for l in range(L):
                z_sb = z_pool.tile([P, SLOTS], F16)
                agg_sb = agg_pool.tile([P, SLOTS], F16)
                stats = stat_pool.tile([P, CHUNKS, nc.vector.BN_STATS_DIM], F32)
                rs_inp = []
                for piece in range(n_pieces):
                    rst = dram_rsi.tile(
                        [N_CORES * P, piece_cols[piece]], F16, tag=f"rsi{piece}"
                    )
                    rs_inp.append(rst)

                gath = sel = None
                grp_lo = 0
                ps4 = None
                dense_done = 0
                for pos in range(NBINS):
                    piece, q, j = pos_info[pos]
                    T = t_proc[pos]
                    tb = tile_base[pos]
                    if tb in grp_start:
                        grp_lo, grp_hi = grp_start[tb]
                        ng = (grp_hi - grp_lo) * P
                        gath = g_pool.tile([P, max_grp_t * P], F16, tag="gath")
                        gv = gath.rearrange("p (t d) -> p t d", t=max_grp_t)
                        nc.gpsimd.dma_gather(
                            out_ap=gv[:, 0 : grp_hi - grp_lo, :],
                            in_ap=h_tab[:],
                            idxs_ap=idx_sb[:, grp_lo * 8 : grp_hi * 8],
                            num_idxs=ng,
                            num_idxs_reg=nidx_regs[ng],
                            elem_size=D,
                            single_packet=False,
                        )
                        sel = build_sel(grp_lo, grp_hi)
                    jj = j - PIECE_J0[piece]
                    npc = PIECES[piece]
                    pg = jj % PGRP
                    pgn = min(PGRP, npc - (jj - pg))
                    if pg == 0:
                        ps4 = psA.tile([P, PGRP * P], F32, space="PSUM")
                    for t in range(T):
                        ft = tb + t - grp_lo
                        nc.tensor.matmul(
                            out=ps4[:, pg * P : (pg + 1) * P],
                            lhsT=gath[:, ft * P : (ft + 1) * P],
                            rhs=sel_tile_ap(sel, grp_lo, pos, t),
                            start=(t == 0),
                            stop=(t == T - 1),
                        )
                    if pg == pgn - 1:
                        stg = stg_pool.tile([P, PGRP * P], F16, tag="stg")
                        nc.scalar.activation(
                            out=stg[:, 0 : pgn * P], in_=ps4[:, 0 : pgn * P],
                            func=mybir.ActivationFunctionType.Copy,
                        )
                        rs_in = rs_inp[piece]
                        pw = piece_cols[piece]
                        col0 = (j - pg) * P - piece_col0[piece]
                        dest = bass.AP(
                            tensor=rs_in.tensor,
                            offset=rs_in[:].offset + q * P * pw + col0,
                            ap=[[pw, P], [1, pgn * P]],
                        )
                        nc.sync.dma_start(out=dest, in_=stg[:, 0 : pgn * P])
                    if pos in piece_end_pos:
                        piece_id = piece_end_pos[pos]
                        ncols = piece_cols[piece_id]
                        rs_out = dram_rso.tile([P, ncols], F16, tag=f"rso{piece_id}")
                        nc.gpsimd.collective_compute(
                            "ReduceScatter",
                            mybir.AluOpType.add,
                            replica_groups=rg,
                            ins=[rs_inp[piece_id][:].opt()],
                            outs=[rs_out[:]],
                        )
                        c0 = piece_col0[piece_id]
                        nc.sync.dma_start(
                            out=agg_sb[:, c0 : c0 + ncols], in_=rs_out[:]
                        )
                    # piece-0 dense work interleaves once piece 2 starts
                    if pos == (PIECES[0] + PIECES[1]) * N_CORES - 1:
                        emit_dense_piece(0, l, z_sb, agg_sb, stats)
                        dense_done = 1

                for piece in range(dense_done, n_pieces):
                    emit_dense_piece(piece, l, z_sb, agg_sb, stats)

                # ---- BatchNorm across all nodes (tiny stats AllGather) ----
                bs = bn_pool.tile([P, 16], F32)
                mv = bs[:, 0:2]
                with tc.high_priority():
                    nc.vector.bn_aggr(out=mv, in_=stats[:])
                cc_sb = bs[:, 3:5]
                with tc.high_priority():
                    nc.vector.tensor_copy(out=cc_sb[:, 0:1], in_=mv[:, 0:1])
                    nc.vector.tensor_scalar(
                        out=cc_sb[:, 1:2], in0=mv[:, 0:1], scalar1=mv[:, 0:1],
                        scalar2=mv[:, 1:2], op0=mybir.AluOpType.mult,
                        op1=mybir.AluOpType.add,
                    )
                cc_in = dram_cc.tile([P, 2], F32)
                cc_out = dram_cc.tile([P * N_CORES, 2], F32, addr_space="Shared")
                nc.sync.dma_start(out=cc_in[:], in_=cc_sb)
                nc.gpsimd.collective_compute(
                    "AllGather", mybir.AluOpType.bypass, replica_groups=rg,
                    ins=[cc_in.opt()], outs=[cc_out.opt()],
                )
                cc_all = bn_pool.tile([P, 2, N_CORES], F32)
                cc_src = bass.AP(
                    tensor=cc_out.tensor,
                    offset=cc_out[:].offset,
                    ap=[[2, P], [1, 2], [2 * P, N_CORES]],
                )
                nc.sync.dma_start(out=cc_all[:], in_=cc_src)
                cc_res = bs[:, 5:7]
                nc.vector.tensor_reduce(
                    out=cc_res.rearrange("p (a b) -> p a b", a=2),
                    in_=cc_all[:],
                    axis=mybir.AxisListType.X,
                    op=mybir.AluOpType.add,
                )
                mu = bs[:, 7:8]
                nc.vector.tensor_scalar(
                    out=mu, in0=cc_res[:, 0:1], scalar2=None,
                    op0=mybir.AluOpType.mult, scalar1=1.0 / N_CORES,
                )
                var = bs[:, 8:9]
                nc.vector.tensor_scalar(
                    out=var, in0=cc_res[:, 1:2], scalar2=None,
                    op0=mybir.AluOpType.mult, scalar1=1.0 / N_CORES,
                )
                mu2 = bs[:, 9:10]
                nc.vector.tensor_tensor(
                    out=mu2, in0=mu, in1=mu, op=mybir.AluOpType.mult
                )
                nc.vector.tensor_tensor(
                    out=var, in0=var, in1=mu2, op=mybir.AluOpType.subtract
                )
                rstd = bs[:, 10:11]
                nc.scalar.activation(
                    out=rstd, in_=var,
                    func=mybir.ActivationFunctionType.Sqrt,
                    bias=eps_sb[:], scale=1.0,
                )
                nc.vector.reciprocal(out=rstd, in_=rstd)
                scale = bs[:, 11:12]
                nc.vector.tensor_tensor(
                    out=scale, in0=rstd, in1=gammaT_sb[:, l : l + 1],
                    op=mybir.AluOpType.mult,
                )
                shift = bs[:, 12:13]
                nc.vector.tensor_tensor(
                    out=shift, in0=mu, in1=scale, op=mybir.AluOpType.mult
                )
                nc.vector.tensor_tensor(
                    out=shift, in0=betaT_sb[:, l : l + 1], in1=shift,
                    op=mybir.AluOpType.subtract,
                )

                # BN apply + relu, zero pad slots, rebuild node-major table
                hT_new = hT_pool.tile([P, SLOTS], F16, tag="hT")
                h_tab_new = dram_tab.tile([SLOTS, D], F16, tag="htab")
                for c0 in range(0, CHUNKS, AGRP):
                    ng = min(AGRP, CHUNKS - c0)
                    gs = slice(c0 * P, (c0 + ng) * P)
                    nc.scalar.activation(
                        out=hT_new[:, gs], in_=z_sb[:, gs],
                        func=mybir.ActivationFunctionType.Relu,
                        bias=shift, scale=scale,
                    )
                    if c0 + ng == CHUNKS:
                        nc.vector.memset(hT_new[:, QUOTA:SLOTS], 0.0)
                    t16g = t16_pool.tile([P, AGRP, P], F16)
                    for k in range(ng):
                        c = c0 + k
                        cs2 = slice(c * P, (c + 1) * P)
                        ps_t = psT.tile([P, P], F16, space="PSUM")
                        nc.tensor.transpose(
                            out=ps_t[:], in_=hT_new[:, cs2], identity=ident_sb[:]
                        )
                        nc.vector.tensor_copy(out=t16g[:, k, :], in_=ps_t[:])
                    dest = bass.AP(
                        tensor=h_tab_new.tensor,
                        offset=h_tab_new[:].offset + c0 * P * D,
                        ap=[[D, P], [P * D, ng], [1, D]],
                    )
                    nc.sync.dma_start(out=dest, in_=t16g[:, 0:ng, :])
                hT_prev = hT_new
                h_tab = h_tab_new

            # ---------------- final GraphConv (OUT=2) ----------------
            # transpose-mode gather -> gathT [feat, edge]; PE projection
            # through Wrel2 -> proj [edge, 2]; segment-sum emits
            # feature-major partials [2, slots] incl. masked root + bias.
            rs_in_f = dram_rsi.tile([N_PAD, OUT], F16, tag="rsif")
            sel = None
            proj_sb = None
            grp_lo = 0
            psf = None
            for pos in range(NBINS):
                piece, q, j = pos_info[pos]
                T = t_proc[pos]
                tb = tile_base[pos]
                if tb in grp_start:
                    grp_lo, grp_hi = grp_start[tb]
                    ng = (grp_hi - grp_lo) * P
                    nt = grp_hi - grp_lo
                    gathT = g_pool.tile([P, max_grp_t * P], F16, tag="gath")
                    gtb = gathT[:]
                    nc.gpsimd.dma_gather(
                        out_ap=bass.AP(
                            tensor=gtb.tensor,
                            offset=gtb.offset,
                            ap=[gtb.ap[0], [ng, 1], [1, ng]],
                        ),
                        in_ap=h_tab[:],
                        idxs_ap=idx_sb[:, grp_lo * 8 : grp_hi * 8],
                        num_idxs=ng,
                        num_idxs_reg=nidx_regs[ng],
                        elem_size=D,
                        transpose=True,
                        single_packet=False,
                    )
                    psp = psP.tile([P, max_grp_t * OUT], F32, space="PSUM", tag="psp")
                    for t in range(nt):
                        nc.tensor.matmul(
                            out=psp[:, t * OUT : (t + 1) * OUT],
                            lhsT=gathT[:, t * P : (t + 1) * P],
                            rhs=wrel2_sb[:],
                            start=True, stop=True,
                        )
                    proj_sb = stg_pool.tile([P, max_grp_t * OUT], F16, tag="proj")
                    nc.scalar.activation(
                        out=proj_sb[:, 0 : nt * OUT], in_=psp[:, 0 : nt * OUT],
                        func=mybir.ActivationFunctionType.Copy,
                    )
                    sel = build_sel(grp_lo, grp_hi)
                jj = j - PIECE_J0[piece]
                npc = PIECES[piece]
                pg = jj % PGRP
                pgn = min(PGRP, npc - (jj - pg))
                if pg == 0:
                    psf = psF.tile([P, PGRP * OUT], F32, space="PSUM", tag="psf")
                fo = slice(pg * OUT, (pg + 1) * OUT)
                for t in range(T):
                    nc.tensor.matmul(
                        out=psf[:, fo],
                        lhsT=sel_tile_ap(sel, grp_lo, pos, t),
                        rhs=proj_sb[:, (tb + t - grp_lo) * OUT : (tb + t - grp_lo + 1) * OUT],
                        start=(t == 0),
                        stop=False,
                    )
                # root + bias: nonzero only on the rank that owns these slots
                nc.tensor.matmul(
                    out=psf[:, fo],
                    lhsT=hT_prev[:, j * P : (j + 1) * P],
                    rhs=wroot2m_sb[:, q * OUT : (q + 1) * OUT],
                    start=False,
                    stop=False,
                )
                nc.tensor.matmul(
                    out=psf[:, fo],
                    lhsT=ones_sb[:],
                    rhs=b2m_sb[:, q * OUT : (q + 1) * OUT],
                    start=False,
                    stop=True,
                )
                if pg == pgn - 1:
                    stgf = stg_pool.tile([P, PGRP * OUT], F16, tag="stgf")
                    nc.scalar.activation(
                        out=stgf[:, 0 : pgn * OUT], in_=psf[:, 0 : pgn * OUT],
                        func=mybir.ActivationFunctionType.Copy,
                    )
                    r0 = q * SLOTS + (j - pg) * P
                    dest = bass.AP(
                        tensor=rs_in_f.tensor,
                        offset=rs_in_f[:].offset + r0 * OUT,
                        ap=[[OUT, P], [P * OUT, pgn], [1, OUT]],
                    )
                    nc.sync.dma_start(out=dest, in_=stgf[:, 0 : pgn * OUT])

            rs_out_f = dram_rso.tile([SLOTS, OUT], F16, tag="rsof")
            nc.gpsimd.collective_compute(
                "ReduceScatter",
                mybir.AluOpType.add,
                replica_groups=rg,
                ins=[rs_in_f[:].opt()],
                outs=[rs_out_f[:]],
            )
            nc.sync.dma_start(out=p_out[:], in_=rs_out_f[:])

    lower_extended_insts(nc)
    _split_multiwait(nc)
    return nc


_PROGRAM_CACHE = {}


def _get_program(meta):
    key = hashlib.sha1(repr(sorted(meta.items())).encode()).hexdigest()
    if key not in _PROGRAM_CACHE:
        _PROGRAM_CACHE[key] = build_program(meta)
    return _PROGRAM_CACHE[key]


def _make_in_maps(idx_cores, dst_cores, x_loc, xT_loc,
                  Wrel, Wroot, gamma, beta, Wrel2, Wroot2, b2):
    iotar = np.zeros((P, P, RMAX), np.float16)
    iotar[:, :, :] = np.arange(P, dtype=np.float16)[None, :, None]
    ident16 = np.eye(P, dtype=np.float16)
    common = dict(
        wrel=np.ascontiguousarray(np.asarray(Wrel, np.float32)),
        wroot=np.ascontiguousarray(np.asarray(Wroot, np.float32)),
        wrel2=np.ascontiguousarray(np.asarray(Wrel2, np.float32)),
        gammaT=np.ascontiguousarray(np.asarray(gamma, np.float32).T),
        betaT=np.ascontiguousarray(np.asarray(beta, np.float32).T),
        iotar=np.ascontiguousarray(iotar.reshape(P, P * RMAX)),
        ident16=ident16,
    )
    wroot2 = np.asarray(Wroot2, np.float16)                 # [D, OUT]
    b2 = np.asarray(b2, np.float16).reshape(1, OUT)
    in_maps = []
    for c in range(N_CORES):
        w2m = np.zeros((D, N_CORES, OUT), np.float16)
        w2m[:, c, :] = wroot2
        b2m = np.zeros((1, N_CORES, OUT), np.float16)
        b2m[:, c, :] = b2
        m = dict(common)
        m["x_loc"] = x_loc[c]
        m["xT_loc"] = xT_loc[c]
        m["gidx"] = idx_cores[c]
        m["dst_loc"] = dst_cores[c]
        m["wroot2m"] = np.ascontiguousarray(w2m.reshape(D, N_CORES * OUT))
        m["b2m"] = np.ascontiguousarray(b2m.reshape(1, N_CORES * OUT))
        in_maps.append(m)
    return in_maps


def run(x, edge_index, Wrel, Wroot, b, gamma, beta, Wrel2, Wroot2, b2):
    """Returns (output [N, OUT] float32, nc, meta) - nc exposed for profiling.
    The per-layer GraphConv bias b cancels inside BatchNorm and is unused."""
    meta, newid, idx_cores, dst_cores, x_loc, xT_loc = _preprocess(x, edge_index)
    nc = _get_program(meta)
    in_maps = _make_in_maps(
        idx_cores, dst_cores, x_loc, xT_loc,
        Wrel, Wroot, gamma, beta, Wrel2, Wroot2, b2,
    )
    from concourse.bass_utils import run_bass_kernel_spmd

    res = run_bass_kernel_spmd(nc, in_maps, list(range(N_CORES)))
    full = np.concatenate(
        [res.results[c]["z4"] for c in range(N_CORES)], axis=0
    )  # [N_PAD, OUT]
    return full[newid].astype(np.float32), nc, meta


def kernel(**inputs):
    out, _, _ = run(**{k: np.asarray(v) for k, v in inputs.items()})
    return out


# revision 19
# speedup vs baseline: 1.0854x; 1.0138x over previous
"""GNN message-passing (3x GraphConv+BN+ReLU, final GraphConv) on 8 trn2 cores.

Source-sharded graph parallelism:
  - Nodes are partitioned across 8 cores (6272 slots each, 49 chunks of 128).
    Each core processes the edges whose SOURCE it owns, so per-edge feature
    gathers read a small local fp16 table (6272 rows, int16 indices).
  - Per layer: indirect-DMA gather of the core's edge source rows, one-hot
    matmuls accumulate partial aggregates for ALL 392 destination chunks in
    PSUM, partials stream to a DRAM buffer, and a ReduceScatter (split in
    three pieces, overlapped with the gather phase) reduces them onto the
    destination owner.  Dense transforms + BatchNorm stats/apply are local;
    only a tiny [128,2] stats AllGather crosses cores per layer.
  - One-hot masks are built in a [edge, dst, tile] layout so every DVE
    operand is packed 2-byte (2x DVE mode); tiles read them back with a
    strided matmul AP.
  - The GraphConv bias cancels inside BatchNorm and is skipped; every core
    holds exactly 6250 real nodes with its 22 pad slots pinned to the tail
    of chunk 48, so BN stats are exact and pads are re-zeroed by one memset.
  - Final layer: transpose-mode gather delivers gathT [feat, edge] tiles of
    h3, a per-tile PE projection through Wrel2 gives 2-col edge values, and
    the segment-sum emits feature-major partials; Wroot2+b2 ride along as
    per-rank masked matmuls so the final ReduceScatter yields the output.
"""

import hashlib
import heapq
import sys

import numpy as np

sys.path.insert(0, "/opt/trn_rl_repo")

import concourse.bass as bass  # noqa: E402
import concourse.mybir as mybir  # noqa: E402
import concourse.tile as tile  # noqa: E402
from concourse.vector_clock import ScopedClock  # noqa: E402
from concourse import library_config  # noqa: E402
from concourse.library_overlay import lower_extended_insts  # noqa: E402

N = 50000
E = 800000
D = 128
L = 3
OUT = 2
EPS = 1e-5
N_CORES = 8
P = 128
CHUNKS = 49                 # local dst chunks per core
SLOTS = CHUNKS * P          # 6272
NBINS = N_CORES * CHUNKS    # 392 global dst chunks
N_PAD = N_CORES * SLOTS     # 50176
QUOTA = N // N_CORES        # 6250 real nodes per core
SHORT = QUOTA - 48 * P      # 106 real slots in chunk 48
PIECES = (25, 16, 8)        # local chunks per RS piece
PIECE_J0 = (0, 25, 41)
GGRPS = ((13, 12), (8, 8), (8,))   # gather-group sizes per piece
PGRP = 4                    # chunks per PSUM bank / staging DMA group
RMAX = 36                   # max tiles covered by one sel build (12 chunks x T3)
AGRP = 25                   # chunks per BN-apply / table-write group

F16 = mybir.dt.float16
F32 = mybir.dt.float32

# ---------------------------------------------------------------------------
# walrus in this container accepts at most ONE semaphore wait per instruction.
# Patch the Tile exit drain and add a post-pass splitting multi-wait insts.
# ---------------------------------------------------------------------------
_MAX_WAITS = 1


def _drain_and_barrier(self, tick_clock, wait_clock):
    nc = self.nc
    drain_inst = nc.sync.drain()
    wait_clock.add_sem_waits(
        drain_inst.ins, ScopedClock({None: tick_clock.global_clock})
    )
    si = drain_inst.ins.sync_info
    if si is not None and si.on_wait is not None and len(si.on_wait) > _MAX_WAITS:
        waits = list(si.on_wait)
        si.on_wait = waits[:_MAX_WAITS]
        rest = waits[_MAX_WAITS:]
        for i in range(0, len(rest), _MAX_WAITS):
            nop = nc.sync.nop(nofuse=True)
            nop.ins.sync_info = mybir.SyncInfo(
                on_wait=rest[i : i + _MAX_WAITS], on_update=[]
            )
    nc.all_engine_barrier()
    assert self.sems is not None
    popped = nc._tile_sem_poison_stack.pop()
    assert popped is self._sem_poison
    nc.clear_and_free_semaphores(list(self.sems.allocated().values()))
    nc.all_engine_barrier()


tile.TileContext._drain_and_barrier = _drain_and_barrier


def _split_multiwait(nc):
    n_split = 0
    for fn in nc.m.functions:
        for blk in fn.blocks:
            out = []
            for inst in blk.instructions:
                si = inst.sync_info
                if si is not None and si.on_wait and len(si.on_wait) > _MAX_WAITS:
                    waits = list(si.on_wait)
                    si.on_wait = waits[-_MAX_WAITS:]
                    rest = waits[:-_MAX_WAITS]
                    for i in range(0, len(rest), _MAX_WAITS):
                        n_split += 1
                        out.append(
                            mybir.InstNoOp(
                                name=f"{inst.name}-ws{i}",
                                engine=inst.engine,
                                ins=[],
                                outs=[],
                                bass_nofuse=True,
                                sync_info=mybir.SyncInfo(
                                    on_wait=rest[i : i + _MAX_WAITS], on_update=[]
                                ),
                                debug=inst.debug,
                            )
                        )
                out.append(inst)
            blk.instructions[:] = out
    return n_split


# ---------------------------------------------------------------------------
# Host-side graph partitioning
# ---------------------------------------------------------------------------
def _lpt(nodes, deg_in, bins, caps, bin_of, slot_of, fill):
    heap = [(0, b) for b in bins]
    heapq.heapify(heap)
    for node in nodes:
        d = int(deg_in[node])
        ld, b = heapq.heappop(heap)
        bin_of[node] = b
        slot_of[node] = fill[b]
        fill[b] += 1
        if fill[b] < caps[b]:
            heapq.heappush(heap, (ld + d, b))


def _partition_nodes(deg_in):
    """Assign nodes to (bin, slot): bin b -> core b%8, local chunk b//8.
    Every bin is filled exactly to its cap (128, or 106 for chunk 48), so
    each core holds exactly 6250 real nodes and pads sit at the tail of
    chunk 48.  The heaviest nodes fill a set of "heavy" bins; the rest are
    LPT'd over "light" bins so per-(core,bin) edge counts pack tightly."""
    caps = np.full(NBINS, P, np.int64)
    caps[48 * N_CORES :] = SHORT                 # bins (q, j=48)
    order = np.argsort(-deg_in, kind="stable")
    sdeg = deg_in[order].astype(np.float64)
    pref = np.concatenate([[0.0], np.cumsum(sdeg)])
    total = pref[-1]
    best = (None, None)
    for nh in range(0, 200, 8):
        nl = NBINS - nh
        s_h = pref[min(nh * P, N)]
        m_h = s_h / max(nh, 1) / N_CORES
        m_l = (total - s_h) / nl / N_CORES
        t_h = int(np.ceil((m_h + 3.0 * np.sqrt(m_h * 0.875 + 1)) / P)) if nh else 0
        t_l = int(np.ceil((m_l + 3.0 * np.sqrt(m_l * 0.875 + 1)) / P))
        st = nh * max(t_h, 1) + nl * t_l
        if best[0] is None or st < best[0]:
            best = (st, nh)
    n_heavy = best[1]
    j_cut = CHUNKS - n_heavy // N_CORES
    allb = np.arange(NBINS)
    heavy_bins = allb[allb // N_CORES >= j_cut]
    light_bins = allb[allb // N_CORES < j_cut]

    bin_of = np.empty(N, np.int32)
    slot_of = np.empty(N, np.int32)
    fill = np.zeros(NBINS, np.int64)
    nh_nodes = int(caps[heavy_bins].sum())
    _lpt(order[:nh_nodes], deg_in, heavy_bins, caps, bin_of, slot_of, fill)
    _lpt(order[nh_nodes:], deg_in, light_bins, caps, bin_of, slot_of, fill)
    assert (fill == caps).all()
    return bin_of, slot_of


def _preprocess(x, edge_index):
    x = np.asarray(x, np.float32)
    ei = np.asarray(edge_index)
    src = ei[0].astype(np.int64)
    dst = ei[1].astype(np.int64)
    deg_in = np.bincount(dst, minlength=N)
    bin_of, slot_of = _partition_nodes(deg_in)

    core_of = bin_of % N_CORES
    newid = (
        core_of.astype(np.int64) * SLOTS
        + (bin_of // N_CORES).astype(np.int64) * P
        + slot_of
    )

    e_core = core_of[src]
    e_bin = bin_of[dst]

    cnt = np.zeros((N_CORES, NBINS), np.int64)
    np.add.at(cnt, (e_core, e_bin), 1)
    t_bin = np.maximum(1, -(-cnt.max(axis=0) // P))

    # chunk processing order: per piece, (q, j) with j in the piece range
    ordered_bins = []
    for piece in range(len(PIECES)):
        jr = range(PIECE_J0[piece], PIECE_J0[piece] + PIECES[piece])
        for q in range(N_CORES):
            for j in jr:
                ordered_bins.append(j * N_CORES + q)
    ordered_bins = np.array(ordered_bins)
    bin_pos = np.empty(NBINS, np.int64)
    bin_pos[ordered_bins] = np.arange(NBINS)
    t_proc = t_bin[ordered_bins]
    tile_base = np.concatenate([[0], np.cumsum(t_proc)[:-1]])
    SUM_T = int(t_proc.sum())

    e_pos = bin_pos[e_bin]
    order = np.lexsort((e_pos, e_core))
    s_core = e_core[order]
    s_pos = e_pos[order]
    s_srcslot = (newid[src[order]] % SLOTS).astype(np.int64)
    s_dstslot = slot_of[dst[order]].astype(np.int64)

    bucket = s_core * NBINS + s_pos
    bnd = np.concatenate(
        [[0], np.cumsum(np.bincount(bucket, minlength=N_CORES * NBINS))]
    )
    within = np.arange(E) - bnd[bucket]
    assert (within < t_proc[s_pos] * P).all()

    flat_off = tile_base * P
    e_slot = s_core * (SUM_T * P) + flat_off[s_pos] + within

    gidx = np.zeros(N_CORES * SUM_T * P, np.int16)
    dloc = np.full(N_CORES * SUM_T * P, -1.0, np.float16)
    gidx[e_slot] = s_srcslot.astype(np.int16)
    dloc[e_slot] = s_dstslot.astype(np.float16)
    gidx = gidx.reshape(N_CORES, SUM_T, P)
    dloc = dloc.reshape(N_CORES, SUM_T, P)

    dst_cores = np.ascontiguousarray(dloc.transpose(0, 2, 1))  # [c, 128, SUM_T]

    # gather groups per (piece, q): fixed chunk-count splits
    groups = []
    pos = 0
    for piece in range(len(PIECES)):
        for q in range(N_CORES):
            c0 = 0
            for gsz in GGRPS[piece]:
                lo = tile_base[pos + c0]
                last = pos + c0 + gsz - 1
                hi = tile_base[last] + t_proc[last]
                groups.append((int(lo), int(hi)))
                c0 += gsz
            assert c0 == PIECES[piece]
            pos += PIECES[piece]

    blocks = []
    for (lo, hi) in groups:
        n = (hi - lo) * P
        w = gidx[:, lo:hi, :].reshape(N_CORES, n // 16, 16).transpose(0, 2, 1)
        blocks.append(w)
    idxw = np.concatenate(blocks, axis=2)
    I_COLS = idxw.shape[2]
    idx_cores = np.ascontiguousarray(
        np.broadcast_to(idxw[:, None, :, :], (N_CORES, 8, 16, I_COLS)).reshape(
            N_CORES, P, I_COLS
        )
    )

    x_pad = np.zeros((N_PAD, D), np.float32)
    x_pad[newid] = x
    x_loc = np.ascontiguousarray(x_pad.reshape(N_CORES, SLOTS, D).astype(np.float16))
    xT_loc = np.ascontiguousarray(x_loc.transpose(0, 2, 1))
    meta = dict(
        SUM_T=SUM_T,
        t_proc=tuple(int(t) for t in t_proc),
        groups=tuple(groups),
        I_COLS=int(I_COLS),
    )
    return meta, newid, idx_cores, dst_cores, x_loc, xT_loc


# ---------------------------------------------------------------------------
# Device program
# ---------------------------------------------------------------------------
def build_program(meta):
    SUM_T = meta["SUM_T"]
    t_proc = meta["t_proc"]
    groups = meta["groups"]
    I_COLS = meta["I_COLS"]
    tile_base = [0]
    for t in t_proc[:-1]:
        tile_base.append(tile_base[-1] + t)

    nc = bass.Bass(num_devices=N_CORES)

    p_xloc = nc.declare_dram_parameter("x_loc", [SLOTS, D], F16, isOutput=False)
    p_xT = nc.declare_dram_parameter("xT_loc", [D, SLOTS], F16, isOutput=False)
    p_idx = nc.declare_dram_parameter("gidx", [P, I_COLS], mybir.dt.int16, isOutput=False)
    p_dst = nc.declare_dram_parameter("dst_loc", [P, SUM_T], F16, isOutput=False)
    p_wrel = nc.declare_dram_parameter("wrel", [L, D, D], F32, isOutput=False)
    p_wroot = nc.declare_dram_parameter("wroot", [L, D, D], F32, isOutput=False)
    p_wrel2 = nc.declare_dram_parameter("wrel2", [D, OUT], F32, isOutput=False)
    p_wroot2m = nc.declare_dram_parameter(
        "wroot2m", [D, N_CORES * OUT], F16, isOutput=False
    )
    p_b2m = nc.declare_dram_parameter("b2m", [1, N_CORES * OUT], F16, isOutput=False)
    p_gammaT = nc.declare_dram_parameter("gammaT", [D, L], F32, isOutput=False)
    p_betaT = nc.declare_dram_parameter("betaT", [D, L], F32, isOutput=False)
    p_iotar = nc.declare_dram_parameter("iotar", [P, P * RMAX], F16, isOutput=False)
    p_ident = nc.declare_dram_parameter("ident16", [P, P], F16, isOutput=False)
    p_out = nc.declare_dram_parameter("z4", [SLOTS, OUT], F16, isOutput=True)

    rg = [list(range(N_CORES))]
    n_pieces = len(PIECES)
    piece_cols = tuple(p * P for p in PIECES)
    piece_col0 = tuple(j * P for j in PIECE_J0)

    pos_info = []
    for piece in range(n_pieces):
        jr = range(PIECE_J0[piece], PIECE_J0[piece] + PIECES[piece])
        for q in range(N_CORES):
            for j in jr:
                pos_info.append((piece, q, j))
    piece_end_pos = {}
    acc = 0
    for piece in range(n_pieces):
        acc += PIECES[piece] * N_CORES
        piece_end_pos[acc - 1] = piece

    grp_start = {lo: (lo, hi) for (lo, hi) in groups}
    max_grp_t = max(hi - lo for (lo, hi) in groups)

    # same-T runs of chunks within each gather group, for packed sel builds
    pos_of_tb = {tile_base[pos]: pos for pos in range(NBINS)}
    group_runs = {}     # grp_lo -> list of (run_tb, nc_chunks, T)
    for (lo, hi) in groups:
        runs = []
        pos = pos_of_tb[lo]
        tb = lo
        while tb < hi:
            T = t_proc[pos]
            ncr = 0
            rtb = tb
            while tb < hi and t_proc[pos] == T:
                ncr += 1
                tb += T
                pos += 1
            runs.append((rtb, ncr, T))
        group_runs[lo] = runs

    from contextlib import ExitStack

    with tile.TileContext(nc) as tc:
        with ExitStack() as stack:
            ep = stack.enter_context
            dram_tab = ep(tc.tile_pool(name="dram_tab", bufs=2, space="DRAM"))
            dram_rsi = ep(tc.tile_pool(name="dram_rsi", bufs=2, space="DRAM"))
            dram_rso = ep(tc.tile_pool(name="dram_rso", bufs=2, space="DRAM"))
            dram_cc = ep(tc.tile_pool(name="dram_cc", bufs=2, space="DRAM"))
            singles = ep(tc.tile_pool(name="singles", bufs=1))
            hT_pool = ep(tc.tile_pool(name="hT", bufs=2))
            z_pool = ep(tc.tile_pool(name="zb", bufs=1))
            agg_pool = ep(tc.tile_pool(name="aggb", bufs=1))
            g_pool = ep(tc.tile_pool(name="gath", bufs=3))
            s_pool = ep(tc.tile_pool(name="sel", bufs=3))
            stg_pool = ep(tc.tile_pool(name="stg", bufs=3))
            t16_pool = ep(tc.tile_pool(name="t16p", bufs=2))
            bn_pool = ep(tc.tile_pool(name="bns", bufs=2))
            stat_pool = ep(tc.tile_pool(name="stat", bufs=2))
            psA = ep(tc.tile_pool(name="psA", bufs=3, space="PSUM"))
            psZ = ep(tc.tile_pool(name="psZ", bufs=1, space="PSUM"))
            psT = ep(tc.tile_pool(name="psT", bufs=2, space="PSUM"))
            psF = ep(tc.tile_pool(name="psF", bufs=1, space="PSUM"))
            psP = ep(tc.tile_pool(name="psP", bufs=1, space="PSUM"))

            with tc.high_priority():
                nc.gpsimd.load_library(library_config.mlp)

            grp_sizes = sorted({(hi - lo) * P for (lo, hi) in groups})
            nidx_regs = {n: nc.gpsimd.to_reg(n) for n in grp_sizes}

            # --- constants / weights in SBUF ---
            idx_sb = singles.tile([P, I_COLS], mybir.dt.int16)
            nc.sync.dma_start(out=idx_sb[:], in_=p_idx[:])
            dst_sb = singles.tile([P, SUM_T], F16)
            nc.sync.dma_start(out=dst_sb[:], in_=p_dst[:])
            iotar_sb = singles.tile([P, P * RMAX], F16)
            nc.sync.dma_start(out=iotar_sb[:], in_=p_iotar[:])
            ident_sb = singles.tile([P, P], F16)
            nc.sync.dma_start(out=ident_sb[:], in_=p_ident[:])
            wtmp = singles.tile([P, D], F32)
            wrel_sb = singles.tile([P, L * D], F16)
            wroot_sb = singles.tile([P, L * D], F16)
            for l in range(L):
                nc.sync.dma_start(out=wtmp[:], in_=p_wrel[l])
                nc.scalar.activation(
                    out=wrel_sb[:, l * D : (l + 1) * D], in_=wtmp[:],
                    func=mybir.ActivationFunctionType.Copy,
                )
                nc.sync.dma_start(out=wtmp[:], in_=p_wroot[l])
                nc.scalar.activation(
                    out=wroot_sb[:, l * D : (l + 1) * D], in_=wtmp[:],
                    func=mybir.ActivationFunctionType.Copy,
                )
            wrel2_sb = singles.tile([P, OUT], F16)
            nc.sync.dma_start(out=wtmp[:, 0:OUT], in_=p_wrel2[:])
            nc.scalar.activation(
                out=wrel2_sb[:], in_=wtmp[:, 0:OUT],
                func=mybir.ActivationFunctionType.Copy,
            )
            wroot2m_sb = singles.tile([P, N_CORES * OUT], F16)
            nc.sync.dma_start(out=wroot2m_sb[:], in_=p_wroot2m[:])
            b2m_sb = singles.tile([1, N_CORES * OUT], F16)
            nc.sync.dma_start(out=b2m_sb[:], in_=p_b2m[:])
            gammaT_sb = singles.tile([P, L], F32)
            nc.sync.dma_start(out=gammaT_sb[:], in_=p_gammaT[:])
            betaT_sb = singles.tile([P, L], F32)
            nc.sync.dma_start(out=betaT_sb[:], in_=p_betaT[:])
            ones_sb = singles.tile([1, P], F16)
            nc.vector.memset(ones_sb[:], 1.0)
            eps_sb = singles.tile([P, 1], F32)
            nc.vector.memset(eps_sb[:], EPS)

            hT_prev = hT_pool.tile([P, SLOTS], F16, tag="hT")
            nc.sync.dma_start(out=hT_prev[:], in_=p_xT[:])
            h_tab = p_xloc

            def build_sel(grp_lo, grp_hi):
                """One-hot masks for the group's tiles in [e, dst, tile]
                layout: all DVE operands packed 2-byte -> 2x mode."""
                st = s_pool.tile([P, max_grp_t * P], F16, tag="sel")
                sb = st[:]
                db = dst_sb[:]
                ib = iotar_sb[:]
                for (rtb, ncr, T) in group_runs[grp_lo]:
                    nct = ncr * T
                    base = (rtb - grp_lo) * P
                    nc.vector.tensor_tensor(
                        out=bass.AP(
                            tensor=sb.tensor, offset=sb.offset + base,
                            ap=[sb.ap[0], [nct, P], [1, nct]],
                        ),
                        in0=bass.AP(
                            tensor=db.tensor, offset=db.offset + rtb,
                            ap=[db.ap[0], [0, P], [1, nct]],
                        ),
                        in1=bass.AP(
                            tensor=ib.tensor, offset=ib.offset,
                            ap=[ib.ap[0], [RMAX, P], [1, nct]],
                        ),
                        op=mybir.AluOpType.is_equal,
                    )
                return st

            def sel_tile_ap(st, grp_lo, pos, t):
                """Matmul operand AP for (chunk at pos, tile t): [e, 128 dst]
                with dst stride = the run's nc*T."""
                for (rtb, ncr, T) in group_runs[grp_lo]:
                    if rtb <= tile_base[pos] < rtb + ncr * T:
                        nct = ncr * T
                        base = (rtb - grp_lo) * P
                        col = tile_base[pos] - rtb + t
                        sb = st[:]
                        return bass.AP(
                            tensor=sb.tensor, offset=sb.offset + base + col,
                            ap=[sb.ap[0], [nct, P]],
                        )
                raise AssertionError("tile not in any run")

            def emit_dense_piece(piece, l, z_sb, agg_sb, stats):
                w_rel = wrel_sb[:, l * D : (l + 1) * D]
                w_root = wroot_sb[:, l * D : (l + 1) * D]
                j0, npc = PIECE_J0[piece], PIECES[piece]
                for jg in range(j0, j0 + npc, PGRP):
                    jn = min(PGRP, j0 + npc - jg)
                    psz = psZ.tile([P, PGRP * P], F32, space="PSUM")
                    for k in range(jn):
                        cs = slice((jg + k) * P, (jg + k + 1) * P)
                        ks = slice(k * P, (k + 1) * P)
                        nc.tensor.matmul(
                            out=psz[:, ks], lhsT=w_rel, rhs=agg_sb[:, cs],
                            start=True, stop=False,
                        )
                        nc.tensor.matmul(
                            out=psz[:, ks], lhsT=w_root, rhs=hT_prev[:, cs],
                            start=False, stop=True,
                        )
                    zs = slice(jg * P, (jg + jn) * P)
                    nc.scalar.activation(
                        out=z_sb[:, zs], in_=psz[:, 0 : jn * P],
                        func=mybir.ActivationFunctionType.Copy,
                    )
                    for k in range(jn):
                        j = jg + k
                        width = SHORT if j == 48 else P
                        nc.vector.bn_stats(
                            out=stats[:, j, :],
                            in_=z_sb[:, j * P : j * P + width],
                        )

            for l in range(L):
                z_sb = z_pool.tile([P, SLOTS], F16)
                agg_sb = agg_pool.tile([P, SLOTS], F16)
                stats = stat_pool.tile([P, CHUNKS, nc.vector.BN_STATS_DIM], F32)
                rs_inp = []
                for piece in range(n_pieces):
                    rst = dram_rsi.tile(
                        [N_CORES * P, piece_cols[piece]], F16, tag=f"rsi{piece}"
                    )
                    rs_inp.append(rst)

                gath = sel = None
                grp_lo = 0
                ps4 = None
                rs_outs = []
                for pos in range(NBINS):
                    piece, q, j = pos_info[pos]
                    T = t_proc[pos]
                    tb = tile_base[pos]
                    if tb in grp_start:
                        grp_lo, grp_hi = grp_start[tb]
                        ng = (grp_hi - grp_lo) * P
                        gath = g_pool.tile([P, max_grp_t * P], F16, tag="gath")
                        gv = gath.rearrange("p (t d) -> p t d", t=max_grp_t)
                        nc.gpsimd.dma_gather(
                            out_ap=gv[:, 0 : grp_hi - grp_lo, :],
                            in_ap=h_tab[:],
                            idxs_ap=idx_sb[:, grp_lo * 8 : grp_hi * 8],
                            num_idxs=ng,
                            num_idxs_reg=nidx_regs[ng],
                            elem_size=D,
                            single_packet=False,
                        )
                        sel = build_sel(grp_lo, grp_hi)
                    jj = j - PIECE_J0[piece]
                    npc = PIECES[piece]
                    pg = jj % PGRP
                    pgn = min(PGRP, npc - (jj - pg))
                    if pg == 0:
                        ps4 = psA.tile([P, PGRP * P], F32, space="PSUM")
                    for t in range(T):
                        ft = tb + t - grp_lo
                        nc.tensor.matmul(
                            out=ps4[:, pg * P : (pg + 1) * P],
                            lhsT=gath[:, ft * P : (ft + 1) * P],
                            rhs=sel_tile_ap(sel, grp_lo, pos, t),
                            start=(t == 0),
                            stop=(t == T - 1),
                        )
                    if pg == pgn - 1:
                        stg = stg_pool.tile([P, PGRP * P], F16, tag="stg")
                        nc.scalar.activation(
                            out=stg[:, 0 : pgn * P], in_=ps4[:, 0 : pgn * P],
                            func=mybir.ActivationFunctionType.Copy,
                        )
                        rs_in = rs_inp[piece]
                        pw = piece_cols[piece]
                        col0 = (j - pg) * P - piece_col0[piece]
                        dest = bass.AP(
                            tensor=rs_in.tensor,
                            offset=rs_in[:].offset + q * P * pw + col0,
                            ap=[[pw, P], [1, pgn * P]],
                        )
                        nc.sync.dma_start(out=dest, in_=stg[:, 0 : pgn * P])
                    if pos in piece_end_pos:
                        piece_id = piece_end_pos[pos]
                        ncols = piece_cols[piece_id]
                        rs_out = dram_rso.tile([P, ncols], F16, tag=f"rso{piece_id}")
                        nc.gpsimd.collective_compute(
                            "ReduceScatter",
                            mybir.AluOpType.add,
                            replica_groups=rg,
                            ins=[rs_inp[piece_id][:].opt()],
                            outs=[rs_out[:]],
                        )
                        rs_outs.append(rs_out)

                # readbacks + dense emitted after the loop: an in-order SP
                # queue must never park a collective-gated DMA ahead of the
                # aggregate staging writes
                for piece_id in range(n_pieces):
                    c0 = piece_col0[piece_id]
                    ncols = piece_cols[piece_id]
                    nc.sync.dma_start(
                        out=agg_sb[:, c0 : c0 + ncols], in_=rs_outs[piece_id][:]
                    )
                for piece in range(n_pieces):
                    emit_dense_piece(piece, l, z_sb, agg_sb, stats)

                # ---- BatchNorm across all nodes (tiny stats AllGather) ----
                bs = bn_pool.tile([P, 16], F32)
                mv = bs[:, 0:2]
                with tc.high_priority():
                    nc.vector.bn_aggr(out=mv, in_=stats[:])
                cc_sb = bs[:, 3:5]
                with tc.high_priority():
                    nc.vector.tensor_copy(out=cc_sb[:, 0:1], in_=mv[:, 0:1])
                    nc.vector.tensor_scalar(
                        out=cc_sb[:, 1:2], in0=mv[:, 0:1], scalar1=mv[:, 0:1],
                        scalar2=mv[:, 1:2], op0=mybir.AluOpType.mult,
                        op1=mybir.AluOpType.add,
                    )
                cc_in = dram_cc.tile([P, 2], F32)
                cc_out = dram_cc.tile([P * N_CORES, 2], F32, addr_space="Shared")
                nc.sync.dma_start(out=cc_in[:], in_=cc_sb)
                nc.gpsimd.collective_compute(
                    "AllGather", mybir.AluOpType.bypass, replica_groups=rg,
                    ins=[cc_in.opt()], outs=[cc_out.opt()],
                )
                cc_all = bn_pool.tile([P, 2, N_CORES], F32)
                cc_src = bass.AP(
                    tensor=cc_out.tensor,
                    offset=cc_out[:].offset,
                    ap=[[2, P], [1, 2], [2 * P, N_CORES]],
                )
                nc.sync.dma_start(out=cc_all[:], in_=cc_src)
                cc_res = bs[:, 5:7]
                nc.vector.tensor_reduce(
                    out=cc_res.rearrange("p (a b) -> p a b", a=2),
                    in_=cc_all[:],
                    axis=mybir.AxisListType.X,
                    op=mybir.AluOpType.add,
                )
                mu = bs[:, 7:8]
                nc.vector.tensor_scalar(
                    out=mu, in0=cc_res[:, 0:1], scalar2=None,
                    op0=mybir.AluOpType.mult, scalar1=1.0 / N_CORES,
                )
                var = bs[:, 8:9]
                nc.vector.tensor_scalar(
                    out=var, in0=cc_res[:, 1:2], scalar2=None,
                    op0=mybir.AluOpType.mult, scalar1=1.0 / N_CORES,
                )
                mu2 = bs[:, 9:10]
                nc.vector.tensor_tensor(
                    out=mu2, in0=mu, in1=mu, op=mybir.AluOpType.mult
                )
                nc.vector.tensor_tensor(
                    out=var, in0=var, in1=mu2, op=mybir.AluOpType.subtract
                )
                rstd = bs[:, 10:11]
                nc.scalar.activation(
                    out=rstd, in_=var,
                    func=mybir.ActivationFunctionType.Sqrt,
                    bias=eps_sb[:], scale=1.0,
                )
                nc.vector.reciprocal(out=rstd, in_=rstd)
                scale = bs[:, 11:12]
                nc.vector.tensor_tensor(
                    out=scale, in0=rstd, in1=gammaT_sb[:, l : l + 1],
                    op=mybir.AluOpType.mult,
                )
                shift = bs[:, 12:13]
                nc.vector.tensor_tensor(
                    out=shift, in0=mu, in1=scale, op=mybir.AluOpType.mult
                )
                nc.vector.tensor_tensor(
                    out=shift, in0=betaT_sb[:, l : l + 1], in1=shift,
                    op=mybir.AluOpType.subtract,
                )

                # BN apply + relu, zero pad slots, rebuild node-major table
                hT_new = hT_pool.tile([P, SLOTS], F16, tag="hT")
                h_tab_new = dram_tab.tile([SLOTS, D], F16, tag="htab")
                for c0 in range(0, CHUNKS, AGRP):
                    ng = min(AGRP, CHUNKS - c0)
                    gs = slice(c0 * P, (c0 + ng) * P)
                    nc.scalar.activation(
                        out=hT_new[:, gs], in_=z_sb[:, gs],
                        func=mybir.ActivationFunctionType.Relu,
                        bias=shift, scale=scale,
                    )
                    if c0 + ng == CHUNKS:
                        nc.vector.memset(hT_new[:, QUOTA:SLOTS], 0.0)
                    t16g = t16_pool.tile([P, AGRP, P], F16)
                    for k0 in range(0, ng, PGRP):
                        kn = min(PGRP, ng - k0)
                        ps_t = psT.tile([P, PGRP * P], F16, space="PSUM")
                        for k in range(k0, k0 + kn):
                            c = c0 + k
                            cs2 = slice(c * P, (c + 1) * P)
                            nc.tensor.transpose(
                                out=ps_t[:, (k - k0) * P : (k - k0 + 1) * P],
                                in_=hT_new[:, cs2], identity=ident_sb[:],
                            )
                        nc.vector.tensor_copy(
                            out=t16g[:, k0 : k0 + kn, :], in_=ps_t[:, 0 : kn * P]
                        )
                    dest = bass.AP(
                        tensor=h_tab_new.tensor,
                        offset=h_tab_new[:].offset + c0 * P * D,
                        ap=[[D, P], [P * D, ng], [1, D]],
                    )
                    nc.sync.dma_start(out=dest, in_=t16g[:, 0:ng, :])
                hT_prev = hT_new
                h_tab = h_tab_new

            # ---------------- final GraphConv (OUT=2) ----------------
            # transpose-mode gather -> gathT [feat, edge]; PE projection
            # through Wrel2 -> proj [edge, 2]; segment-sum emits
            # feature-major partials [2, slots] incl. masked root + bias.
            rs_in_f = dram_rsi.tile([N_PAD, OUT], F16, tag="rsif")
            sel = None
            proj_sb = None
            grp_lo = 0
            psf = None
            for pos in range(NBINS):
                piece, q, j = pos_info[pos]
                T = t_proc[pos]
                tb = tile_base[pos]
                if tb in grp_start:
                    grp_lo, grp_hi = grp_start[tb]
                    ng = (grp_hi - grp_lo) * P
                    nt = grp_hi - grp_lo
                    gathT = g_pool.tile([P, max_grp_t * P], F16, tag="gath")
                    gtb = gathT[:]
                    nc.gpsimd.dma_gather(
                        out_ap=bass.AP(
                            tensor=gtb.tensor,
                            offset=gtb.offset,
                            ap=[gtb.ap[0], [ng, 1], [1, ng]],
                        ),
                        in_ap=h_tab[:],
                        idxs_ap=idx_sb[:, grp_lo * 8 : grp_hi * 8],
                        num_idxs=ng,
                        num_idxs_reg=nidx_regs[ng],
                        elem_size=D,
                        transpose=True,
                        single_packet=False,
                    )
                    psp = psP.tile([P, max_grp_t * OUT], F32, space="PSUM", tag="psp")
                    for t in range(nt):
                        nc.tensor.matmul(
                            out=psp[:, t * OUT : (t + 1) * OUT],
                            lhsT=gathT[:, t * P : (t + 1) * P],
                            rhs=wrel2_sb[:],
                            start=True, stop=True,
                        )
                    proj_sb = stg_pool.tile([P, max_grp_t * OUT], F16, tag="proj")
                    nc.scalar.activation(
                        out=proj_sb[:, 0 : nt * OUT], in_=psp[:, 0 : nt * OUT],
                        func=mybir.ActivationFunctionType.Copy,
                    )
                    sel = build_sel(grp_lo, grp_hi)
                jj = j - PIECE_J0[piece]
                npc = PIECES[piece]
                pg = jj % PGRP
                pgn = min(PGRP, npc - (jj - pg))
                if pg == 0:
                    psf = psF.tile([P, PGRP * OUT], F32, space="PSUM", tag="psf")
                fo = slice(pg * OUT, (pg + 1) * OUT)
                for t in range(T):
                    nc.tensor.matmul(
                        out=psf[:, fo],
                        lhsT=sel_tile_ap(sel, grp_lo, pos, t),
                        rhs=proj_sb[:, (tb + t - grp_lo) * OUT : (tb + t - grp_lo + 1) * OUT],
                        start=(t == 0),
                        stop=False,
                    )
                # root + bias: nonzero only on the rank that owns these slots
                nc.tensor.matmul(
                    out=psf[:, fo],
                    lhsT=hT_prev[:, j * P : (j + 1) * P],
                    rhs=wroot2m_sb[:, q * OUT : (q + 1) * OUT],
                    start=False,
                    stop=False,
                )
                nc.tensor.matmul(
                    out=psf[:, fo],
                    lhsT=ones_sb[:],
                    rhs=b2m_sb[:, q * OUT : (q + 1) * OUT],
                    start=False,
                    stop=True,
                )
                if pg == pgn - 1:
                    stgf = stg_pool.tile([P, PGRP * OUT], F16, tag="stgf")
                    nc.scalar.activation(
                        out=stgf[:, 0 : pgn * OUT], in_=psf[:, 0 : pgn * OUT],
                        func=mybir.ActivationFunctionType.Copy,
                    )
                    r0 = q * SLOTS + (j - pg) * P
                    dest = bass.AP(
                        tensor=rs_in_f.tensor,
                        offset=rs_in_f[:].offset + r0 * OUT,
                        ap=[[OUT, P], [P * OUT, pgn], [1, OUT]],
                    )
                    nc.sync.dma_start(out=dest, in_=stgf[:, 0 : pgn * OUT])

            rs_out_f = dram_rso.tile([SLOTS, OUT], F16, tag="rsof")
            nc.gpsimd.collective_compute(
                "ReduceScatter",
                mybir.AluOpType.add,
                replica_groups=rg,
                ins=[rs_in_f[:].opt()],
                outs=[rs_out_f[:]],
            )
            nc.sync.dma_start(out=p_out[:], in_=rs_out_f[:])

    lower_extended_insts(nc)
    _split_multiwait(nc)
    return nc


_PROGRAM_CACHE = {}


def _get_program(meta):
    key = hashlib.sha1(repr(sorted(meta.items())).encode()).hexdigest()
    if key not in _PROGRAM_CACHE:
        _PROGRAM_CACHE[key] = build_program(meta)
    return _PROGRAM_CACHE[key]


def _make_in_maps(idx_cores, dst_cores, x_loc, xT_loc,
                  Wrel, Wroot, gamma, beta, Wrel2, Wroot2, b2):
    iotar = np.zeros((P, P, RMAX), np.float16)
    iotar[:, :, :] = np.arange(P, dtype=np.float16)[None, :, None]
    ident16 = np.eye(P, dtype=np.float16)
    common = dict(
        wrel=np.ascontiguousarray(np.asarray(Wrel, np.float32)),
        wroot=np.ascontiguousarray(np.asarray(Wroot, np.float32)),
        wrel2=np.ascontiguousarray(np.asarray(Wrel2, np.float32)),
        gammaT=np.ascontiguousarray(np.asarray(gamma, np.float32).T),
        betaT=np.ascontiguousarray(np.asarray(beta, np.float32).T),
        iotar=np.ascontiguousarray(iotar.reshape(P, P * RMAX)),
        ident16=ident16,
    )
    wroot2 = np.asarray(Wroot2, np.float16)                 # [D, OUT]
    b2 = np.asarray(b2, np.float16).reshape(1, OUT)
    in_maps = []
    for c in range(N_CORES):
        w2m = np.zeros((D, N_CORES, OUT), np.float16)
        w2m[:, c, :] = wroot2
        b2m = np.zeros((1, N_CORES, OUT), np.float16)
        b2m[:, c, :] = b2
        m = dict(common)
        m["x_loc"] = x_loc[c]
        m["xT_loc"] = xT_loc[c]
        m["gidx"] = idx_cores[c]
        m["dst_loc"] = dst_cores[c]
        m["wroot2m"] = np.ascontiguousarray(w2m.reshape(D, N_CORES * OUT))
        m["b2m"] = np.ascontiguousarray(b2m.reshape(1, N_CORES * OUT))
        in_maps.append(m)
    return in_maps


def run(x, edge_index, Wrel, Wroot, b, gamma, beta, Wrel2, Wroot2, b2):
    """Returns (output [N, OUT] float32, nc, meta) - nc exposed for profiling.
    The per-layer GraphConv bias b cancels inside BatchNorm and is unused."""
    meta, newid, idx_cores, dst_cores, x_loc, xT_loc = _preprocess(x, edge_index)
    nc = _get_program(meta)
    in_maps = _make_in_maps(
        idx_cores, dst_cores, x_loc, xT_loc,
        Wrel, Wroot, gamma, beta, Wrel2, Wroot2, b2,
    )
    from concourse.bass_utils import run_bass_kernel_spmd

    res = run_bass_kernel_spmd(nc, in_maps, list(range(N_CORES)))
    full = np.concatenate(
        [res.results[c]["z4"] for c in range(N_CORES)], axis=0
    )  # [N_PAD, OUT]
    return full[newid].astype(np.float32), nc, meta


def kernel(**inputs):
    out, _, _ = run(**{k: np.asarray(v) for k, v in inputs.items()})
    return out


# revision 20
# speedup vs baseline: 1.0854x; 1.0000x over previous
"""GNN message-passing (3x GraphConv+BN+ReLU, final GraphConv) on 8 trn2 cores.

Source-sharded graph parallelism:
  - Nodes are partitioned across 8 cores (6272 slots each, 49 chunks of 128).
    Each core processes the edges whose SOURCE it owns, so per-edge feature
    gathers read a small local fp16 table (6272 rows, int16 indices).
  - Per layer: indirect-DMA gather of the core's edge source rows, one-hot
    matmuls accumulate partial aggregates for ALL 392 destination chunks in
    PSUM, partials stream to a DRAM buffer, and a ReduceScatter (split in
    three pieces, overlapped with the gather phase) reduces them onto the
    destination owner.  Dense transforms + BatchNorm stats/apply are local;
    only a tiny [128,2] stats AllGather crosses cores per layer.
  - One-hot masks are built in a [edge, dst, tile] layout so every DVE
    operand is packed 2-byte (2x DVE mode); tiles read them back with a
    strided matmul AP.
  - The GraphConv bias cancels inside BatchNorm and is skipped; every core
    holds exactly 6250 real nodes with its 22 pad slots pinned to the tail
    of chunk 48, so BN stats are exact and pads are re-zeroed by one memset.
  - Final layer: transpose-mode gather delivers gathT [feat, edge] tiles of
    h3, a per-tile PE projection through Wrel2 gives 2-col edge values, and
    the segment-sum emits feature-major partials; Wroot2+b2 ride along as
    per-rank masked matmuls so the final ReduceScatter yields the output.
"""

import hashlib
import heapq
import sys

import numpy as np

sys.path.insert(0, "/opt/trn_rl_repo")

import concourse.bass as bass  # noqa: E402
import concourse.mybir as mybir  # noqa: E402
import concourse.tile as tile  # noqa: E402
from concourse.vector_clock import ScopedClock  # noqa: E402
from concourse import library_config  # noqa: E402
from concourse.library_overlay import lower_extended_insts  # noqa: E402

N = 50000
E = 800000
D = 128
L = 3
OUT = 2
EPS = 1e-5
N_CORES = 8
P = 128
CHUNKS = 49                 # local dst chunks per core
SLOTS = CHUNKS * P          # 6272
NBINS = N_CORES * CHUNKS    # 392 global dst chunks
N_PAD = N_CORES * SLOTS     # 50176
QUOTA = N // N_CORES        # 6250 real nodes per core
SHORT = QUOTA - 48 * P      # 106 real slots in chunk 48
PIECES = (25, 18, 6)        # local chunks per RS piece
PIECE_J0 = (0, 25, 43)
GGRPS = ((13, 12), (9, 9), (6,))   # gather-group sizes per piece
PGRP = 4                    # chunks per PSUM bank / staging DMA group
RMAX = 36                   # max tiles covered by one sel build (12 chunks x T3)
AGRP = 25                   # chunks per BN-apply / table-write group

F16 = mybir.dt.float16
F32 = mybir.dt.float32

# ---------------------------------------------------------------------------
# walrus in this container accepts at most ONE semaphore wait per instruction.
# Patch the Tile exit drain and add a post-pass splitting multi-wait insts.
# ---------------------------------------------------------------------------
_MAX_WAITS = 1


def _drain_and_barrier(self, tick_clock, wait_clock):
    nc = self.nc
    drain_inst = nc.sync.drain()
    wait_clock.add_sem_waits(
        drain_inst.ins, ScopedClock({None: tick_clock.global_clock})
    )
    si = drain_inst.ins.sync_info
    if si is not None and si.on_wait is not None and len(si.on_wait) > _MAX_WAITS:
        waits = list(si.on_wait)
        si.on_wait = waits[:_MAX_WAITS]
        rest = waits[_MAX_WAITS:]
        for i in range(0, len(rest), _MAX_WAITS):
            nop = nc.sync.nop(nofuse=True)
            nop.ins.sync_info = mybir.SyncInfo(
                on_wait=rest[i : i + _MAX_WAITS], on_update=[]
            )
    nc.all_engine_barrier()
    assert self.sems is not None
    popped = nc._tile_sem_poison_stack.pop()
    assert popped is self._sem_poison
    nc.clear_and_free_semaphores(list(self.sems.allocated().values()))
    nc.all_engine_barrier()


tile.TileContext._drain_and_barrier = _drain_and_barrier


def _split_multiwait(nc):
    n_split = 0
    for fn in nc.m.functions:
        for blk in fn.blocks:
            out = []
            for inst in blk.instructions:
                si = inst.sync_info
                if si is not None and si.on_wait and len(si.on_wait) > _MAX_WAITS:
                    waits = list(si.on_wait)
                    si.on_wait = waits[-_MAX_WAITS:]
                    rest = waits[:-_MAX_WAITS]
                    for i in range(0, len(rest), _MAX_WAITS):
                        n_split += 1
                        out.append(
                            mybir.InstNoOp(
                                name=f"{inst.name}-ws{i}",
                                engine=inst.engine,
                                ins=[],
                                outs=[],
                                bass_nofuse=True,
                                sync_info=mybir.SyncInfo(
                                    on_wait=rest[i : i + _MAX_WAITS], on_update=[]
                                ),
                                debug=inst.debug,
                            )
                        )
                out.append(inst)
            blk.instructions[:] = out
    return n_split


# ---------------------------------------------------------------------------
# Host-side graph partitioning
# ---------------------------------------------------------------------------
def _lpt(nodes, deg_in, bins, caps, bin_of, slot_of, fill):
    heap = [(0, b) for b in bins]
    heapq.heapify(heap)
    for node in nodes:
        d = int(deg_in[node])
        ld, b = heapq.heappop(heap)
        bin_of[node] = b
        slot_of[node] = fill[b]
        fill[b] += 1
        if fill[b] < caps[b]:
            heapq.heappush(heap, (ld + d, b))


def _partition_nodes(deg_in):
    """Assign nodes to (bin, slot): bin b -> core b%8, local chunk b//8.
    Every bin is filled exactly to its cap (128, or 106 for chunk 48), so
    each core holds exactly 6250 real nodes and pads sit at the tail of
    chunk 48.  The heaviest nodes fill a set of "heavy" bins; the rest are
    LPT'd over "light" bins so per-(core,bin) edge counts pack tightly."""
    caps = np.full(NBINS, P, np.int64)
    caps[48 * N_CORES :] = SHORT                 # bins (q, j=48)
    order = np.argsort(-deg_in, kind="stable")
    sdeg = deg_in[order].astype(np.float64)
    pref = np.concatenate([[0.0], np.cumsum(sdeg)])
    total = pref[-1]
    best = (None, None)
    for nh in range(0, 200, 8):
        nl = NBINS - nh
        s_h = pref[min(nh * P, N)]
        m_h = s_h / max(nh, 1) / N_CORES
        m_l = (total - s_h) / nl / N_CORES
        t_h = int(np.ceil((m_h + 3.0 * np.sqrt(m_h * 0.875 + 1)) / P)) if nh else 0
        t_l = int(np.ceil((m_l + 3.0 * np.sqrt(m_l * 0.875 + 1)) / P))
        st = nh * max(t_h, 1) + nl * t_l
        if best[0] is None or st < best[0]:
            best = (st, nh)
    n_heavy = best[1]
    j_cut = CHUNKS - n_heavy // N_CORES
    allb = np.arange(NBINS)
    heavy_bins = allb[allb // N_CORES >= j_cut]
    light_bins = allb[allb // N_CORES < j_cut]

    bin_of = np.empty(N, np.int32)
    slot_of = np.empty(N, np.int32)
    fill = np.zeros(NBINS, np.int64)
    nh_nodes = int(caps[heavy_bins].sum())
    _lpt(order[:nh_nodes], deg_in, heavy_bins, caps, bin_of, slot_of, fill)
    _lpt(order[nh_nodes:], deg_in, light_bins, caps, bin_of, slot_of, fill)
    assert (fill == caps).all()
    return bin_of, slot_of


def _preprocess(x, edge_index):
    x = np.asarray(x, np.float32)
    ei = np.asarray(edge_index)
    src = ei[0].astype(np.int64)
    dst = ei[1].astype(np.int64)
    deg_in = np.bincount(dst, minlength=N)
    bin_of, slot_of = _partition_nodes(deg_in)

    core_of = bin_of % N_CORES
    newid = (
        core_of.astype(np.int64) * SLOTS
        + (bin_of // N_CORES).astype(np.int64) * P
        + slot_of
    )

    e_core = core_of[src]
    e_bin = bin_of[dst]

    cnt = np.zeros((N_CORES, NBINS), np.int64)
    np.add.at(cnt, (e_core, e_bin), 1)
    t_bin = np.maximum(1, -(-cnt.max(axis=0) // P))

    # chunk processing order: per piece, (q, j) with j in the piece range
    ordered_bins = []
    for piece in range(len(PIECES)):
        jr = range(PIECE_J0[piece], PIECE_J0[piece] + PIECES[piece])
        for q in range(N_CORES):
            for j in jr:
                ordered_bins.append(j * N_CORES + q)
    ordered_bins = np.array(ordered_bins)
    bin_pos = np.empty(NBINS, np.int64)
    bin_pos[ordered_bins] = np.arange(NBINS)
    t_proc = t_bin[ordered_bins]
    tile_base = np.concatenate([[0], np.cumsum(t_proc)[:-1]])
    SUM_T = int(t_proc.sum())

    e_pos = bin_pos[e_bin]
    order = np.lexsort((e_pos, e_core))
    s_core = e_core[order]
    s_pos = e_pos[order]
    s_srcslot = (newid[src[order]] % SLOTS).astype(np.int64)
    s_dstslot = slot_of[dst[order]].astype(np.int64)

    bucket = s_core * NBINS + s_pos
    bnd = np.concatenate(
        [[0], np.cumsum(np.bincount(bucket, minlength=N_CORES * NBINS))]
    )
    within = np.arange(E) - bnd[bucket]
    assert (within < t_proc[s_pos] * P).all()

    flat_off = tile_base * P
    e_slot = s_core * (SUM_T * P) + flat_off[s_pos] + within

    gidx = np.zeros(N_CORES * SUM_T * P, np.int16)
    dloc = np.full(N_CORES * SUM_T * P, -1.0, np.float16)
    gidx[e_slot] = s_srcslot.astype(np.int16)
    dloc[e_slot] = s_dstslot.astype(np.float16)
    gidx = gidx.reshape(N_CORES, SUM_T, P)
    dloc = dloc.reshape(N_CORES, SUM_T, P)

    dst_cores = np.ascontiguousarray(dloc.transpose(0, 2, 1))  # [c, 128, SUM_T]

    # gather groups per (piece, q): fixed chunk-count splits
    groups = []
    pos = 0
    for piece in range(len(PIECES)):
        for q in range(N_CORES):
            c0 = 0
            for gsz in GGRPS[piece]:
                lo = tile_base[pos + c0]
                last = pos + c0 + gsz - 1
                hi = tile_base[last] + t_proc[last]
                groups.append((int(lo), int(hi)))
                c0 += gsz
            assert c0 == PIECES[piece]
            pos += PIECES[piece]

    blocks = []
    for (lo, hi) in groups:
        n = (hi - lo) * P
        w = gidx[:, lo:hi, :].reshape(N_CORES, n // 16, 16).transpose(0, 2, 1)
        blocks.append(w)
    idxw = np.concatenate(blocks, axis=2)
    I_COLS = idxw.shape[2]
    idx_cores = np.ascontiguousarray(
        np.broadcast_to(idxw[:, None, :, :], (N_CORES, 8, 16, I_COLS)).reshape(
            N_CORES, P, I_COLS
        )
    )

    x_pad = np.zeros((N_PAD, D), np.float32)
    x_pad[newid] = x
    x_loc = np.ascontiguousarray(x_pad.reshape(N_CORES, SLOTS, D).astype(np.float16))
    xT_loc = np.ascontiguousarray(x_loc.transpose(0, 2, 1))
    meta = dict(
        SUM_T=SUM_T,
        t_proc=tuple(int(t) for t in t_proc),
        groups=tuple(groups),
        I_COLS=int(I_COLS),
    )
    return meta, newid, idx_cores, dst_cores, x_loc, xT_loc


# ---------------------------------------------------------------------------
# Device program
# ---------------------------------------------------------------------------
def build_program(meta):
    SUM_T = meta["SUM_T"]
    t_proc = meta["t_proc"]
    groups = meta["groups"]
    I_COLS = meta["I_COLS"]
    tile_base = [0]
    for t in t_proc[:-1]:
        tile_base.append(tile_base[-1] + t)

    nc = bass.Bass(num_devices=N_CORES)

    p_xloc = nc.declare_dram_parameter("x_loc", [SLOTS, D], F16, isOutput=False)
    p_xT = nc.declare_dram_parameter("xT_loc", [D, SLOTS], F16, isOutput=False)
    p_idx = nc.declare_dram_parameter("gidx", [P, I_COLS], mybir.dt.int16, isOutput=False)
    p_dst = nc.declare_dram_parameter("dst_loc", [P, SUM_T], F16, isOutput=False)
    p_wrel = nc.declare_dram_parameter("wrel", [L, D, D], F32, isOutput=False)
    p_wroot = nc.declare_dram_parameter("wroot", [L, D, D], F32, isOutput=False)
    p_wrel2 = nc.declare_dram_parameter("wrel2", [D, OUT], F32, isOutput=False)
    p_wroot2m = nc.declare_dram_parameter(
        "wroot2m", [D, N_CORES * OUT], F16, isOutput=False
    )
    p_b2m = nc.declare_dram_parameter("b2m", [1, N_CORES * OUT], F16, isOutput=False)
    p_gammaT = nc.declare_dram_parameter("gammaT", [D, L], F32, isOutput=False)
    p_betaT = nc.declare_dram_parameter("betaT", [D, L], F32, isOutput=False)
    p_iotar = nc.declare_dram_parameter("iotar", [P, P * RMAX], F16, isOutput=False)
    p_ident = nc.declare_dram_parameter("ident16", [P, P], F16, isOutput=False)
    p_out = nc.declare_dram_parameter("z4", [SLOTS, OUT], F16, isOutput=True)

    rg = [list(range(N_CORES))]
    n_pieces = len(PIECES)
    piece_cols = tuple(p * P for p in PIECES)
    piece_col0 = tuple(j * P for j in PIECE_J0)

    pos_info = []
    for piece in range(n_pieces):
        jr = range(PIECE_J0[piece], PIECE_J0[piece] + PIECES[piece])
        for q in range(N_CORES):
            for j in jr:
                pos_info.append((piece, q, j))
    piece_end_pos = {}
    acc = 0
    for piece in range(n_pieces):
        acc += PIECES[piece] * N_CORES
        piece_end_pos[acc - 1] = piece

    grp_start = {lo: (lo, hi) for (lo, hi) in groups}
    max_grp_t = max(hi - lo for (lo, hi) in groups)

    # same-T runs of chunks within each gather group, for packed sel builds
    pos_of_tb = {tile_base[pos]: pos for pos in range(NBINS)}
    group_runs = {}     # grp_lo -> list of (run_tb, nc_chunks, T)
    for (lo, hi) in groups:
        runs = []
        pos = pos_of_tb[lo]
        tb = lo
        while tb < hi:
            T = t_proc[pos]
            ncr = 0
            rtb = tb
            while tb < hi and t_proc[pos] == T:
                ncr += 1
                tb += T
                pos += 1
            runs.append((rtb, ncr, T))
        group_runs[lo] = runs

    from contextlib import ExitStack

    with tile.TileContext(nc) as tc:
        with ExitStack() as stack:
            ep = stack.enter_context
            dram_tab = ep(tc.tile_pool(name="dram_tab", bufs=2, space="DRAM"))
            dram_rsi = ep(tc.tile_pool(name="dram_rsi", bufs=2, space="DRAM"))
            dram_rso = ep(tc.tile_pool(name="dram_rso", bufs=2, space="DRAM"))
            dram_cc = ep(tc.tile_pool(name="dram_cc", bufs=2, space="DRAM"))
            singles = ep(tc.tile_pool(name="singles", bufs=1))
            hT_pool = ep(tc.tile_pool(name="hT", bufs=2))
            z_pool = ep(tc.tile_pool(name="zb", bufs=1))
            agg_pool = ep(tc.tile_pool(name="aggb", bufs=1))
            g_pool = ep(tc.tile_pool(name="gath", bufs=3))
            s_pool = ep(tc.tile_pool(name="sel", bufs=3))
            stg_pool = ep(tc.tile_pool(name="stg", bufs=3))
            t16_pool = ep(tc.tile_pool(name="t16p", bufs=2))
            bn_pool = ep(tc.tile_pool(name="bns", bufs=2))
            stat_pool = ep(tc.tile_pool(name="stat", bufs=2))
            psA = ep(tc.tile_pool(name="psA", bufs=3, space="PSUM"))
            psZ = ep(tc.tile_pool(name="psZ", bufs=1, space="PSUM"))
            psT = ep(tc.tile_pool(name="psT", bufs=2, space="PSUM"))
            psF = ep(tc.tile_pool(name="psF", bufs=1, space="PSUM"))
            psP = ep(tc.tile_pool(name="psP", bufs=1, space="PSUM"))

            with tc.high_priority():
                nc.gpsimd.load_library(library_config.mlp)

            grp_sizes = sorted({(hi - lo) * P for (lo, hi) in groups})
            nidx_regs = {n: nc.gpsimd.to_reg(n) for n in grp_sizes}

            # --- constants / weights in SBUF ---
            idx_sb = singles.tile([P, I_COLS], mybir.dt.int16)
            for i0 in range(0, I_COLS, (I_COLS + 3) // 4):
                i1 = min(I_COLS, i0 + (I_COLS + 3) // 4)
                nc.sync.dma_start(out=idx_sb[:, i0:i1], in_=p_idx[:, i0:i1])
            dst_sb = singles.tile([P, SUM_T], F16)
            nc.sync.dma_start(out=dst_sb[:], in_=p_dst[:])
            iotar_sb = singles.tile([P, P * RMAX], F16)
            nc.sync.dma_start(out=iotar_sb[:], in_=p_iotar[:])
            ident_sb = singles.tile([P, P], F16)
            nc.sync.dma_start(out=ident_sb[:], in_=p_ident[:])
            wtmp = singles.tile([P, D], F32)
            wrel_sb = singles.tile([P, L * D], F16)
            wroot_sb = singles.tile([P, L * D], F16)
            for l in range(L):
                nc.sync.dma_start(out=wtmp[:], in_=p_wrel[l])
                nc.scalar.activation(
                    out=wrel_sb[:, l * D : (l + 1) * D], in_=wtmp[:],
                    func=mybir.ActivationFunctionType.Copy,
                )
                nc.sync.dma_start(out=wtmp[:], in_=p_wroot[l])
                nc.scalar.activation(
                    out=wroot_sb[:, l * D : (l + 1) * D], in_=wtmp[:],
                    func=mybir.ActivationFunctionType.Copy,
                )
            wrel2_sb = singles.tile([P, OUT], F16)
            nc.sync.dma_start(out=wtmp[:, 0:OUT], in_=p_wrel2[:])
            nc.scalar.activation(
                out=wrel2_sb[:], in_=wtmp[:, 0:OUT],
                func=mybir.ActivationFunctionType.Copy,
            )
            wroot2m_sb = singles.tile([P, N_CORES * OUT], F16)
            nc.sync.dma_start(out=wroot2m_sb[:], in_=p_wroot2m[:])
            b2m_sb = singles.tile([1, N_CORES * OUT], F16)
            nc.sync.dma_start(out=b2m_sb[:], in_=p_b2m[:])
            gammaT_sb = singles.tile([P, L], F32)
            nc.sync.dma_start(out=gammaT_sb[:], in_=p_gammaT[:])
            betaT_sb = singles.tile([P, L], F32)
            nc.sync.dma_start(out=betaT_sb[:], in_=p_betaT[:])
            ones_sb = singles.tile([1, P], F16)
            nc.vector.memset(ones_sb[:], 1.0)
            eps_sb = singles.tile([P, 1], F32)
            nc.vector.memset(eps_sb[:], EPS)

            hT_prev = hT_pool.tile([P, SLOTS], F16, tag="hT")
            nc.sync.dma_start(out=hT_prev[:], in_=p_xT[:])
            h_tab = p_xloc

            def build_sel(grp_lo, grp_hi):
                """One-hot masks for the group's tiles in [e, dst, tile]
                layout: all DVE operands packed 2-byte -> 2x mode."""
                st = s_pool.tile([P, max_grp_t * P], F16, tag="sel")
                sb = st[:]
                db = dst_sb[:]
                ib = iotar_sb[:]
                for (rtb, ncr, T) in group_runs[grp_lo]:
                    nct = ncr * T
                    base = (rtb - grp_lo) * P
                    nc.vector.tensor_tensor(
                        out=bass.AP(
                            tensor=sb.tensor, offset=sb.offset + base,
                            ap=[sb.ap[0], [nct, P], [1, nct]],
                        ),
                        in0=bass.AP(
                            tensor=db.tensor, offset=db.offset + rtb,
                            ap=[db.ap[0], [0, P], [1, nct]],
                        ),
                        in1=bass.AP(
                            tensor=ib.tensor, offset=ib.offset,
                            ap=[ib.ap[0], [RMAX, P], [1, nct]],
                        ),
                        op=mybir.AluOpType.is_equal,
                    )
                return st

            def sel_tile_ap(st, grp_lo, pos, t):
                """Matmul operand AP for (chunk at pos, tile t): [e, 128 dst]
                with dst stride = the run's nc*T."""
                for (rtb, ncr, T) in group_runs[grp_lo]:
                    if rtb <= tile_base[pos] < rtb + ncr * T:
                        nct = ncr * T
                        base = (rtb - grp_lo) * P
                        col = tile_base[pos] - rtb + t
                        sb = st[:]
                        return bass.AP(
                            tensor=sb.tensor, offset=sb.offset + base + col,
                            ap=[sb.ap[0], [nct, P]],
                        )
                raise AssertionError("tile not in any run")

            def emit_dense_piece(piece, l, z_sb, agg_sb, stats):
                w_rel = wrel_sb[:, l * D : (l + 1) * D]
                w_root = wroot_sb[:, l * D : (l + 1) * D]
                j0, npc = PIECE_J0[piece], PIECES[piece]
                for jg in range(j0, j0 + npc, PGRP):
                    jn = min(PGRP, j0 + npc - jg)
                    psz = psZ.tile([P, PGRP * P], F32, space="PSUM")
                    for k in range(jn):
                        cs = slice((jg + k) * P, (jg + k + 1) * P)
                        ks = slice(k * P, (k + 1) * P)
                        nc.tensor.matmul(
                            out=psz[:, ks], lhsT=w_rel, rhs=agg_sb[:, cs],
                            start=True, stop=False,
                        )
                        nc.tensor.matmul(
                            out=psz[:, ks], lhsT=w_root, rhs=hT_prev[:, cs],
                            start=False, stop=True,
                        )
                    zs = slice(jg * P, (jg + jn) * P)
                    nc.scalar.activation(
                        out=z_sb[:, zs], in_=psz[:, 0 : jn * P],
                        func=mybir.ActivationFunctionType.Copy,
                    )
                    for k in range(jn):
                        j = jg + k
                        width = SHORT if j == 48 else P
                        nc.vector.bn_stats(
                            out=stats[:, j, :],
                            in_=z_sb[:, j * P : j * P + width],
                        )

            for l in range(L):
                z_sb = z_pool.tile([P, SLOTS], F16)
                agg_sb = agg_pool.tile([P, SLOTS], F16)
                stats = stat_pool.tile([P, CHUNKS, nc.vector.BN_STATS_DIM], F32)
                rs_inp = []
                for piece in range(n_pieces):
                    rst = dram_rsi.tile(
                        [N_CORES * P, piece_cols[piece]], F16, tag=f"rsi{piece}"
                    )
                    rs_inp.append(rst)

                gath = sel = None
                grp_lo = 0
                ps4 = None
                rs_outs = []
                gi = 0
                n_grp_p = [len(GGRPS[p]) * N_CORES for p in range(n_pieces)]
                rs_emit_at = {
                    sum(n_grp_p[: p + 1]) + 4: p for p in range(n_pieces - 1)
                }

                def emit_rs(piece_id):
                    ncols = piece_cols[piece_id]
                    rs_out = dram_rso.tile([P, ncols], F16, tag=f"rso{piece_id}")
                    nc.gpsimd.collective_compute(
                        "ReduceScatter",
                        mybir.AluOpType.add,
                        replica_groups=rg,
                        ins=[rs_inp[piece_id][:].opt()],
                        outs=[rs_out[:]],
                    )
                    rs_outs.append(rs_out)

                for pos in range(NBINS):
                    piece, q, j = pos_info[pos]
                    T = t_proc[pos]
                    tb = tile_base[pos]
                    if tb in grp_start:
                        if gi in rs_emit_at:
                            emit_rs(rs_emit_at[gi])
                        gi += 1
                        grp_lo, grp_hi = grp_start[tb]
                        ng = (grp_hi - grp_lo) * P
                        gath = g_pool.tile([P, max_grp_t * P], F16, tag="gath")
                        gv = gath.rearrange("p (t d) -> p t d", t=max_grp_t)
                        nc.gpsimd.dma_gather(
                            out_ap=gv[:, 0 : grp_hi - grp_lo, :],
                            in_ap=h_tab[:],
                            idxs_ap=idx_sb[:, grp_lo * 8 : grp_hi * 8],
                            num_idxs=ng,
                            num_idxs_reg=nidx_regs[ng],
                            elem_size=D,
                            single_packet=False,
                        )
                        sel = build_sel(grp_lo, grp_hi)
                    jj = j - PIECE_J0[piece]
                    npc = PIECES[piece]
                    pg = jj % PGRP
                    pgn = min(PGRP, npc - (jj - pg))
                    if pg == 0:
                        ps4 = psA.tile([P, PGRP * P], F32, space="PSUM")
                    for t in range(T):
                        ft = tb + t - grp_lo
                        nc.tensor.matmul(
                            out=ps4[:, pg * P : (pg + 1) * P],
                            lhsT=gath[:, ft * P : (ft + 1) * P],
                            rhs=sel_tile_ap(sel, grp_lo, pos, t),
                            start=(t == 0),
                            stop=(t == T - 1),
                        )
                    if pg == pgn - 1:
                        stg = stg_pool.tile([P, PGRP * P], F16, tag="stg")
                        nc.scalar.activation(
                            out=stg[:, 0 : pgn * P], in_=ps4[:, 0 : pgn * P],
                            func=mybir.ActivationFunctionType.Copy,
                        )
                        rs_in = rs_inp[piece]
                        pw = piece_cols[piece]
                        col0 = (j - pg) * P - piece_col0[piece]
                        dest = bass.AP(
                            tensor=rs_in.tensor,
                            offset=rs_in[:].offset + q * P * pw + col0,
                            ap=[[pw, P], [1, pgn * P]],
                        )
                        nc.sync.dma_start(out=dest, in_=stg[:, 0 : pgn * P])
                emit_rs(n_pieces - 1)

                # readbacks + dense emitted after the loop: an in-order SP
                # queue must never park a collective-gated DMA ahead of the
                # aggregate staging writes
                for piece_id in range(n_pieces):
                    c0 = piece_col0[piece_id]
                    ncols = piece_cols[piece_id]
                    nc.sync.dma_start(
                        out=agg_sb[:, c0 : c0 + ncols], in_=rs_outs[piece_id][:]
                    )
                for piece in range(n_pieces):
                    emit_dense_piece(piece, l, z_sb, agg_sb, stats)

                # ---- BatchNorm across all nodes (tiny stats AllGather) ----
                bs = bn_pool.tile([P, 16], F32)
                mv = bs[:, 0:2]
                with tc.high_priority():
                    nc.vector.bn_aggr(out=mv, in_=stats[:])
                cc_sb = bs[:, 3:5]
                with tc.high_priority():
                    nc.vector.tensor_copy(out=cc_sb[:, 0:1], in_=mv[:, 0:1])
                    nc.vector.tensor_scalar(
                        out=cc_sb[:, 1:2], in0=mv[:, 0:1], scalar1=mv[:, 0:1],
                        scalar2=mv[:, 1:2], op0=mybir.AluOpType.mult,
                        op1=mybir.AluOpType.add,
                    )
                cc_in = dram_cc.tile([P, 2], F32)
                cc_out = dram_cc.tile([P * N_CORES, 2], F32, addr_space="Shared")
                nc.sync.dma_start(out=cc_in[:], in_=cc_sb)
                nc.gpsimd.collective_compute(
                    "AllGather", mybir.AluOpType.bypass, replica_groups=rg,
                    ins=[cc_in.opt()], outs=[cc_out.opt()],
                )
                cc_all = bn_pool.tile([P, 2, N_CORES], F32)
                cc_src = bass.AP(
                    tensor=cc_out.tensor,
                    offset=cc_out[:].offset,
                    ap=[[2, P], [1, 2], [2 * P, N_CORES]],
                )
                nc.sync.dma_start(out=cc_all[:], in_=cc_src)
                cc_res = bs[:, 5:7]
                nc.vector.tensor_reduce(
                    out=cc_res.rearrange("p (a b) -> p a b", a=2),
                    in_=cc_all[:],
                    axis=mybir.AxisListType.X,
                    op=mybir.AluOpType.add,
                )
                mu = bs[:, 7:8]
                nc.vector.tensor_scalar(
                    out=mu, in0=cc_res[:, 0:1], scalar2=None,
                    op0=mybir.AluOpType.mult, scalar1=1.0 / N_CORES,
                )
                var = bs[:, 8:9]
                nc.vector.tensor_scalar(
                    out=var, in0=cc_res[:, 1:2], scalar2=None,
                    op0=mybir.AluOpType.mult, scalar1=1.0 / N_CORES,
                )
                mu2 = bs[:, 9:10]
                nc.vector.tensor_tensor(
                    out=mu2, in0=mu, in1=mu, op=mybir.AluOpType.mult
                )
                nc.vector.tensor_tensor(
                    out=var, in0=var, in1=mu2, op=mybir.AluOpType.subtract
                )
                rstd = bs[:, 10:11]
                nc.scalar.activation(
                    out=rstd, in_=var,
                    func=mybir.ActivationFunctionType.Sqrt,
                    bias=eps_sb[:], scale=1.0,
                )
                nc.vector.reciprocal(out=rstd, in_=rstd)
                scale = bs[:, 11:12]
                nc.vector.tensor_tensor(
                    out=scale, in0=rstd, in1=gammaT_sb[:, l : l + 1],
                    op=mybir.AluOpType.mult,
                )
                shift = bs[:, 12:13]
                nc.vector.tensor_tensor(
                    out=shift, in0=mu, in1=scale, op=mybir.AluOpType.mult
                )
                nc.vector.tensor_tensor(
                    out=shift, in0=betaT_sb[:, l : l + 1], in1=shift,
                    op=mybir.AluOpType.subtract,
                )

                # BN apply + relu, zero pad slots, rebuild node-major table
                hT_new = hT_pool.tile([P, SLOTS], F16, tag="hT")
                h_tab_new = dram_tab.tile([SLOTS, D], F16, tag="htab")
                for c0 in range(0, CHUNKS, AGRP):
                    ng = min(AGRP, CHUNKS - c0)
                    gs = slice(c0 * P, (c0 + ng) * P)
                    nc.scalar.activation(
                        out=hT_new[:, gs], in_=z_sb[:, gs],
                        func=mybir.ActivationFunctionType.Relu,
                        bias=shift, scale=scale,
                    )
                    if c0 + ng == CHUNKS:
                        nc.vector.memset(hT_new[:, QUOTA:SLOTS], 0.0)
                    t16g = t16_pool.tile([P, AGRP, P], F16)
                    for k0 in range(0, ng, PGRP):
                        kn = min(PGRP, ng - k0)
                        ps_t = psT.tile([P, PGRP * P], F16, space="PSUM")
                        for k in range(k0, k0 + kn):
                            c = c0 + k
                            cs2 = slice(c * P, (c + 1) * P)
                            nc.tensor.transpose(
                                out=ps_t[:, (k - k0) * P : (k - k0 + 1) * P],
                                in_=hT_new[:, cs2], identity=ident_sb[:],
                            )
                        nc.vector.tensor_copy(
                            out=t16g[:, k0 : k0 + kn, :], in_=ps_t[:, 0 : kn * P]
                        )
                    dest = bass.AP(
                        tensor=h_tab_new.tensor,
                        offset=h_tab_new[:].offset + c0 * P * D,
                        ap=[[D, P], [P * D, ng], [1, D]],
                    )
                    nc.sync.dma_start(out=dest, in_=t16g[:, 0:ng, :])
                hT_prev = hT_new
                h_tab = h_tab_new

            # ---------------- final GraphConv (OUT=2) ----------------
            # transpose-mode gather -> gathT [feat, edge]; PE projection
            # through Wrel2 -> proj [edge, 2]; segment-sum emits
            # feature-major partials [2, slots] incl. masked root + bias.
            rs_in_f = dram_rsi.tile([N_PAD, OUT], F16, tag="rsif")
            sel = None
            proj_sb = None
            grp_lo = 0
            psf = None
            for pos in range(NBINS):
                piece, q, j = pos_info[pos]
                T = t_proc[pos]
                tb = tile_base[pos]
                if tb in grp_start:
                    grp_lo, grp_hi = grp_start[tb]
                    ng = (grp_hi - grp_lo) * P
                    nt = grp_hi - grp_lo
                    gathT = g_pool.tile([P, max_grp_t * P], F16, tag="gath")
                    gtb = gathT[:]
                    nc.gpsimd.dma_gather(
                        out_ap=bass.AP(
                            tensor=gtb.tensor,
                            offset=gtb.offset,
                            ap=[gtb.ap[0], [ng, 1], [1, ng]],
                        ),
                        in_ap=h_tab[:],
                        idxs_ap=idx_sb[:, grp_lo * 8 : grp_hi * 8],
                        num_idxs=ng,
                        num_idxs_reg=nidx_regs[ng],
                        elem_size=D,
                        transpose=True,
                        single_packet=False,
                    )
                    psp = psP.tile([P, max_grp_t * OUT], F32, space="PSUM", tag="psp")
                    for t in range(nt):
                        nc.tensor.matmul(
                            out=psp[:, t * OUT : (t + 1) * OUT],
                            lhsT=gathT[:, t * P : (t + 1) * P],
                            rhs=wrel2_sb[:],
                            start=True, stop=True,
                        )
                    proj_sb = stg_pool.tile([P, max_grp_t * OUT], F16, tag="proj")
                    nc.scalar.activation(
                        out=proj_sb[:, 0 : nt * OUT], in_=psp[:, 0 : nt * OUT],
                        func=mybir.ActivationFunctionType.Copy,
                    )
                    sel = build_sel(grp_lo, grp_hi)
                jj = j - PIECE_J0[piece]
                npc = PIECES[piece]
                pg = jj % PGRP
                pgn = min(PGRP, npc - (jj - pg))
                if pg == 0:
                    psf = psF.tile([P, PGRP * OUT], F32, space="PSUM", tag="psf")
                fo = slice(pg * OUT, (pg + 1) * OUT)
                for t in range(T):
                    nc.tensor.matmul(
                        out=psf[:, fo],
                        lhsT=sel_tile_ap(sel, grp_lo, pos, t),
                        rhs=proj_sb[:, (tb + t - grp_lo) * OUT : (tb + t - grp_lo + 1) * OUT],
                        start=(t == 0),
                        stop=False,
                    )
                # root + bias: nonzero only on the rank that owns these slots
                nc.tensor.matmul(
                    out=psf[:, fo],
                    lhsT=hT_prev[:, j * P : (j + 1) * P],
                    rhs=wroot2m_sb[:, q * OUT : (q + 1) * OUT],
                    start=False,
                    stop=False,
                )
                nc.tensor.matmul(
                    out=psf[:, fo],
                    lhsT=ones_sb[:],
                    rhs=b2m_sb[:, q * OUT : (q + 1) * OUT],
                    start=False,
                    stop=True,
                )
                if pg == pgn - 1:
                    stgf = stg_pool.tile([P, PGRP * OUT], F16, tag="stgf")
                    nc.scalar.activation(
                        out=stgf[:, 0 : pgn * OUT], in_=psf[:, 0 : pgn * OUT],
                        func=mybir.ActivationFunctionType.Copy,
                    )
                    r0 = q * SLOTS + (j - pg) * P
                    dest = bass.AP(
                        tensor=rs_in_f.tensor,
                        offset=rs_in_f[:].offset + r0 * OUT,
                        ap=[[OUT, P], [P * OUT, pgn], [1, OUT]],
                    )
                    nc.sync.dma_start(out=dest, in_=stgf[:, 0 : pgn * OUT])

            rs_out_f = dram_rso.tile([SLOTS, OUT], F16, tag="rsof")
            nc.gpsimd.collective_compute(
                "ReduceScatter",
                mybir.AluOpType.add,
                replica_groups=rg,
                ins=[rs_in_f[:].opt()],
                outs=[rs_out_f[:]],
            )
            nc.sync.dma_start(out=p_out[:], in_=rs_out_f[:])

    lower_extended_insts(nc)
    _split_multiwait(nc)
    return nc


_PROGRAM_CACHE = {}


def _get_program(meta):
    key = hashlib.sha1(repr(sorted(meta.items())).encode()).hexdigest()
    if key not in _PROGRAM_CACHE:
        _PROGRAM_CACHE[key] = build_program(meta)
    return _PROGRAM_CACHE[key]


def _make_in_maps(idx_cores, dst_cores, x_loc, xT_loc,
                  Wrel, Wroot, gamma, beta, Wrel2, Wroot2, b2):
    iotar = np.zeros((P, P, RMAX), np.float16)
    iotar[:, :, :] = np.arange(P, dtype=np.float16)[None, :, None]
    ident16 = np.eye(P, dtype=np.float16)
    common = dict(
        wrel=np.ascontiguousarray(np.asarray(Wrel, np.float32)),
        wroot=np.ascontiguousarray(np.asarray(Wroot, np.float32)),
        wrel2=np.ascontiguousarray(np.asarray(Wrel2, np.float32)),
        gammaT=np.ascontiguousarray(np.asarray(gamma, np.float32).T),
        betaT=np.ascontiguousarray(np.asarray(beta, np.float32).T),
        iotar=np.ascontiguousarray(iotar.reshape(P, P * RMAX)),
        ident16=ident16,
    )
    wroot2 = np.asarray(Wroot2, np.float16)                 # [D, OUT]
    b2 = np.asarray(b2, np.float16).reshape(1, OUT)
    in_maps = []
    for c in range(N_CORES):
        w2m = np.zeros((D, N_CORES, OUT), np.float16)
        w2m[:, c, :] = wroot2
        b2m = np.zeros((1, N_CORES, OUT), np.float16)
        b2m[:, c, :] = b2
        m = dict(common)
        m["x_loc"] = x_loc[c]
        m["xT_loc"] = xT_loc[c]
        m["gidx"] = idx_cores[c]
        m["dst_loc"] = dst_cores[c]
        m["wroot2m"] = np.ascontiguousarray(w2m.reshape(D, N_CORES * OUT))
        m["b2m"] = np.ascontiguousarray(b2m.reshape(1, N_CORES * OUT))
        in_maps.append(m)
    return in_maps


def run(x, edge_index, Wrel, Wroot, b, gamma, beta, Wrel2, Wroot2, b2):
    """Returns (output [N, OUT] float32, nc, meta) - nc exposed for profiling.
    The per-layer GraphConv bias b cancels inside BatchNorm and is unused."""
    meta, newid, idx_cores, dst_cores, x_loc, xT_loc = _preprocess(x, edge_index)
    nc = _get_program(meta)
    in_maps = _make_in_maps(
        idx_cores, dst_cores, x_loc, xT_loc,
        Wrel, Wroot, gamma, beta, Wrel2, Wroot2, b2,
    )
    from concourse.bass_utils import run_bass_kernel_spmd

    res = run_bass_kernel_spmd(nc, in_maps, list(range(N_CORES)))
    full = np.concatenate(
        [res.results[c]["z4"] for c in range(N_CORES)], axis=0
    )  # [N_PAD, OUT]
    return full[newid].astype(np.float32), nc, meta


def kernel(**inputs):
    out, _, _ = run(**{k: np.asarray(v) for k, v in inputs.items()})
    return out


# revision 21
# speedup vs baseline: 1.1112x; 1.0238x over previous
"""GNN message-passing (3x GraphConv+BN+ReLU, final GraphConv) on 8 trn2 cores.

Source-sharded graph parallelism:
  - Nodes are partitioned across 8 cores (6272 slots each, 49 chunks of 128).
    Each core processes the edges whose SOURCE it owns, so per-edge feature
    gathers read a small local fp16 table (6272 rows, int16 indices).
  - Per layer: indirect-DMA gather of the core's edge source rows, one-hot
    matmuls accumulate partial aggregates for ALL 392 destination chunks in
    PSUM, partials stream to a DRAM buffer, and a ReduceScatter (split in
    three pieces, overlapped with the gather phase) reduces them onto the
    destination owner.  Dense transforms + BatchNorm stats/apply are local;
    only a tiny [128,2] stats AllGather crosses cores per layer.
  - One-hot masks are built in a [edge, dst, tile] layout so every DVE
    operand is packed 2-byte (2x DVE mode); tiles read them back with a
    strided matmul AP.
  - The GraphConv bias cancels inside BatchNorm and is skipped; every core
    holds exactly 6250 real nodes with its 22 pad slots pinned to the tail
    of chunk 48, so BN stats are exact and pads are re-zeroed by one memset.
  - Final layer: transpose-mode gather delivers gathT [feat, edge] tiles of
    h3, a per-tile PE projection through Wrel2 gives 2-col edge values, and
    the segment-sum emits feature-major partials; Wroot2+b2 ride along as
    per-rank masked matmuls so the final ReduceScatter yields the output.
"""

import hashlib
import heapq
import sys

import numpy as np

sys.path.insert(0, "/opt/trn_rl_repo")

import concourse.bass as bass  # noqa: E402
import concourse.mybir as mybir  # noqa: E402
import concourse.tile as tile  # noqa: E402
from concourse.vector_clock import ScopedClock  # noqa: E402
from concourse import library_config  # noqa: E402
from concourse.library_overlay import lower_extended_insts  # noqa: E402

N = 50000
E = 800000
D = 128
L = 3
OUT = 2
EPS = 1e-5
N_CORES = 8
P = 128
CHUNKS = 49                 # local dst chunks per core
SLOTS = CHUNKS * P          # 6272
NBINS = N_CORES * CHUNKS    # 392 global dst chunks
N_PAD = N_CORES * SLOTS     # 50176
QUOTA = N // N_CORES        # 6250 real nodes per core
SHORT = QUOTA - 48 * P      # 106 real slots in chunk 48
PIECES = (25, 18, 6)        # local chunks per RS piece
PIECE_J0 = (0, 25, 43)
GGRPS = ((13, 12), (9, 9), (6,))   # gather-group sizes per piece
PGRP = 4                    # chunks per PSUM bank / staging DMA group
RMAX = 36                   # max tiles covered by one sel build (12 chunks x T3)
AGRP = 25                   # chunks per BN-apply / table-write group

F16 = mybir.dt.float16
F32 = mybir.dt.float32

# ---------------------------------------------------------------------------
# walrus in this container accepts at most ONE semaphore wait per instruction.
# Patch the Tile exit drain and add a post-pass splitting multi-wait insts.
# ---------------------------------------------------------------------------
_MAX_WAITS = 1


def _drain_and_barrier(self, tick_clock, wait_clock):
    nc = self.nc
    drain_inst = nc.sync.drain()
    wait_clock.add_sem_waits(
        drain_inst.ins, ScopedClock({None: tick_clock.global_clock})
    )
    si = drain_inst.ins.sync_info
    if si is not None and si.on_wait is not None and len(si.on_wait) > _MAX_WAITS:
        waits = list(si.on_wait)
        si.on_wait = waits[:_MAX_WAITS]
        rest = waits[_MAX_WAITS:]
        for i in range(0, len(rest), _MAX_WAITS):
            nop = nc.sync.nop(nofuse=True)
            nop.ins.sync_info = mybir.SyncInfo(
                on_wait=rest[i : i + _MAX_WAITS], on_update=[]
            )
    nc.all_engine_barrier()
    assert self.sems is not None
    popped = nc._tile_sem_poison_stack.pop()
    assert popped is self._sem_poison
    nc.clear_and_free_semaphores(list(self.sems.allocated().values()))
    nc.all_engine_barrier()


tile.TileContext._drain_and_barrier = _drain_and_barrier


def _split_multiwait(nc):
    n_split = 0
    for fn in nc.m.functions:
        for blk in fn.blocks:
            out = []
            for inst in blk.instructions:
                si = inst.sync_info
                if si is not None and si.on_wait and len(si.on_wait) > _MAX_WAITS:
                    waits = list(si.on_wait)
                    si.on_wait = waits[-_MAX_WAITS:]
                    rest = waits[:-_MAX_WAITS]
                    for i in range(0, len(rest), _MAX_WAITS):
                        n_split += 1
                        out.append(
                            mybir.InstNoOp(
                                name=f"{inst.name}-ws{i}",
                                engine=inst.engine,
                                ins=[],
                                outs=[],
                                bass_nofuse=True,
                                sync_info=mybir.SyncInfo(
                                    on_wait=rest[i : i + _MAX_WAITS], on_update=[]
                                ),
                                debug=inst.debug,
                            )
                        )
                out.append(inst)
            blk.instructions[:] = out
    return n_split


# ---------------------------------------------------------------------------
# Host-side graph partitioning
# ---------------------------------------------------------------------------
def _lpt(nodes, deg_in, bins, caps, bin_of, slot_of, fill):
    heap = [(0, b) for b in bins]
    heapq.heapify(heap)
    for node in nodes:
        d = int(deg_in[node])
        ld, b = heapq.heappop(heap)
        bin_of[node] = b
        slot_of[node] = fill[b]
        fill[b] += 1
        if fill[b] < caps[b]:
            heapq.heappush(heap, (ld + d, b))


def _partition_nodes(deg_in):
    """Assign nodes to (bin, slot): bin b -> core b%8, local chunk b//8.
    Every bin is filled exactly to its cap (128, or 106 for chunk 48), so
    each core holds exactly 6250 real nodes and pads sit at the tail of
    chunk 48.  The heaviest nodes fill a set of "heavy" bins; the rest are
    LPT'd over "light" bins so per-(core,bin) edge counts pack tightly."""
    caps = np.full(NBINS, P, np.int64)
    caps[48 * N_CORES :] = SHORT                 # bins (q, j=48)
    order = np.argsort(-deg_in, kind="stable")
    sdeg = deg_in[order].astype(np.float64)
    pref = np.concatenate([[0.0], np.cumsum(sdeg)])
    total = pref[-1]
    best = (None, None)
    for nh in range(0, 200, 8):
        nl = NBINS - nh
        s_h = pref[min(nh * P, N)]
        m_h = s_h / max(nh, 1) / N_CORES
        m_l = (total - s_h) / nl / N_CORES
        t_h = int(np.ceil((m_h + 3.0 * np.sqrt(m_h * 0.875 + 1)) / P)) if nh else 0
        t_l = int(np.ceil((m_l + 3.0 * np.sqrt(m_l * 0.875 + 1)) / P))
        st = nh * max(t_h, 1) + nl * t_l
        if best[0] is None or st < best[0]:
            best = (st, nh)
    n_heavy = best[1]
    j_cut = CHUNKS - n_heavy // N_CORES
    allb = np.arange(NBINS)
    heavy_bins = allb[allb // N_CORES >= j_cut]
    light_bins = allb[allb // N_CORES < j_cut]

    bin_of = np.empty(N, np.int32)
    slot_of = np.empty(N, np.int32)
    fill = np.zeros(NBINS, np.int64)
    nh_nodes = int(caps[heavy_bins].sum())
    _lpt(order[:nh_nodes], deg_in, heavy_bins, caps, bin_of, slot_of, fill)
    _lpt(order[nh_nodes:], deg_in, light_bins, caps, bin_of, slot_of, fill)
    assert (fill == caps).all()
    return bin_of, slot_of


def _preprocess(x, edge_index):
    x = np.asarray(x, np.float32)
    ei = np.asarray(edge_index)
    src = ei[0].astype(np.int64)
    dst = ei[1].astype(np.int64)
    deg_in = np.bincount(dst, minlength=N)
    bin_of, slot_of = _partition_nodes(deg_in)

    core_of = bin_of % N_CORES
    newid = (
        core_of.astype(np.int64) * SLOTS
        + (bin_of // N_CORES).astype(np.int64) * P
        + slot_of
    )

    e_core = core_of[src]
    e_bin = bin_of[dst]

    cnt = np.zeros((N_CORES, NBINS), np.int64)
    np.add.at(cnt, (e_core, e_bin), 1)
    t_bin = np.maximum(1, -(-cnt.max(axis=0) // P))

    # chunk processing order: per piece, (q, j) with j in the piece range
    ordered_bins = []
    for piece in range(len(PIECES)):
        jr = range(PIECE_J0[piece], PIECE_J0[piece] + PIECES[piece])
        for q in range(N_CORES):
            for j in jr:
                ordered_bins.append(j * N_CORES + q)
    ordered_bins = np.array(ordered_bins)
    bin_pos = np.empty(NBINS, np.int64)
    bin_pos[ordered_bins] = np.arange(NBINS)
    t_proc = t_bin[ordered_bins]
    tile_base = np.concatenate([[0], np.cumsum(t_proc)[:-1]])
    SUM_T = int(t_proc.sum())

    e_pos = bin_pos[e_bin]
    order = np.lexsort((e_pos, e_core))
    s_core = e_core[order]
    s_pos = e_pos[order]
    s_srcslot = (newid[src[order]] % SLOTS).astype(np.int64)
    s_dstslot = slot_of[dst[order]].astype(np.int64)

    bucket = s_core * NBINS + s_pos
    bnd = np.concatenate(
        [[0], np.cumsum(np.bincount(bucket, minlength=N_CORES * NBINS))]
    )
    within = np.arange(E) - bnd[bucket]
    assert (within < t_proc[s_pos] * P).all()

    flat_off = tile_base * P
    e_slot = s_core * (SUM_T * P) + flat_off[s_pos] + within

    gidx = np.zeros(N_CORES * SUM_T * P, np.int16)
    dloc = np.full(N_CORES * SUM_T * P, -1.0, np.float16)
    gidx[e_slot] = s_srcslot.astype(np.int16)
    dloc[e_slot] = s_dstslot.astype(np.float16)
    gidx = gidx.reshape(N_CORES, SUM_T, P)
    dloc = dloc.reshape(N_CORES, SUM_T, P)

    dst_cores = np.ascontiguousarray(dloc.transpose(0, 2, 1))  # [c, 128, SUM_T]

    # gather groups per (piece, q): fixed chunk-count splits
    groups = []
    pos = 0
    for piece in range(len(PIECES)):
        for q in range(N_CORES):
            c0 = 0
            for gsz in GGRPS[piece]:
                lo = tile_base[pos + c0]
                last = pos + c0 + gsz - 1
                hi = tile_base[last] + t_proc[last]
                groups.append((int(lo), int(hi)))
                c0 += gsz
            assert c0 == PIECES[piece]
            pos += PIECES[piece]

    blocks = []
    for (lo, hi) in groups:
        n = (hi - lo) * P
        w = gidx[:, lo:hi, :].reshape(N_CORES, n // 16, 16).transpose(0, 2, 1)
        blocks.append(w)
    idxw = np.concatenate(blocks, axis=2)
    I_COLS = idxw.shape[2]
    idx_cores = np.ascontiguousarray(
        np.broadcast_to(idxw[:, None, :, :], (N_CORES, 8, 16, I_COLS)).reshape(
            N_CORES, P, I_COLS
        )
    )

    x_pad = np.zeros((N_PAD, D), np.float32)
    x_pad[newid] = x
    x_loc = np.ascontiguousarray(x_pad.reshape(N_CORES, SLOTS, D).astype(np.float16))
    xT_loc = np.ascontiguousarray(x_loc.transpose(0, 2, 1))
    meta = dict(
        SUM_T=SUM_T,
        t_proc=tuple(int(t) for t in t_proc),
        groups=tuple(groups),
        I_COLS=int(I_COLS),
    )
    return meta, newid, idx_cores, dst_cores, x_loc, xT_loc


# ---------------------------------------------------------------------------
# Device program
# ---------------------------------------------------------------------------
def build_program(meta):
    SUM_T = meta["SUM_T"]
    t_proc = meta["t_proc"]
    groups = meta["groups"]
    I_COLS = meta["I_COLS"]
    tile_base = [0]
    for t in t_proc[:-1]:
        tile_base.append(tile_base[-1] + t)

    nc = bass.Bass(num_devices=N_CORES)

    p_xloc = nc.declare_dram_parameter("x_loc", [SLOTS, D], F16, isOutput=False)
    p_xT = nc.declare_dram_parameter("xT_loc", [D, SLOTS], F16, isOutput=False)
    p_idx = nc.declare_dram_parameter("gidx", [P, I_COLS], mybir.dt.int16, isOutput=False)
    p_dst = nc.declare_dram_parameter("dst_loc", [P, SUM_T], F16, isOutput=False)
    p_wrel = nc.declare_dram_parameter("wrel", [L, D, D], F32, isOutput=False)
    p_wroot = nc.declare_dram_parameter("wroot", [L, D, D], F32, isOutput=False)
    p_wrel2 = nc.declare_dram_parameter("wrel2", [D, OUT], F32, isOutput=False)
    p_wroot2m = nc.declare_dram_parameter(
        "wroot2m", [D, N_CORES * OUT], F16, isOutput=False
    )
    p_b2m = nc.declare_dram_parameter("b2m", [1, N_CORES * OUT], F16, isOutput=False)
    p_gammaT = nc.declare_dram_parameter("gammaT", [D, L], F32, isOutput=False)
    p_betaT = nc.declare_dram_parameter("betaT", [D, L], F32, isOutput=False)
    p_iotar = nc.declare_dram_parameter("iotar", [P, P * RMAX], F16, isOutput=False)
    p_ident = nc.declare_dram_parameter("ident16", [P, P], F16, isOutput=False)
    p_out = nc.declare_dram_parameter("z4", [SLOTS, OUT], F16, isOutput=True)

    rg = [list(range(N_CORES))]
    n_pieces = len(PIECES)
    piece_cols = tuple(p * P for p in PIECES)
    piece_col0 = tuple(j * P for j in PIECE_J0)

    pos_info = []
    for piece in range(n_pieces):
        jr = range(PIECE_J0[piece], PIECE_J0[piece] + PIECES[piece])
        for q in range(N_CORES):
            for j in jr:
                pos_info.append((piece, q, j))
    piece_end_pos = {}
    acc = 0
    for piece in range(n_pieces):
        acc += PIECES[piece] * N_CORES
        piece_end_pos[acc - 1] = piece

    grp_start = {lo: (lo, hi) for (lo, hi) in groups}
    max_grp_t = max(hi - lo for (lo, hi) in groups)

    # same-T runs of chunks within each gather group, for packed sel builds
    pos_of_tb = {tile_base[pos]: pos for pos in range(NBINS)}
    group_runs = {}     # grp_lo -> list of (run_tb, nc_chunks, T)
    for (lo, hi) in groups:
        runs = []
        pos = pos_of_tb[lo]
        tb = lo
        while tb < hi:
            T = t_proc[pos]
            ncr = 0
            rtb = tb
            while tb < hi and t_proc[pos] == T:
                ncr += 1
                tb += T
                pos += 1
            runs.append((rtb, ncr, T))
        group_runs[lo] = runs

    from contextlib import ExitStack

    with tile.TileContext(nc) as tc:
        with ExitStack() as stack:
            ep = stack.enter_context
            dram_tab = ep(tc.tile_pool(name="dram_tab", bufs=2, space="DRAM"))
            dram_rsi = ep(tc.tile_pool(name="dram_rsi", bufs=2, space="DRAM"))
            dram_rso = ep(tc.tile_pool(name="dram_rso", bufs=2, space="DRAM"))
            dram_cc = ep(tc.tile_pool(name="dram_cc", bufs=2, space="DRAM"))
            singles = ep(tc.tile_pool(name="singles", bufs=1))
            hT_pool = ep(tc.tile_pool(name="hT", bufs=2))
            z_pool = ep(tc.tile_pool(name="zb", bufs=1))
            agg_pool = ep(tc.tile_pool(name="aggb", bufs=1))
            g_pool = ep(tc.tile_pool(name="gath", bufs=4))
            s_pool = ep(tc.tile_pool(name="sel", bufs=4))
            stg_pool = ep(tc.tile_pool(name="stg", bufs=4))
            t16_pool = ep(tc.tile_pool(name="t16p", bufs=2))
            bn_pool = ep(tc.tile_pool(name="bns", bufs=2))
            stat_pool = ep(tc.tile_pool(name="stat", bufs=2))
            psA = ep(tc.tile_pool(name="psA", bufs=3, space="PSUM"))
            psZ = ep(tc.tile_pool(name="psZ", bufs=1, space="PSUM"))
            psT = ep(tc.tile_pool(name="psT", bufs=2, space="PSUM"))
            psF = ep(tc.tile_pool(name="psF", bufs=1, space="PSUM"))
            psP = ep(tc.tile_pool(name="psP", bufs=1, space="PSUM"))

            with tc.high_priority():
                nc.gpsimd.load_library(library_config.mlp)

            grp_sizes = sorted({(hi - lo) * P for (lo, hi) in groups})
            nidx_regs = {n: nc.gpsimd.to_reg(n) for n in grp_sizes}

            # --- constants / weights in SBUF ---
            idx_sb = singles.tile([P, I_COLS], mybir.dt.int16)
            for i0 in range(0, I_COLS, (I_COLS + 3) // 4):
                i1 = min(I_COLS, i0 + (I_COLS + 3) // 4)
                nc.sync.dma_start(out=idx_sb[:, i0:i1], in_=p_idx[:, i0:i1])
            dst_sb = singles.tile([P, SUM_T], F16)
            nc.sync.dma_start(out=dst_sb[:], in_=p_dst[:])
            iotar_sb = singles.tile([P, P * RMAX], F16)
            nc.sync.dma_start(out=iotar_sb[:], in_=p_iotar[:])
            ident_sb = singles.tile([P, P], F16)
            nc.sync.dma_start(out=ident_sb[:], in_=p_ident[:])
            wtmp = singles.tile([P, D], F32)
            wrel_sb = singles.tile([P, L * D], F16)
            wroot_sb = singles.tile([P, L * D], F16)
            for l in range(L):
                nc.sync.dma_start(out=wtmp[:], in_=p_wrel[l])
                nc.scalar.activation(
                    out=wrel_sb[:, l * D : (l + 1) * D], in_=wtmp[:],
                    func=mybir.ActivationFunctionType.Copy,
                )
                nc.sync.dma_start(out=wtmp[:], in_=p_wroot[l])
                nc.scalar.activation(
                    out=wroot_sb[:, l * D : (l + 1) * D], in_=wtmp[:],
                    func=mybir.ActivationFunctionType.Copy,
                )
            wrel2_sb = singles.tile([P, OUT], F16)
            nc.sync.dma_start(out=wtmp[:, 0:OUT], in_=p_wrel2[:])
            nc.scalar.activation(
                out=wrel2_sb[:], in_=wtmp[:, 0:OUT],
                func=mybir.ActivationFunctionType.Copy,
            )
            wroot2m_sb = singles.tile([P, N_CORES * OUT], F16)
            nc.sync.dma_start(out=wroot2m_sb[:], in_=p_wroot2m[:])
            b2m_sb = singles.tile([1, N_CORES * OUT], F16)
            nc.sync.dma_start(out=b2m_sb[:], in_=p_b2m[:])
            gammaT_sb = singles.tile([P, L], F32)
            nc.sync.dma_start(out=gammaT_sb[:], in_=p_gammaT[:])
            betaT_sb = singles.tile([P, L], F32)
            nc.sync.dma_start(out=betaT_sb[:], in_=p_betaT[:])
            ones_sb = singles.tile([1, P], F16)
            nc.vector.memset(ones_sb[:], 1.0)
            eps_sb = singles.tile([P, 1], F32)
            nc.vector.memset(eps_sb[:], EPS)

            hT_prev = hT_pool.tile([P, SLOTS], F16, tag="hT")
            nc.sync.dma_start(out=hT_prev[:], in_=p_xT[:])
            h_tab = p_xloc

            def build_sel(grp_lo, grp_hi):
                """One-hot masks for the group's tiles in [e, dst, tile]
                layout: all DVE operands packed 2-byte -> 2x mode."""
                st = s_pool.tile([P, max_grp_t * P], F16, tag="sel")
                sb = st[:]
                db = dst_sb[:]
                ib = iotar_sb[:]
                for (rtb, ncr, T) in group_runs[grp_lo]:
                    nct = ncr * T
                    base = (rtb - grp_lo) * P
                    nc.vector.tensor_tensor(
                        out=bass.AP(
                            tensor=sb.tensor, offset=sb.offset + base,
                            ap=[sb.ap[0], [nct, P], [1, nct]],
                        ),
                        in0=bass.AP(
                            tensor=db.tensor, offset=db.offset + rtb,
                            ap=[db.ap[0], [0, P], [1, nct]],
                        ),
                        in1=bass.AP(
                            tensor=ib.tensor, offset=ib.offset,
                            ap=[ib.ap[0], [RMAX, P], [1, nct]],
                        ),
                        op=mybir.AluOpType.is_equal,
                    )
                return st

            def sel_tile_ap(st, grp_lo, pos, t):
                """Matmul operand AP for (chunk at pos, tile t): [e, 128 dst]
                with dst stride = the run's nc*T."""
                for (rtb, ncr, T) in group_runs[grp_lo]:
                    if rtb <= tile_base[pos] < rtb + ncr * T:
                        nct = ncr * T
                        base = (rtb - grp_lo) * P
                        col = tile_base[pos] - rtb + t
                        sb = st[:]
                        return bass.AP(
                            tensor=sb.tensor, offset=sb.offset + base + col,
                            ap=[sb.ap[0], [nct, P]],
                        )
                raise AssertionError("tile not in any run")

            def emit_dense_piece(piece, l, z_sb, agg_sb, stats):
                w_rel = wrel_sb[:, l * D : (l + 1) * D]
                w_root = wroot_sb[:, l * D : (l + 1) * D]
                j0, npc = PIECE_J0[piece], PIECES[piece]
                for jg in range(j0, j0 + npc, PGRP):
                    jn = min(PGRP, j0 + npc - jg)
                    psz = psZ.tile([P, PGRP * P], F32, space="PSUM")
                    for k in range(jn):
                        cs = slice((jg + k) * P, (jg + k + 1) * P)
                        ks = slice(k * P, (k + 1) * P)
                        nc.tensor.matmul(
                            out=psz[:, ks], lhsT=w_rel, rhs=agg_sb[:, cs],
                            start=True, stop=False,
                        )
                        nc.tensor.matmul(
                            out=psz[:, ks], lhsT=w_root, rhs=hT_prev[:, cs],
                            start=False, stop=True,
                        )
                    zs = slice(jg * P, (jg + jn) * P)
                    nc.scalar.activation(
                        out=z_sb[:, zs], in_=psz[:, 0 : jn * P],
                        func=mybir.ActivationFunctionType.Copy,
                    )
                    for k in range(jn):
                        j = jg + k
                        width = SHORT if j == 48 else P
                        nc.vector.bn_stats(
                            out=stats[:, j, :],
                            in_=z_sb[:, j * P : j * P + width],
                        )

            for l in range(L):
                z_sb = z_pool.tile([P, SLOTS], F16)
                agg_sb = agg_pool.tile([P, SLOTS], F16)
                stats = stat_pool.tile([P, CHUNKS, nc.vector.BN_STATS_DIM], F32)
                rs_inp = []
                for piece in range(n_pieces):
                    rst = dram_rsi.tile(
                        [N_CORES * P, piece_cols[piece]], F16, tag=f"rsi{piece}"
                    )
                    rs_inp.append(rst)

                gath = sel = None
                grp_lo = 0
                ps4 = None
                rs_outs = []
                gi = 0
                n_grp_p = [len(GGRPS[p]) * N_CORES for p in range(n_pieces)]
                rs_emit_at = {
                    sum(n_grp_p[: p + 1]) + 4: p for p in range(n_pieces - 1)
                }

                def emit_rs(piece_id):
                    ncols = piece_cols[piece_id]
                    rs_out = dram_rso.tile([P, ncols], F16, tag=f"rso{piece_id}")
                    nc.gpsimd.collective_compute(
                        "ReduceScatter",
                        mybir.AluOpType.add,
                        replica_groups=rg,
                        ins=[rs_inp[piece_id][:].opt()],
                        outs=[rs_out[:]],
                    )
                    rs_outs.append(rs_out)

                for pos in range(NBINS):
                    piece, q, j = pos_info[pos]
                    T = t_proc[pos]
                    tb = tile_base[pos]
                    if tb in grp_start:
                        if gi in rs_emit_at:
                            emit_rs(rs_emit_at[gi])
                        gi += 1
                        grp_lo, grp_hi = grp_start[tb]
                        ng = (grp_hi - grp_lo) * P
                        gath = g_pool.tile([P, max_grp_t * P], F16, tag="gath")
                        gv = gath.rearrange("p (t d) -> p t d", t=max_grp_t)
                        nc.gpsimd.dma_gather(
                            out_ap=gv[:, 0 : grp_hi - grp_lo, :],
                            in_ap=h_tab[:],
                            idxs_ap=idx_sb[:, grp_lo * 8 : grp_hi * 8],
                            num_idxs=ng,
                            num_idxs_reg=nidx_regs[ng],
                            elem_size=D,
                            single_packet=False,
                        )
                        sel = build_sel(grp_lo, grp_hi)
                    jj = j - PIECE_J0[piece]
                    npc = PIECES[piece]
                    pg = jj % PGRP
                    pgn = min(PGRP, npc - (jj - pg))
                    if pg == 0:
                        ps4 = psA.tile([P, PGRP * P], F32, space="PSUM")
                    for t in range(T):
                        ft = tb + t - grp_lo
                        nc.tensor.matmul(
                            out=ps4[:, pg * P : (pg + 1) * P],
                            lhsT=gath[:, ft * P : (ft + 1) * P],
                            rhs=sel_tile_ap(sel, grp_lo, pos, t),
                            start=(t == 0),
                            stop=(t == T - 1),
                        )
                    if pg == pgn - 1:
                        stg = stg_pool.tile([P, PGRP * P], F16, tag="stg")
                        nc.scalar.activation(
                            out=stg[:, 0 : pgn * P], in_=ps4[:, 0 : pgn * P],
                            func=mybir.ActivationFunctionType.Copy,
                        )
                        rs_in = rs_inp[piece]
                        pw = piece_cols[piece]
                        col0 = (j - pg) * P - piece_col0[piece]
                        dest = bass.AP(
                            tensor=rs_in.tensor,
                            offset=rs_in[:].offset + q * P * pw + col0,
                            ap=[[pw, P], [1, pgn * P]],
                        )
                        nc.sync.dma_start(out=dest, in_=stg[:, 0 : pgn * P])
                emit_rs(n_pieces - 1)

                # readbacks + dense emitted after the loop: an in-order SP
                # queue must never park a collective-gated DMA ahead of the
                # aggregate staging writes
                for piece_id in range(n_pieces):
                    c0 = piece_col0[piece_id]
                    ncols = piece_cols[piece_id]
                    nc.sync.dma_start(
                        out=agg_sb[:, c0 : c0 + ncols], in_=rs_outs[piece_id][:]
                    )
                for piece in range(n_pieces):
                    emit_dense_piece(piece, l, z_sb, agg_sb, stats)

                # ---- BatchNorm across all nodes (tiny stats AllGather) ----
                bs = bn_pool.tile([P, 16], F32)
                mv = bs[:, 0:2]
                with tc.high_priority():
                    nc.vector.bn_aggr(out=mv, in_=stats[:])
                cc_sb = bs[:, 3:5]
                with tc.high_priority():
                    nc.vector.tensor_copy(out=cc_sb[:, 0:1], in_=mv[:, 0:1])
                    nc.vector.tensor_scalar(
                        out=cc_sb[:, 1:2], in0=mv[:, 0:1], scalar1=mv[:, 0:1],
                        scalar2=mv[:, 1:2], op0=mybir.AluOpType.mult,
                        op1=mybir.AluOpType.add,
                    )
                cc_in = dram_cc.tile([P, 2], F32)
                cc_out = dram_cc.tile([P * N_CORES, 2], F32, addr_space="Shared")
                nc.sync.dma_start(out=cc_in[:], in_=cc_sb)
                nc.gpsimd.collective_compute(
                    "AllGather", mybir.AluOpType.bypass, replica_groups=rg,
                    ins=[cc_in.opt()], outs=[cc_out.opt()],
                )
                cc_all = bn_pool.tile([P, 2, N_CORES], F32)
                cc_src = bass.AP(
                    tensor=cc_out.tensor,
                    offset=cc_out[:].offset,
                    ap=[[2, P], [1, 2], [2 * P, N_CORES]],
                )
                nc.sync.dma_start(out=cc_all[:], in_=cc_src)
                cc_res = bs[:, 5:7]
                nc.vector.tensor_reduce(
                    out=cc_res.rearrange("p (a b) -> p a b", a=2),
                    in_=cc_all[:],
                    axis=mybir.AxisListType.X,
                    op=mybir.AluOpType.add,
                )
                mu = bs[:, 7:8]
                nc.vector.tensor_scalar(
                    out=mu, in0=cc_res[:, 0:1], scalar2=None,
                    op0=mybir.AluOpType.mult, scalar1=1.0 / N_CORES,
                )
                var = bs[:, 8:9]
                nc.vector.tensor_scalar(
                    out=var, in0=cc_res[:, 1:2], scalar2=None,
                    op0=mybir.AluOpType.mult, scalar1=1.0 / N_CORES,
                )
                mu2 = bs[:, 9:10]
                nc.vector.tensor_tensor(
                    out=mu2, in0=mu, in1=mu, op=mybir.AluOpType.mult
                )
                nc.vector.tensor_tensor(
                    out=var, in0=var, in1=mu2, op=mybir.AluOpType.subtract
                )
                rstd = bs[:, 10:11]
                nc.scalar.activation(
                    out=rstd, in_=var,
                    func=mybir.ActivationFunctionType.Sqrt,
                    bias=eps_sb[:], scale=1.0,
                )
                nc.vector.reciprocal(out=rstd, in_=rstd)
                scale = bs[:, 11:12]
                nc.vector.tensor_tensor(
                    out=scale, in0=rstd, in1=gammaT_sb[:, l : l + 1],
                    op=mybir.AluOpType.mult,
                )
                shift = bs[:, 12:13]
                nc.vector.tensor_tensor(
                    out=shift, in0=mu, in1=scale, op=mybir.AluOpType.mult
                )
                nc.vector.tensor_tensor(
                    out=shift, in0=betaT_sb[:, l : l + 1], in1=shift,
                    op=mybir.AluOpType.subtract,
                )

                # BN apply + relu, zero pad slots, rebuild node-major table
                hT_new = hT_pool.tile([P, SLOTS], F16, tag="hT")
                h_tab_new = dram_tab.tile([SLOTS, D], F16, tag="htab")
                for c0 in range(0, CHUNKS, AGRP):
                    ng = min(AGRP, CHUNKS - c0)
                    gs = slice(c0 * P, (c0 + ng) * P)
                    nc.scalar.activation(
                        out=hT_new[:, gs], in_=z_sb[:, gs],
                        func=mybir.ActivationFunctionType.Relu,
                        bias=shift, scale=scale,
                    )
                    if c0 + ng == CHUNKS:
                        nc.vector.memset(hT_new[:, QUOTA:SLOTS], 0.0)
                    t16g = t16_pool.tile([P, AGRP, P], F16)
                    for k0 in range(0, ng, PGRP):
                        kn = min(PGRP, ng - k0)
                        ps_t = psT.tile([P, PGRP * P], F16, space="PSUM")
                        for k in range(k0, k0 + kn):
                            c = c0 + k
                            cs2 = slice(c * P, (c + 1) * P)
                            nc.tensor.transpose(
                                out=ps_t[:, (k - k0) * P : (k - k0 + 1) * P],
                                in_=hT_new[:, cs2], identity=ident_sb[:],
                            )
                        nc.vector.tensor_copy(
                            out=t16g[:, k0 : k0 + kn, :], in_=ps_t[:, 0 : kn * P]
                        )
                    dest = bass.AP(
                        tensor=h_tab_new.tensor,
                        offset=h_tab_new[:].offset + c0 * P * D,
                        ap=[[D, P], [P * D, ng], [1, D]],
                    )
                    nc.sync.dma_start(out=dest, in_=t16g[:, 0:ng, :])
                hT_prev = hT_new
                h_tab = h_tab_new

            # ---------------- final GraphConv (OUT=2) ----------------
            # transpose-mode gather -> gathT [feat, edge]; PE projection
            # through Wrel2 -> proj [edge, 2]; segment-sum emits
            # feature-major partials [2, slots] incl. masked root + bias.
            rs_in_f = dram_rsi.tile([N_PAD, OUT], F16, tag="rsif")
            sel = None
            proj_sb = None
            grp_lo = 0
            psf = None
            for pos in range(NBINS):
                piece, q, j = pos_info[pos]
                T = t_proc[pos]
                tb = tile_base[pos]
                if tb in grp_start:
                    grp_lo, grp_hi = grp_start[tb]
                    ng = (grp_hi - grp_lo) * P
                    nt = grp_hi - grp_lo
                    gathT = g_pool.tile([P, max_grp_t * P], F16, tag="gath")
                    gtb = gathT[:]
                    nc.gpsimd.dma_gather(
                        out_ap=bass.AP(
                            tensor=gtb.tensor,
                            offset=gtb.offset,
                            ap=[gtb.ap[0], [ng, 1], [1, ng]],
                        ),
                        in_ap=h_tab[:],
                        idxs_ap=idx_sb[:, grp_lo * 8 : grp_hi * 8],
                        num_idxs=ng,
                        num_idxs_reg=nidx_regs[ng],
                        elem_size=D,
                        transpose=True,
                        single_packet=False,
                    )
                    psp = psP.tile([P, max_grp_t * OUT], F32, space="PSUM", tag="psp")
                    for t in range(nt):
                        nc.tensor.matmul(
                            out=psp[:, t * OUT : (t + 1) * OUT],
                            lhsT=gathT[:, t * P : (t + 1) * P],
                            rhs=wrel2_sb[:],
                            start=True, stop=True,
                        )
                    proj_sb = stg_pool.tile([P, max_grp_t * OUT], F16, tag="proj")
                    nc.scalar.activation(
                        out=proj_sb[:, 0 : nt * OUT], in_=psp[:, 0 : nt * OUT],
                        func=mybir.ActivationFunctionType.Copy,
                    )
                    sel = build_sel(grp_lo, grp_hi)
                jj = j - PIECE_J0[piece]
                npc = PIECES[piece]
                pg = jj % PGRP
                pgn = min(PGRP, npc - (jj - pg))
                if pg == 0:
                    psf = psF.tile([P, PGRP * OUT], F32, space="PSUM", tag="psf")
                fo = slice(pg * OUT, (pg + 1) * OUT)
                for t in range(T):
                    nc.tensor.matmul(
                        out=psf[:, fo],
                        lhsT=sel_tile_ap(sel, grp_lo, pos, t),
                        rhs=proj_sb[:, (tb + t - grp_lo) * OUT : (tb + t - grp_lo + 1) * OUT],
                        start=(t == 0),
                        stop=False,
                    )
                # root + bias: nonzero only on the rank that owns these slots
                nc.tensor.matmul(
                    out=psf[:, fo],
                    lhsT=hT_prev[:, j * P : (j + 1) * P],
                    rhs=wroot2m_sb[:, q * OUT : (q + 1) * OUT],
                    start=False,
                    stop=False,
                )
                nc.tensor.matmul(
                    out=psf[:, fo],
                    lhsT=ones_sb[:],
                    rhs=b2m_sb[:, q * OUT : (q + 1) * OUT],
                    start=False,
                    stop=True,
                )
                if pg == pgn - 1:
                    stgf = stg_pool.tile([P, PGRP * OUT], F16, tag="stgf")
                    nc.scalar.activation(
                        out=stgf[:, 0 : pgn * OUT], in_=psf[:, 0 : pgn * OUT],
                        func=mybir.ActivationFunctionType.Copy,
                    )
                    r0 = q * SLOTS + (j - pg) * P
                    dest = bass.AP(
                        tensor=rs_in_f.tensor,
                        offset=rs_in_f[:].offset + r0 * OUT,
                        ap=[[OUT, P], [P * OUT, pgn], [1, OUT]],
                    )
                    nc.sync.dma_start(out=dest, in_=stgf[:, 0 : pgn * OUT])

            rs_out_f = dram_rso.tile([SLOTS, OUT], F16, tag="rsof")
            nc.gpsimd.collective_compute(
                "ReduceScatter",
                mybir.AluOpType.add,
                replica_groups=rg,
                ins=[rs_in_f[:].opt()],
                outs=[rs_out_f[:]],
            )
            nc.sync.dma_start(out=p_out[:], in_=rs_out_f[:])

    lower_extended_insts(nc)
    _split_multiwait(nc)
    return nc


_PROGRAM_CACHE = {}


def _get_program(meta):
    key = hashlib.sha1(repr(sorted(meta.items())).encode()).hexdigest()
    if key not in _PROGRAM_CACHE:
        _PROGRAM_CACHE[key] = build_program(meta)
    return _PROGRAM_CACHE[key]


def _make_in_maps(idx_cores, dst_cores, x_loc, xT_loc,
                  Wrel, Wroot, gamma, beta, Wrel2, Wroot2, b2):
    iotar = np.zeros((P, P, RMAX), np.float16)
    iotar[:, :, :] = np.arange(P, dtype=np.float16)[None, :, None]
    ident16 = np.eye(P, dtype=np.float16)
    common = dict(
        wrel=np.ascontiguousarray(np.asarray(Wrel, np.float32)),
        wroot=np.ascontiguousarray(np.asarray(Wroot, np.float32)),
        wrel2=np.ascontiguousarray(np.asarray(Wrel2, np.float32)),
        gammaT=np.ascontiguousarray(np.asarray(gamma, np.float32).T),
        betaT=np.ascontiguousarray(np.asarray(beta, np.float32).T),
        iotar=np.ascontiguousarray(iotar.reshape(P, P * RMAX)),
        ident16=ident16,
    )
    wroot2 = np.asarray(Wroot2, np.float16)                 # [D, OUT]
    b2 = np.asarray(b2, np.float16).reshape(1, OUT)
    in_maps = []
    for c in range(N_CORES):
        w2m = np.zeros((D, N_CORES, OUT), np.float16)
        w2m[:, c, :] = wroot2
        b2m = np.zeros((1, N_CORES, OUT), np.float16)
        b2m[:, c, :] = b2
        m = dict(common)
        m["x_loc"] = x_loc[c]
        m["xT_loc"] = xT_loc[c]
        m["gidx"] = idx_cores[c]
        m["dst_loc"] = dst_cores[c]
        m["wroot2m"] = np.ascontiguousarray(w2m.reshape(D, N_CORES * OUT))
        m["b2m"] = np.ascontiguousarray(b2m.reshape(1, N_CORES * OUT))
        in_maps.append(m)
    return in_maps


def run(x, edge_index, Wrel, Wroot, b, gamma, beta, Wrel2, Wroot2, b2):
    """Returns (output [N, OUT] float32, nc, meta) - nc exposed for profiling.
    The per-layer GraphConv bias b cancels inside BatchNorm and is unused."""
    meta, newid, idx_cores, dst_cores, x_loc, xT_loc = _preprocess(x, edge_index)
    nc = _get_program(meta)
    in_maps = _make_in_maps(
        idx_cores, dst_cores, x_loc, xT_loc,
        Wrel, Wroot, gamma, beta, Wrel2, Wroot2, b2,
    )
    from concourse.bass_utils import run_bass_kernel_spmd

    res = run_bass_kernel_spmd(nc, in_maps, list(range(N_CORES)))
    full = np.concatenate(
        [res.results[c]["z4"] for c in range(N_CORES)], axis=0
    )  # [N_PAD, OUT]
    return full[newid].astype(np.float32), nc, meta


def kernel(**inputs):
    out, _, _ = run(**{k: np.asarray(v) for k, v in inputs.items()})
    return out


# revision 22
# speedup vs baseline: 1.1647x; 1.0482x over previous
"""GNN message-passing (3x GraphConv+BN+ReLU, final GraphConv) on 8 trn2 cores.

Source-sharded graph parallelism:
  - Nodes are partitioned across 8 cores (6272 slots each, 49 chunks of 128).
    Each core processes the edges whose SOURCE it owns, so per-edge feature
    gathers read a small local fp16 table (6272 rows, int16 indices).
  - Per layer: indirect-DMA gather of the core's edge source rows, one-hot
    matmuls accumulate partial aggregates for ALL 392 destination chunks in
    PSUM, partials stream to a DRAM buffer, and a ReduceScatter (split in
    three pieces, overlapped with the gather phase) reduces them onto the
    destination owner.  Dense transforms + BatchNorm stats/apply are local;
    only a tiny [128,2] stats AllGather crosses cores per layer.
  - One-hot masks are built in a [edge, dst, tile] layout so every DVE
    operand is packed 2-byte (2x DVE mode); tiles read them back with a
    strided matmul AP.
  - The GraphConv bias cancels inside BatchNorm and is skipped; every core
    holds exactly 6250 real nodes with its 22 pad slots pinned to the tail
    of chunk 48, so BN stats are exact and pads are re-zeroed by one memset.
  - Final layer: transpose-mode gather delivers gathT [feat, edge] tiles of
    h3, a per-tile PE projection through Wrel2 gives 2-col edge values, and
    the segment-sum emits feature-major partials; Wroot2+b2 ride along as
    per-rank masked matmuls so the final ReduceScatter yields the output.
"""

import hashlib
import heapq
import sys

import numpy as np

sys.path.insert(0, "/opt/trn_rl_repo")

import concourse.bass as bass  # noqa: E402
import concourse.mybir as mybir  # noqa: E402
import concourse.tile as tile  # noqa: E402
from concourse.vector_clock import ScopedClock  # noqa: E402
from concourse import library_config  # noqa: E402
from concourse.library_overlay import lower_extended_insts  # noqa: E402

N = 50000
E = 800000
D = 128
L = 3
OUT = 2
EPS = 1e-5
N_CORES = 8
P = 128
CHUNKS = 49                 # local dst chunks per core
SLOTS = CHUNKS * P          # 6272
NBINS = N_CORES * CHUNKS    # 392 global dst chunks
N_PAD = N_CORES * SLOTS     # 50176
QUOTA = N // N_CORES        # 6250 real nodes per core
SHORT = QUOTA - 48 * P      # 106 real slots in chunk 48
PIECES = (25, 18, 6)        # local chunks per RS piece
PIECE_J0 = (0, 25, 43)
GGRPS = ((13, 12), (9, 9), (6,))   # gather-group sizes per piece
PGRP = 4                    # chunks per PSUM bank / staging DMA group
RMAX = 36                   # max tiles covered by one sel build (12 chunks x T3)
AGRP = 25                   # chunks per BN-apply / table-write group

F16 = mybir.dt.float16
F32 = mybir.dt.float32

# ---------------------------------------------------------------------------
# walrus in this container accepts at most ONE semaphore wait per instruction.
# Patch the Tile exit drain and add a post-pass splitting multi-wait insts.
# ---------------------------------------------------------------------------
_MAX_WAITS = 1


def _drain_and_barrier(self, tick_clock, wait_clock):
    nc = self.nc
    drain_inst = nc.sync.drain()
    wait_clock.add_sem_waits(
        drain_inst.ins, ScopedClock({None: tick_clock.global_clock})
    )
    si = drain_inst.ins.sync_info
    if si is not None and si.on_wait is not None and len(si.on_wait) > _MAX_WAITS:
        waits = list(si.on_wait)
        si.on_wait = waits[:_MAX_WAITS]
        rest = waits[_MAX_WAITS:]
        for i in range(0, len(rest), _MAX_WAITS):
            nop = nc.sync.nop(nofuse=True)
            nop.ins.sync_info = mybir.SyncInfo(
                on_wait=rest[i : i + _MAX_WAITS], on_update=[]
            )
    nc.all_engine_barrier()
    assert self.sems is not None
    popped = nc._tile_sem_poison_stack.pop()
    assert popped is self._sem_poison
    nc.clear_and_free_semaphores(list(self.sems.allocated().values()))
    nc.all_engine_barrier()


tile.TileContext._drain_and_barrier = _drain_and_barrier


def _split_multiwait(nc):
    n_split = 0
    for fn in nc.m.functions:
        for blk in fn.blocks:
            out = []
            for inst in blk.instructions:
                si = inst.sync_info
                if si is not None and si.on_wait and len(si.on_wait) > _MAX_WAITS:
                    waits = list(si.on_wait)
                    si.on_wait = waits[-_MAX_WAITS:]
                    rest = waits[:-_MAX_WAITS]
                    for i in range(0, len(rest), _MAX_WAITS):
                        n_split += 1
                        out.append(
                            mybir.InstNoOp(
                                name=f"{inst.name}-ws{i}",
                                engine=inst.engine,
                                ins=[],
                                outs=[],
                                bass_nofuse=True,
                                sync_info=mybir.SyncInfo(
                                    on_wait=rest[i : i + _MAX_WAITS], on_update=[]
                                ),
                                debug=inst.debug,
                            )
                        )
                out.append(inst)
            blk.instructions[:] = out
    return n_split


# ---------------------------------------------------------------------------
# Host-side graph partitioning
# ---------------------------------------------------------------------------
def _lpt(nodes, deg_in, bins, caps, bin_of, slot_of, fill):
    heap = [(0, b) for b in bins]
    heapq.heapify(heap)
    for node in nodes:
        d = int(deg_in[node])
        ld, b = heapq.heappop(heap)
        bin_of[node] = b
        slot_of[node] = fill[b]
        fill[b] += 1
        if fill[b] < caps[b]:
            heapq.heappush(heap, (ld + d, b))


def _partition_nodes(deg_in):
    """Assign nodes to (bin, slot): bin b -> core b%8, local chunk b//8.
    Every bin is filled exactly to its cap (128, or 106 for chunk 48), so
    each core holds exactly 6250 real nodes and pads sit at the tail of
    chunk 48.  The heaviest nodes fill a set of "heavy" bins; the rest are
    LPT'd over "light" bins so per-(core,bin) edge counts pack tightly."""
    caps = np.full(NBINS, P, np.int64)
    caps[48 * N_CORES :] = SHORT                 # bins (q, j=48)
    order = np.argsort(-deg_in, kind="stable")
    sdeg = deg_in[order].astype(np.float64)
    pref = np.concatenate([[0.0], np.cumsum(sdeg)])
    total = pref[-1]
    best = (None, None)
    for nh in range(0, 200, 8):
        nl = NBINS - nh
        s_h = pref[min(nh * P, N)]
        m_h = s_h / max(nh, 1) / N_CORES
        m_l = (total - s_h) / nl / N_CORES
        t_h = int(np.ceil((m_h + 3.0 * np.sqrt(m_h * 0.875 + 1)) / P)) if nh else 0
        t_l = int(np.ceil((m_l + 3.0 * np.sqrt(m_l * 0.875 + 1)) / P))
        st = nh * max(t_h, 1) + nl * t_l
        if best[0] is None or st < best[0]:
            best = (st, nh)
    n_heavy = best[1]
    j_cut = CHUNKS - n_heavy // N_CORES
    allb = np.arange(NBINS)
    heavy_bins = allb[allb // N_CORES >= j_cut]
    light_bins = allb[allb // N_CORES < j_cut]

    bin_of = np.empty(N, np.int32)
    slot_of = np.empty(N, np.int32)
    fill = np.zeros(NBINS, np.int64)
    nh_nodes = int(caps[heavy_bins].sum())
    _lpt(order[:nh_nodes], deg_in, heavy_bins, caps, bin_of, slot_of, fill)
    _lpt(order[nh_nodes:], deg_in, light_bins, caps, bin_of, slot_of, fill)
    assert (fill == caps).all()
    return bin_of, slot_of


def _preprocess(x, edge_index):
    x = np.asarray(x, np.float32)
    ei = np.asarray(edge_index)
    src = ei[0].astype(np.int64)
    dst = ei[1].astype(np.int64)
    deg_in = np.bincount(dst, minlength=N)
    bin_of, slot_of = _partition_nodes(deg_in)

    core_of = bin_of % N_CORES
    newid = (
        core_of.astype(np.int64) * SLOTS
        + (bin_of // N_CORES).astype(np.int64) * P
        + slot_of
    )

    e_core = core_of[src]
    e_bin = bin_of[dst]

    cnt = np.zeros((N_CORES, NBINS), np.int64)
    np.add.at(cnt, (e_core, e_bin), 1)
    t_bin = np.maximum(1, -(-cnt.max(axis=0) // P))

    # chunk processing order: per piece, (q, j) with j in the piece range
    ordered_bins = []
    for piece in range(len(PIECES)):
        jr = range(PIECE_J0[piece], PIECE_J0[piece] + PIECES[piece])
        for q in range(N_CORES):
            for j in jr:
                ordered_bins.append(j * N_CORES + q)
    ordered_bins = np.array(ordered_bins)
    bin_pos = np.empty(NBINS, np.int64)
    bin_pos[ordered_bins] = np.arange(NBINS)
    t_proc = t_bin[ordered_bins]
    tile_base = np.concatenate([[0], np.cumsum(t_proc)[:-1]])
    SUM_T = int(t_proc.sum())

    e_pos = bin_pos[e_bin]
    order = np.lexsort((e_pos, e_core))
    s_core = e_core[order]
    s_pos = e_pos[order]
    s_srcslot = (newid[src[order]] % SLOTS).astype(np.int64)
    s_dstslot = slot_of[dst[order]].astype(np.int64)

    bucket = s_core * NBINS + s_pos
    bnd = np.concatenate(
        [[0], np.cumsum(np.bincount(bucket, minlength=N_CORES * NBINS))]
    )
    within = np.arange(E) - bnd[bucket]
    assert (within < t_proc[s_pos] * P).all()

    flat_off = tile_base * P
    e_slot = s_core * (SUM_T * P) + flat_off[s_pos] + within

    gidx = np.zeros(N_CORES * SUM_T * P, np.int16)
    dloc = np.full(N_CORES * SUM_T * P, -1.0, np.float16)
    gidx[e_slot] = s_srcslot.astype(np.int16)
    dloc[e_slot] = s_dstslot.astype(np.float16)
    gidx = gidx.reshape(N_CORES, SUM_T, P)
    dloc = dloc.reshape(N_CORES, SUM_T, P)

    dst_cores = np.ascontiguousarray(dloc.transpose(0, 2, 1))  # [c, 128, SUM_T]

    # gather groups per (piece, q): fixed chunk-count splits
    groups = []
    pos = 0
    for piece in range(len(PIECES)):
        for q in range(N_CORES):
            c0 = 0
            for gsz in GGRPS[piece]:
                lo = tile_base[pos + c0]
                last = pos + c0 + gsz - 1
                hi = tile_base[last] + t_proc[last]
                groups.append((int(lo), int(hi)))
                c0 += gsz
            assert c0 == PIECES[piece]
            pos += PIECES[piece]

    blocks = []
    for (lo, hi) in groups:
        n = (hi - lo) * P
        w = gidx[:, lo:hi, :].reshape(N_CORES, n // 16, 16).transpose(0, 2, 1)
        blocks.append(w)
    idxw = np.concatenate(blocks, axis=2)
    I_COLS = idxw.shape[2]
    idx_cores = np.ascontiguousarray(
        np.broadcast_to(idxw[:, None, :, :], (N_CORES, 8, 16, I_COLS)).reshape(
            N_CORES, P, I_COLS
        )
    )

    x_pad = np.zeros((N_PAD, D), np.float32)
    x_pad[newid] = x
    x_loc = np.ascontiguousarray(x_pad.reshape(N_CORES, SLOTS, D).astype(np.float16))
    xT_loc = np.ascontiguousarray(x_loc.transpose(0, 2, 1))
    meta = dict(
        SUM_T=SUM_T,
        t_proc=tuple(int(t) for t in t_proc),
        groups=tuple(groups),
        I_COLS=int(I_COLS),
    )
    return meta, newid, idx_cores, dst_cores, x_loc, xT_loc


# ---------------------------------------------------------------------------
# Device program
# ---------------------------------------------------------------------------
def build_program(meta):
    SUM_T = meta["SUM_T"]
    t_proc = meta["t_proc"]
    groups = meta["groups"]
    I_COLS = meta["I_COLS"]
    tile_base = [0]
    for t in t_proc[:-1]:
        tile_base.append(tile_base[-1] + t)

    nc = bass.Bass(num_devices=N_CORES)

    p_xloc = nc.declare_dram_parameter("x_loc", [SLOTS, D], F16, isOutput=False)
    p_xT = nc.declare_dram_parameter("xT_loc", [D, SLOTS], F16, isOutput=False)
    p_idx = nc.declare_dram_parameter("gidx", [P, I_COLS], mybir.dt.int16, isOutput=False)
    p_dst = nc.declare_dram_parameter("dst_loc", [P, SUM_T], F16, isOutput=False)
    p_wrel = nc.declare_dram_parameter("wrel", [L, D, D], F32, isOutput=False)
    p_wroot = nc.declare_dram_parameter("wroot", [L, D, D], F32, isOutput=False)
    p_wrel2 = nc.declare_dram_parameter("wrel2", [D, OUT], F32, isOutput=False)
    p_wroot2m = nc.declare_dram_parameter(
        "wroot2m", [D, N_CORES * OUT], F16, isOutput=False
    )
    p_b2m = nc.declare_dram_parameter("b2m", [1, N_CORES * OUT], F16, isOutput=False)
    p_gammaT = nc.declare_dram_parameter("gammaT", [D, L], F32, isOutput=False)
    p_betaT = nc.declare_dram_parameter("betaT", [D, L], F32, isOutput=False)
    p_iotar = nc.declare_dram_parameter("iotar", [P, P * RMAX], F16, isOutput=False)
    p_ident = nc.declare_dram_parameter("ident16", [P, P], F16, isOutput=False)
    p_out = nc.declare_dram_parameter("z4", [SLOTS, OUT], F16, isOutput=True)

    rg = [list(range(N_CORES))]
    n_pieces = len(PIECES)
    piece_cols = tuple(p * P for p in PIECES)
    piece_col0 = tuple(j * P for j in PIECE_J0)

    pos_info = []
    for piece in range(n_pieces):
        jr = range(PIECE_J0[piece], PIECE_J0[piece] + PIECES[piece])
        for q in range(N_CORES):
            for j in jr:
                pos_info.append((piece, q, j))
    piece_end_pos = {}
    acc = 0
    for piece in range(n_pieces):
        acc += PIECES[piece] * N_CORES
        piece_end_pos[acc - 1] = piece

    grp_start = {lo: (lo, hi) for (lo, hi) in groups}
    max_grp_t = max(hi - lo for (lo, hi) in groups)

    # same-T runs of chunks within each gather group, for packed sel builds
    pos_of_tb = {tile_base[pos]: pos for pos in range(NBINS)}
    group_runs = {}     # grp_lo -> list of (run_tb, nc_chunks, T)
    for (lo, hi) in groups:
        runs = []
        pos = pos_of_tb[lo]
        tb = lo
        while tb < hi:
            T = t_proc[pos]
            ncr = 0
            rtb = tb
            while tb < hi and t_proc[pos] == T:
                ncr += 1
                tb += T
                pos += 1
            runs.append((rtb, ncr, T))
        group_runs[lo] = runs

    from contextlib import ExitStack

    with tile.TileContext(nc) as tc:
        with ExitStack() as stack:
            ep = stack.enter_context
            dram_tab = ep(tc.tile_pool(name="dram_tab", bufs=2, space="DRAM"))
            dram_rsi = ep(tc.tile_pool(name="dram_rsi", bufs=2, space="DRAM"))
            dram_rso = ep(tc.tile_pool(name="dram_rso", bufs=2, space="DRAM"))
            dram_cc = ep(tc.tile_pool(name="dram_cc", bufs=2, space="DRAM"))
            singles = ep(tc.tile_pool(name="singles", bufs=1))
            hT_pool = ep(tc.tile_pool(name="hT", bufs=2))
            z_pool = ep(tc.tile_pool(name="zb", bufs=1))
            agg_pool = ep(tc.tile_pool(name="aggb", bufs=1))
            g_pool = ep(tc.tile_pool(name="gath", bufs=4))
            s_pool = ep(tc.tile_pool(name="sel", bufs=4))
            stg_pool = ep(tc.tile_pool(name="stg", bufs=4))
            t16_pool = ep(tc.tile_pool(name="t16p", bufs=2))
            bn_pool = ep(tc.tile_pool(name="bns", bufs=2))
            stat_pool = ep(tc.tile_pool(name="stat", bufs=2))
            psA = ep(tc.tile_pool(name="psA", bufs=3, space="PSUM"))
            psZ = ep(tc.tile_pool(name="psZ", bufs=1, space="PSUM"))
            psT = ep(tc.tile_pool(name="psT", bufs=2, space="PSUM"))
            psF = ep(tc.tile_pool(name="psF", bufs=1, space="PSUM"))
            psP = ep(tc.tile_pool(name="psP", bufs=1, space="PSUM"))

            with tc.high_priority():
                nc.gpsimd.load_library(library_config.mlp)

            grp_sizes = sorted({(hi - lo) * P for (lo, hi) in groups})
            nidx_regs = {n: nc.gpsimd.to_reg(n) for n in grp_sizes}

            # --- constants / weights in SBUF ---
            idx_sb = singles.tile([P, I_COLS], mybir.dt.int16)
            for i0 in range(0, I_COLS, (I_COLS + 3) // 4):
                i1 = min(I_COLS, i0 + (I_COLS + 3) // 4)
                nc.sync.dma_start(out=idx_sb[:, i0:i1], in_=p_idx[:, i0:i1])
            dst_sb = singles.tile([P, SUM_T], F16)
            nc.sync.dma_start(out=dst_sb[:], in_=p_dst[:])
            iotar_sb = singles.tile([P, P * RMAX], F16)
            nc.sync.dma_start(out=iotar_sb[:], in_=p_iotar[:])
            ident_sb = singles.tile([P, P], F16)
            nc.sync.dma_start(out=ident_sb[:], in_=p_ident[:])
            wtmp = singles.tile([P, D], F32)
            wrel_sb = singles.tile([P, L * D], F16)
            wroot_sb = singles.tile([P, L * D], F16)
            for l in range(L):
                nc.sync.dma_start(out=wtmp[:], in_=p_wrel[l])
                nc.scalar.activation(
                    out=wrel_sb[:, l * D : (l + 1) * D], in_=wtmp[:],
                    func=mybir.ActivationFunctionType.Copy,
                )
                nc.sync.dma_start(out=wtmp[:], in_=p_wroot[l])
                nc.scalar.activation(
                    out=wroot_sb[:, l * D : (l + 1) * D], in_=wtmp[:],
                    func=mybir.ActivationFunctionType.Copy,
                )
            wrel2_sb = singles.tile([P, OUT], F16)
            nc.sync.dma_start(out=wtmp[:, 0:OUT], in_=p_wrel2[:])
            nc.scalar.activation(
                out=wrel2_sb[:], in_=wtmp[:, 0:OUT],
                func=mybir.ActivationFunctionType.Copy,
            )
            wroot2m_sb = singles.tile([P, N_CORES * OUT], F16)
            nc.sync.dma_start(out=wroot2m_sb[:], in_=p_wroot2m[:])
            b2m_sb = singles.tile([1, N_CORES * OUT], F16)
            nc.sync.dma_start(out=b2m_sb[:], in_=p_b2m[:])
            gammaT_sb = singles.tile([P, L], F32)
            nc.sync.dma_start(out=gammaT_sb[:], in_=p_gammaT[:])
            betaT_sb = singles.tile([P, L], F32)
            nc.sync.dma_start(out=betaT_sb[:], in_=p_betaT[:])
            ones_sb = singles.tile([1, P], F16)
            nc.vector.memset(ones_sb[:], 1.0)
            eps_sb = singles.tile([P, 1], F32)
            nc.vector.memset(eps_sb[:], EPS)

            hT_prev = hT_pool.tile([P, SLOTS], F16, tag="hT")
            nc.sync.dma_start(out=hT_prev[:], in_=p_xT[:])
            h_tab = p_xloc

            def build_sel(grp_lo, grp_hi):
                """One-hot masks for the group's tiles in [e, dst, tile]
                layout: all DVE operands packed 2-byte -> 2x mode."""
                st = s_pool.tile([P, max_grp_t * P], F16, tag="sel")
                sb = st[:]
                db = dst_sb[:]
                ib = iotar_sb[:]
                for (rtb, ncr, T) in group_runs[grp_lo]:
                    nct = ncr * T
                    base = (rtb - grp_lo) * P
                    nc.vector.tensor_tensor(
                        out=bass.AP(
                            tensor=sb.tensor, offset=sb.offset + base,
                            ap=[sb.ap[0], [nct, P], [1, nct]],
                        ),
                        in0=bass.AP(
                            tensor=db.tensor, offset=db.offset + rtb,
                            ap=[db.ap[0], [0, P], [1, nct]],
                        ),
                        in1=bass.AP(
                            tensor=ib.tensor, offset=ib.offset,
                            ap=[ib.ap[0], [RMAX, P], [1, nct]],
                        ),
                        op=mybir.AluOpType.is_equal,
                    )
                return st

            def sel_tile_ap(st, grp_lo, pos, t):
                """Matmul operand AP for (chunk at pos, tile t): [e, 128 dst]
                with dst stride = the run's nc*T."""
                for (rtb, ncr, T) in group_runs[grp_lo]:
                    if rtb <= tile_base[pos] < rtb + ncr * T:
                        nct = ncr * T
                        base = (rtb - grp_lo) * P
                        col = tile_base[pos] - rtb + t
                        sb = st[:]
                        return bass.AP(
                            tensor=sb.tensor, offset=sb.offset + base + col,
                            ap=[sb.ap[0], [nct, P]],
                        )
                raise AssertionError("tile not in any run")

            def emit_dense_piece(piece, l, z_sb, agg_sb, stats):
                w_rel = wrel_sb[:, l * D : (l + 1) * D]
                w_root = wroot_sb[:, l * D : (l + 1) * D]
                j0, npc = PIECE_J0[piece], PIECES[piece]
                for jg in range(j0, j0 + npc, PGRP):
                    jn = min(PGRP, j0 + npc - jg)
                    psz = psZ.tile([P, PGRP * P], F32, space="PSUM")
                    for k in range(jn):
                        cs = slice((jg + k) * P, (jg + k + 1) * P)
                        ks = slice(k * P, (k + 1) * P)
                        nc.tensor.matmul(
                            out=psz[:, ks], lhsT=w_rel, rhs=agg_sb[:, cs],
                            start=True, stop=False,
                        )
                        nc.tensor.matmul(
                            out=psz[:, ks], lhsT=w_root, rhs=hT_prev[:, cs],
                            start=False, stop=True,
                        )
                    zs = slice(jg * P, (jg + jn) * P)
                    nc.scalar.activation(
                        out=z_sb[:, zs], in_=psz[:, 0 : jn * P],
                        func=mybir.ActivationFunctionType.Copy,
                    )
                    for k in range(jn):
                        j = jg + k
                        width = SHORT if j == 48 else P
                        nc.vector.bn_stats(
                            out=stats[:, j, :],
                            in_=z_sb[:, j * P : j * P + width],
                        )

            for l in range(L):
                z_sb = z_pool.tile([P, SLOTS], F16)
                agg_sb = agg_pool.tile([P, SLOTS], F16)
                stats = stat_pool.tile([P, CHUNKS, nc.vector.BN_STATS_DIM], F32)
                rs_inp = []
                for piece in range(n_pieces):
                    rst = dram_rsi.tile(
                        [N_CORES * P, piece_cols[piece]], F16, tag=f"rsi{piece}"
                    )
                    rs_inp.append(rst)

                gath = sel = None
                grp_lo = 0
                ps4 = None
                rs_outs = []
                gi = 0
                n_grp_p = [len(GGRPS[p]) * N_CORES for p in range(n_pieces)]
                rs_emit_at = {sum(n_grp_p[:1]) + 10: 0}

                def emit_rs(piece_id):
                    ncols = piece_cols[piece_id]
                    rs_out = dram_rso.tile([P, ncols], F16, tag=f"rso{piece_id}")
                    nc.gpsimd.collective_compute(
                        "ReduceScatter",
                        mybir.AluOpType.add,
                        replica_groups=rg,
                        ins=[rs_inp[piece_id][:].opt()],
                        outs=[rs_out[:]],
                    )
                    rs_outs.append(rs_out)

                for pos in range(NBINS):
                    piece, q, j = pos_info[pos]
                    T = t_proc[pos]
                    tb = tile_base[pos]
                    if tb in grp_start:
                        if gi in rs_emit_at:
                            emit_rs(rs_emit_at[gi])
                        gi += 1
                        grp_lo, grp_hi = grp_start[tb]
                        ng = (grp_hi - grp_lo) * P
                        gath = g_pool.tile([P, max_grp_t * P], F16, tag="gath")
                        gv = gath.rearrange("p (t d) -> p t d", t=max_grp_t)
                        nc.gpsimd.dma_gather(
                            out_ap=gv[:, 0 : grp_hi - grp_lo, :],
                            in_ap=h_tab[:],
                            idxs_ap=idx_sb[:, grp_lo * 8 : grp_hi * 8],
                            num_idxs=ng,
                            num_idxs_reg=nidx_regs[ng],
                            elem_size=D,
                            single_packet=False,
                        )
                        sel = build_sel(grp_lo, grp_hi)
                    jj = j - PIECE_J0[piece]
                    npc = PIECES[piece]
                    pg = jj % PGRP
                    pgn = min(PGRP, npc - (jj - pg))
                    if pg == 0:
                        ps4 = psA.tile([P, PGRP * P], F32, space="PSUM")
                    for t in range(T):
                        ft = tb + t - grp_lo
                        nc.tensor.matmul(
                            out=ps4[:, pg * P : (pg + 1) * P],
                            lhsT=gath[:, ft * P : (ft + 1) * P],
                            rhs=sel_tile_ap(sel, grp_lo, pos, t),
                            start=(t == 0),
                            stop=(t == T - 1),
                        )
                    if pg == pgn - 1:
                        stg = stg_pool.tile([P, PGRP * P], F16, tag="stg")
                        nc.scalar.activation(
                            out=stg[:, 0 : pgn * P], in_=ps4[:, 0 : pgn * P],
                            func=mybir.ActivationFunctionType.Copy,
                        )
                        rs_in = rs_inp[piece]
                        pw = piece_cols[piece]
                        col0 = (j - pg) * P - piece_col0[piece]
                        dest = bass.AP(
                            tensor=rs_in.tensor,
                            offset=rs_in[:].offset + q * P * pw + col0,
                            ap=[[pw, P], [1, pgn * P]],
                        )
                        nc.sync.dma_start(out=dest, in_=stg[:, 0 : pgn * P])
                for piece_id in range(1, n_pieces):
                    emit_rs(piece_id)

                # readbacks + dense emitted after the loop: an in-order SP
                # queue must never park a collective-gated DMA ahead of the
                # aggregate staging writes
                for piece_id in range(n_pieces):
                    c0 = piece_col0[piece_id]
                    ncols = piece_cols[piece_id]
                    nc.sync.dma_start(
                        out=agg_sb[:, c0 : c0 + ncols], in_=rs_outs[piece_id][:]
                    )
                for piece in range(n_pieces):
                    emit_dense_piece(piece, l, z_sb, agg_sb, stats)

                # ---- BatchNorm across all nodes (tiny stats AllGather) ----
                bs = bn_pool.tile([P, 16], F32)
                mv = bs[:, 0:2]
                with tc.high_priority():
                    nc.vector.bn_aggr(out=mv, in_=stats[:])
                cc_sb = bs[:, 3:5]
                with tc.high_priority():
                    nc.vector.tensor_copy(out=cc_sb[:, 0:1], in_=mv[:, 0:1])
                    nc.vector.tensor_scalar(
                        out=cc_sb[:, 1:2], in0=mv[:, 0:1], scalar1=mv[:, 0:1],
                        scalar2=mv[:, 1:2], op0=mybir.AluOpType.mult,
                        op1=mybir.AluOpType.add,
                    )
                cc_in = dram_cc.tile([P, 2], F32)
                cc_out = dram_cc.tile([P * N_CORES, 2], F32, addr_space="Shared")
                nc.sync.dma_start(out=cc_in[:], in_=cc_sb)
                nc.gpsimd.collective_compute(
                    "AllGather", mybir.AluOpType.bypass, replica_groups=rg,
                    ins=[cc_in.opt()], outs=[cc_out.opt()],
                )
                cc_all = bn_pool.tile([P, 2, N_CORES], F32)
                cc_src = bass.AP(
                    tensor=cc_out.tensor,
                    offset=cc_out[:].offset,
                    ap=[[2, P], [1, 2], [2 * P, N_CORES]],
                )
                nc.sync.dma_start(out=cc_all[:], in_=cc_src)
                cc_res = bs[:, 5:7]
                nc.vector.tensor_reduce(
                    out=cc_res.rearrange("p (a b) -> p a b", a=2),
                    in_=cc_all[:],
                    axis=mybir.AxisListType.X,
                    op=mybir.AluOpType.add,
                )
                mu = bs[:, 7:8]
                nc.vector.tensor_scalar(
                    out=mu, in0=cc_res[:, 0:1], scalar2=None,
                    op0=mybir.AluOpType.mult, scalar1=1.0 / N_CORES,
                )
                var = bs[:, 8:9]
                nc.vector.tensor_scalar(
                    out=var, in0=cc_res[:, 1:2], scalar2=None,
                    op0=mybir.AluOpType.mult, scalar1=1.0 / N_CORES,
                )
                mu2 = bs[:, 9:10]
                nc.vector.tensor_tensor(
                    out=mu2, in0=mu, in1=mu, op=mybir.AluOpType.mult
                )
                nc.vector.tensor_tensor(
                    out=var, in0=var, in1=mu2, op=mybir.AluOpType.subtract
                )
                rstd = bs[:, 10:11]
                nc.scalar.activation(
                    out=rstd, in_=var,
                    func=mybir.ActivationFunctionType.Sqrt,
                    bias=eps_sb[:], scale=1.0,
                )
                nc.vector.reciprocal(out=rstd, in_=rstd)
                scale = bs[:, 11:12]
                nc.vector.tensor_tensor(
                    out=scale, in0=rstd, in1=gammaT_sb[:, l : l + 1],
                    op=mybir.AluOpType.mult,
                )
                shift = bs[:, 12:13]
                nc.vector.tensor_tensor(
                    out=shift, in0=mu, in1=scale, op=mybir.AluOpType.mult
                )
                nc.vector.tensor_tensor(
                    out=shift, in0=betaT_sb[:, l : l + 1], in1=shift,
                    op=mybir.AluOpType.subtract,
                )

                # BN apply + relu, zero pad slots, rebuild node-major table
                hT_new = hT_pool.tile([P, SLOTS], F16, tag="hT")
                h_tab_new = dram_tab.tile([SLOTS, D], F16, tag="htab")
                for c0 in range(0, CHUNKS, AGRP):
                    ng = min(AGRP, CHUNKS - c0)
                    gs = slice(c0 * P, (c0 + ng) * P)
                    nc.scalar.activation(
                        out=hT_new[:, gs], in_=z_sb[:, gs],
                        func=mybir.ActivationFunctionType.Relu,
                        bias=shift, scale=scale,
                    )
                    if c0 + ng == CHUNKS:
                        nc.vector.memset(hT_new[:, QUOTA:SLOTS], 0.0)
                    t16g = t16_pool.tile([P, AGRP, P], F16)
                    for k0 in range(0, ng, PGRP):
                        kn = min(PGRP, ng - k0)
                        ps_t = psT.tile([P, PGRP * P], F16, space="PSUM")
                        for k in range(k0, k0 + kn):
                            c = c0 + k
                            cs2 = slice(c * P, (c + 1) * P)
                            nc.tensor.transpose(
                                out=ps_t[:, (k - k0) * P : (k - k0 + 1) * P],
                                in_=hT_new[:, cs2], identity=ident_sb[:],
                            )
                        nc.vector.tensor_copy(
                            out=t16g[:, k0 : k0 + kn, :], in_=ps_t[:, 0 : kn * P]
                        )
                    dest = bass.AP(
                        tensor=h_tab_new.tensor,
                        offset=h_tab_new[:].offset + c0 * P * D,
                        ap=[[D, P], [P * D, ng], [1, D]],
                    )
                    nc.sync.dma_start(out=dest, in_=t16g[:, 0:ng, :])
                hT_prev = hT_new
                h_tab = h_tab_new

            # ---------------- final GraphConv (OUT=2) ----------------
            # transpose-mode gather -> gathT [feat, edge]; PE projection
            # through Wrel2 -> proj [edge, 2]; segment-sum emits
            # feature-major partials [2, slots] incl. masked root + bias.
            rs_in_f = dram_rsi.tile([N_PAD, OUT], F16, tag="rsif")
            sel = None
            proj_sb = None
            grp_lo = 0
            psf = None
            for pos in range(NBINS):
                piece, q, j = pos_info[pos]
                T = t_proc[pos]
                tb = tile_base[pos]
                if tb in grp_start:
                    grp_lo, grp_hi = grp_start[tb]
                    ng = (grp_hi - grp_lo) * P
                    nt = grp_hi - grp_lo
                    gathT = g_pool.tile([P, max_grp_t * P], F16, tag="gath")
                    gtb = gathT[:]
                    nc.gpsimd.dma_gather(
                        out_ap=bass.AP(
                            tensor=gtb.tensor,
                            offset=gtb.offset,
                            ap=[gtb.ap[0], [ng, 1], [1, ng]],
                        ),
                        in_ap=h_tab[:],
                        idxs_ap=idx_sb[:, grp_lo * 8 : grp_hi * 8],
                        num_idxs=ng,
                        num_idxs_reg=nidx_regs[ng],
                        elem_size=D,
                        transpose=True,
                        single_packet=False,
                    )
                    psp = psP.tile([P, max_grp_t * OUT], F32, space="PSUM", tag="psp")
                    for t in range(nt):
                        nc.tensor.matmul(
                            out=psp[:, t * OUT : (t + 1) * OUT],
                            lhsT=gathT[:, t * P : (t + 1) * P],
                            rhs=wrel2_sb[:],
                            start=True, stop=True,
                        )
                    proj_sb = stg_pool.tile([P, max_grp_t * OUT], F16, tag="proj")
                    nc.scalar.activation(
                        out=proj_sb[:, 0 : nt * OUT], in_=psp[:, 0 : nt * OUT],
                        func=mybir.ActivationFunctionType.Copy,
                    )
                    sel = build_sel(grp_lo, grp_hi)
                jj = j - PIECE_J0[piece]
                npc = PIECES[piece]
                pg = jj % PGRP
                pgn = min(PGRP, npc - (jj - pg))
                if pg == 0:
                    psf = psF.tile([P, PGRP * OUT], F32, space="PSUM", tag="psf")
                fo = slice(pg * OUT, (pg + 1) * OUT)
                for t in range(T):
                    nc.tensor.matmul(
                        out=psf[:, fo],
                        lhsT=sel_tile_ap(sel, grp_lo, pos, t),
                        rhs=proj_sb[:, (tb + t - grp_lo) * OUT : (tb + t - grp_lo + 1) * OUT],
                        start=(t == 0),
                        stop=False,
                    )
                # root + bias: nonzero only on the rank that owns these slots
                nc.tensor.matmul(
                    out=psf[:, fo],
                    lhsT=hT_prev[:, j * P : (j + 1) * P],
                    rhs=wroot2m_sb[:, q * OUT : (q + 1) * OUT],
                    start=False,
                    stop=False,
                )
                nc.tensor.matmul(
                    out=psf[:, fo],
                    lhsT=ones_sb[:],
                    rhs=b2m_sb[:, q * OUT : (q + 1) * OUT],
                    start=False,
                    stop=True,
                )
                if pg == pgn - 1:
                    stgf = stg_pool.tile([P, PGRP * OUT], F16, tag="stgf")
                    nc.scalar.activation(
                        out=stgf[:, 0 : pgn * OUT], in_=psf[:, 0 : pgn * OUT],
                        func=mybir.ActivationFunctionType.Copy,
                    )
                    r0 = q * SLOTS + (j - pg) * P
                    dest = bass.AP(
                        tensor=rs_in_f.tensor,
                        offset=rs_in_f[:].offset + r0 * OUT,
                        ap=[[OUT, P], [P * OUT, pgn], [1, OUT]],
                    )
                    nc.sync.dma_start(out=dest, in_=stgf[:, 0 : pgn * OUT])

            rs_out_f = dram_rso.tile([SLOTS, OUT], F16, tag="rsof")
            nc.gpsimd.collective_compute(
                "ReduceScatter",
                mybir.AluOpType.add,
                replica_groups=rg,
                ins=[rs_in_f[:].opt()],
                outs=[rs_out_f[:]],
            )
            nc.sync.dma_start(out=p_out[:], in_=rs_out_f[:])

    lower_extended_insts(nc)
    _split_multiwait(nc)
    return nc


_PROGRAM_CACHE = {}


def _get_program(meta):
    key = hashlib.sha1(repr(sorted(meta.items())).encode()).hexdigest()
    if key not in _PROGRAM_CACHE:
        _PROGRAM_CACHE[key] = build_program(meta)
    return _PROGRAM_CACHE[key]


def _make_in_maps(idx_cores, dst_cores, x_loc, xT_loc,
                  Wrel, Wroot, gamma, beta, Wrel2, Wroot2, b2):
    iotar = np.zeros((P, P, RMAX), np.float16)
    iotar[:, :, :] = np.arange(P, dtype=np.float16)[None, :, None]
    ident16 = np.eye(P, dtype=np.float16)
    common = dict(
        wrel=np.ascontiguousarray(np.asarray(Wrel, np.float32)),
        wroot=np.ascontiguousarray(np.asarray(Wroot, np.float32)),
        wrel2=np.ascontiguousarray(np.asarray(Wrel2, np.float32)),
        gammaT=np.ascontiguousarray(np.asarray(gamma, np.float32).T),
        betaT=np.ascontiguousarray(np.asarray(beta, np.float32).T),
        iotar=np.ascontiguousarray(iotar.reshape(P, P * RMAX)),
        ident16=ident16,
    )
    wroot2 = np.asarray(Wroot2, np.float16)                 # [D, OUT]
    b2 = np.asarray(b2, np.float16).reshape(1, OUT)
    in_maps = []
    for c in range(N_CORES):
        w2m = np.zeros((D, N_CORES, OUT), np.float16)
        w2m[:, c, :] = wroot2
        b2m = np.zeros((1, N_CORES, OUT), np.float16)
        b2m[:, c, :] = b2
        m = dict(common)
        m["x_loc"] = x_loc[c]
        m["xT_loc"] = xT_loc[c]
        m["gidx"] = idx_cores[c]
        m["dst_loc"] = dst_cores[c]
        m["wroot2m"] = np.ascontiguousarray(w2m.reshape(D, N_CORES * OUT))
        m["b2m"] = np.ascontiguousarray(b2m.reshape(1, N_CORES * OUT))
        in_maps.append(m)
    return in_maps


def run(x, edge_index, Wrel, Wroot, b, gamma, beta, Wrel2, Wroot2, b2):
    """Returns (output [N, OUT] float32, nc, meta) - nc exposed for profiling.
    The per-layer GraphConv bias b cancels inside BatchNorm and is unused."""
    meta, newid, idx_cores, dst_cores, x_loc, xT_loc = _preprocess(x, edge_index)
    nc = _get_program(meta)
    in_maps = _make_in_maps(
        idx_cores, dst_cores, x_loc, xT_loc,
        Wrel, Wroot, gamma, beta, Wrel2, Wroot2, b2,
    )
    from concourse.bass_utils import run_bass_kernel_spmd

    res = run_bass_kernel_spmd(nc, in_maps, list(range(N_CORES)))
    full = np.concatenate(
        [res.results[c]["z4"] for c in range(N_CORES)], axis=0
    )  # [N_PAD, OUT]
    return full[newid].astype(np.float32), nc, meta


def kernel(**inputs):
    out, _, _ = run(**{k: np.asarray(v) for k, v in inputs.items()})
    return out
